# revision 28
# baseline (speedup 1.0000x reference)
"""Trainium2 Bass kernel for nn_CPCLoss (self-contained).

Strategy (8 NeuronCores, full inputs in / full output out):
  NEFF-A, SPMD on 8 cores — core k = (batch b=k//4, row-block blk=k%4 of 112
  dst rows). Each core reads its cam shard [20, 112, 448] and computes:
    * per-pixel top1/second/argmax over classes -> pseudo-label class map
    * A_partial[c] = Wr_blk^T @ onehot(q==c+1) @ Wc  (28x28 bilinear-downsample
      coefficient grid per class; Wr/Wc are the static jax.image.resize
      bilinear matrices) via PE matmuls
    * exact per-class top-256 (values+indices) over the 50176 shard pixels via
      the gpsimd topk instruction; top-32 shipped as merge candidates
  Host only reshapes/concats partials (no arithmetic).
  NEFF-B, 1 core — sums partials, merges exact top-25 per (b,c), builds the
  bilinear gather matrix G, selects coef = count==0 ? G/25 : A/max(count,1),
  fsm = coef @ fmap^T, then runs the 2-step EMA memory-bank scan and emits the
  scalar loss.
"""
import os
import sys

os.environ.setdefault("MYCRO_LOCAL_CACHE", "1")
if "/opt/trn_rl_repo" not in sys.path:
    sys.path.insert(0, "/opt/trn_rl_repo")

from contextlib import ExitStack

import numpy as np

from concourse import bacc, bass_isa, mybir, tile
from concourse.bass_utils import run_bass_kernel_spmd


class _StageDone(Exception):
    pass

f32 = mybir.dt.float32
u32 = mybir.dt.uint32
ALU = mybir.AluOpType
AFT = mybir.ActivationFunctionType
AX = mybir.AxisListType

B, C, D = 2, 20, 256
H = W = 448
FH = FW = 28
K_TOP = 25
NBLK = 4
RB = H // NBLK            # 112
NPIX = RB * W             # 50176
NCAND = 32                # candidates shipped per (core, class)
MARGIN = 0.3


def _make_w1d():
    scale = FH / H
    w = np.zeros((H, FH), dtype=np.float64)
    for x in range(H):
        s = (x + 0.5) * scale - 0.5
        i0 = int(np.floor(s))
        f = s - i0
        for i, wt in ((i0, 1.0 - f), (i0 + 1, f)):
            if 0 <= i < FH:
                w[x, i] += wt
        w[x] /= w[x].sum()
    return w.astype(np.float32)


W1D = _make_w1d()


def _emit_topk(nc, out_ap, in_ap, tokens):
    g = nc.gpsimd
    return g.add_instruction(bass_isa.InstTopk(
        name=f"I-{nc.next_id()}",
        ins=[g.lower_ap(in_ap, for_isa=True)],
        outs=[g.lower_ap(out_ap, for_isa=True)],
        _tokens=tokens, _n=NPIX, _k=256))


# --------------------------------------------------------------------------
# NEFF-A
# --------------------------------------------------------------------------

def _build_a(hig, low, bg, CP=C):
    nc = bacc.Bacc("TRN2", target_bir_lowering=False, debug=False, num_devices=8)

    camv = nc.dram_tensor("camv", [CP, NPIX], f32, kind="ExternalInput").ap()
    labt = nc.dram_tensor("labt", [RB, CP], f32, kind="ExternalInput").ap()
    clst = nc.dram_tensor("clst", [RB, CP], f32, kind="ExternalInput").ap()
    iodt = nc.dram_tensor("iodt", [RB, CP], f32, kind="ExternalInput").ap()
    wrt = nc.dram_tensor("wrt", [RB, 28], f32, kind="ExternalInput").ap()
    wct = nc.dram_tensor("wct", [RB, 4 * 28], f32, kind="ExternalInput").ap()
    idn = nc.dram_tensor("idn", [128, 128], f32, kind="ExternalInput").ap()

    o_a = nc.dram_tensor("o_a", [28, CP * 28], f32, kind="ExternalOutput").ap()
    ntk = (CP + 7) // 8
    tok = [min(8, CP - 8 * t) for t in range(ntk)]
    o_tk = [nc.dram_tensor(f"o_tk{t}", [16 * tok[t], 32], u32,
                           kind="ExternalOutput").ap() for t in range(ntk)]

    thmax = float(max(hig, low, bg))

    with tile.TileContext(nc) as tc, ExitStack() as ctx:
        pool = ctx.enter_context(tc.tile_pool(name="p", bufs=1))
        psum = ctx.enter_context(tc.tile_pool(name="ps", bufs=1, space="PSUM"))
        nv = nc.vector

        VP = pool.tile([RB, CP * W], f32)
        nc.sync.dma_start(VP[:], camv.rearrange("c (r w) -> r c w", w=W))
        VT = []
        for t in range(ntk):
            vt = pool.tile([16 * tok[t], NPIX // 16], f32, name=f"VT{t}")
            nc.sync.dma_start(vt[:], camv[8 * t:8 * t + tok[t]]
                              .rearrange("c (g f) -> (c g) f", f=NPIX // 16))
            VT.append(vt)

        LB = pool.tile([RB, CP], f32); nc.sync.dma_start(LB[:], labt)
        CL = pool.tile([RB, CP], f32); nc.sync.dma_start(CL[:], clst)
        IO = pool.tile([RB, CP], f32); nc.sync.dma_start(IO[:], iodt)
        WR = pool.tile([RB, 28], f32); nc.sync.dma_start(WR[:], wrt)
        WC = pool.tile([RB, 4 * 28], f32); nc.sync.dma_start(WC[:], wct)
        IDN = pool.tile([128, 128], f32); nc.sync.dma_start(IDN[:], idn)

        # ---- pseudo-label phase ----
        V_cw = VP[:].rearrange("p (c w) -> p c w", w=W)
        V_wc = VP[:].rearrange("p (c w) -> p w c", w=W)
        LB_b = LB[:].unsqueeze(2).broadcast_to([RB, CP, W])
        nv.tensor_tensor(out=V_cw, in0=V_cw, in1=LB_b, op=ALU.mult)  # valid in-place

        T1 = pool.tile([RB, W], f32)
        nv.tensor_reduce(out=T1[:], in_=V_wc, axis=AX.X, op=ALU.max)

        GE = pool.tile([RB, CP * W], f32)
        GE_cw = GE[:].rearrange("p (c w) -> p c w", w=W)
        T1_b = T1[:].unsqueeze(1).broadcast_to([RB, CP, W])
        nv.tensor_tensor(out=GE_cw, in0=V_cw, in1=T1_b, op=ALU.is_ge)

        EN = pool.tile([RB, CP * W], f32, tag="scr")
        EN_cw = EN[:].rearrange("p (c w) -> p c w", w=W)
        IO_b = IO[:].unsqueeze(2).broadcast_to([RB, CP, W])
        nv.tensor_tensor(out=EN_cw, in0=GE_cw, in1=IO_b, op=ALU.mult)
        AM = pool.tile([RB, W], f32)
        nv.tensor_reduce(out=AM[:], in_=EN[:].rearrange("p (c w) -> p w c", w=W),
                         axis=AX.X, op=ALU.max)

        MK = pool.tile([RB, CP * W], f32, tag="scr")
        MK_cw = MK[:].rearrange("p (c w) -> p c w", w=W)
        nv.scalar_tensor_tensor(out=MK_cw, in0=GE_cw, scalar=-1e9, in1=V_cw,
                                op0=ALU.mult, op1=ALU.add)
        SC = pool.tile([RB, W], f32)
        nv.tensor_reduce(out=SC[:], in_=MK[:].rearrange("p (c w) -> p w c", w=W),
                         axis=AX.X, op=ALU.max)

        # keep iff top1 >= max(hig,low,bg) and (margin >= 0.3 or top1 <= hig)
        KG = pool.tile([RB, W], f32)
        nv.tensor_scalar(out=KG[:], in0=T1[:], scalar1=thmax, scalar2=None, op0=ALU.is_ge)
        MGOK = pool.tile([RB, W], f32)
        nv.tensor_tensor(out=MGOK[:], in0=T1[:], in1=SC[:], op=ALU.subtract)
        nv.tensor_scalar(out=MGOK[:], in0=MGOK[:], scalar1=MARGIN, scalar2=None, op0=ALU.is_ge)
        LEH = pool.tile([RB, W], f32)
        nv.tensor_scalar(out=LEH[:], in0=T1[:], scalar1=float(hig), scalar2=None, op0=ALU.is_le)
        nv.tensor_tensor(out=MGOK[:], in0=MGOK[:], in1=LEH[:], op=ALU.max)
        nv.tensor_tensor(out=KG[:], in0=KG[:], in1=MGOK[:], op=ALU.mult)
        Q = pool.tile([RB, W], f32)
        nv.tensor_scalar(out=Q[:], in0=AM[:], scalar1=-1.0, scalar2=float(CP + 1),
                         op0=ALU.mult, op1=ALU.add)
        nv.tensor_tensor(out=Q[:], in0=Q[:], in1=KG[:], op=ALU.mult)

        # ---- q transpose + one-hot EQT + matmuls for A ----
        QT = pool.tile([RB, 4 * RB], f32)
        for u in range(4):
            QTP = psum.tile([RB, RB], f32, tag="qtp")
            nc.tensor.transpose(QTP[:], Q[:, u * RB:(u + 1) * RB], IDN[:RB, :RB])
            nc.scalar.copy(QT[:, u * RB:(u + 1) * RB], QTP[:])

        EQT = pool.tile([RB, 4 * CP * RB], f32)
        for u in range(4):
            sl = EQT[:, u * CP * RB:(u + 1) * CP * RB]
            sl_cw = sl.rearrange("p (c r) -> p c r", r=RB)
            QT_b = QT[:, u * RB:(u + 1) * RB].unsqueeze(1).broadcast_to([RB, CP, RB])
            CL_b = CL[:].unsqueeze(2).broadcast_to([RB, CP, RB])
            nv.tensor_tensor(out=sl_cw, in0=QT_b, in1=CL_b, op=ALU.is_equal)
        # PSUM bank = 512 f32: hold 5 classes (140 cols) per bank-tile
        ngrp = (CP + 4) // 5
        T0sb = pool.tile([RB, CP * 28], f32)
        Asb = pool.tile([28, CP * 28], f32)
        T0ps = [psum.tile([RB, 5 * 28], f32, name=f"t0ps{i}", tag="accps", bufs=4)
                for i in range(ngrp)]
        Aps = [psum.tile([28, 5 * 28], f32, name=f"aps{i}", tag="accps", bufs=4)
               for i in range(ngrp)]
        for c in range(CP):
            grp, off = c // 5, (c % 5) * 28
            for u in range(4):
                nc.tensor.matmul(
                    T0ps[grp][:, off:off + 28],
                    lhsT=EQT[:, u * CP * RB + c * RB:u * CP * RB + (c + 1) * RB],
                    rhs=WC[:, u * 28:(u + 1) * 28],
                    start=(u == 0), stop=(u == 3))
        for i in range(ngrp):
            w0 = i * 140
            w1 = min(w0 + 140, CP * 28)
            nc.scalar.copy(T0sb[:, w0:w1], T0ps[i][:, 0:w1 - w0])
        for c in range(CP):
            grp, off = c // 5, (c % 5) * 28
            nc.tensor.matmul(Aps[grp][:, off:off + 28], lhsT=WR[:],
                             rhs=T0sb[:, c * 28:(c + 1) * 28], start=True, stop=True)
        for i in range(ngrp):
            w0 = i * 140
            w1 = min(w0 + 140, CP * 28)
            nc.scalar.copy(Asb[:, w0:w1], Aps[i][:, 0:w1 - w0])
        nc.sync.dma_start(o_a, Asb[:])

        # ---- per-class topk ----
        for t in range(ntk):
            tkt = pool.tile([16 * tok[t], 32], u32, name=f"TK{t}")
            _emit_topk(nc, tkt[:], VT[t][:], tokens=tok[t])
            nc.sync.dma_start(o_tk[t], tkt[:])

    nc.compile()
    return nc


# --------------------------------------------------------------------------
# NEFF-B
# --------------------------------------------------------------------------

def _build_b(stage=99):
    nc = bacc.Bacc("TRN2", target_bir_lowering=False, debug=False, num_devices=1)
    P = B * C  # 40 (b,c) pairs

    ain = nc.dram_tensor("ain", [P, 784 * NBLK], f32, kind="ExternalInput").ap()
    cdv = nc.dram_tensor("cdv", [P, NBLK * NCAND], f32, kind="ExternalInput").ap()
    cdi = nc.dram_tensor("cdi", [P, NBLK * NCAND], u32, kind="ExternalInput").ap()
    bbs = nc.dram_tensor("bbs", [P, NBLK * NCAND], f32, kind="ExternalInput").ap()
    fmi = nc.dram_tensor("fmi", [112, 7 * B * D], f32, kind="ExternalInput").ap()
    prj = nc.dram_tensor("prj", [128, 2 * C], f32, kind="ExternalInput").ap()
    lab = nc.dram_tensor("lab", [P, 1], f32, kind="ExternalInput").ap()
    lab2 = nc.dram_tensor("lab2", [C, B], f32, kind="ExternalInput").ap()
    fc0 = nc.dram_tensor("fc0", [C, D], f32, kind="ExternalInput").ap()
    eye = nc.dram_tensor("eye", [C, C], f32, kind="ExternalInput").ap()
    i28 = nc.dram_tensor("i28", [128, 28], f32, kind="ExternalInput").ap()
    i128 = nc.dram_tensor("i128", [P, 128], f32, kind="ExternalInput").ap()
    mmb = nc.dram_tensor("mmb", [128, 76], f32, kind="ExternalInput").ap()
    rnk = nc.dram_tensor("rnk", [P, NCAND], f32, kind="ExternalInput").ap()
    idn = nc.dram_tensor("idn", [128, 128], f32, kind="ExternalInput").ap()

    o_loss = nc.dram_tensor("o_loss", [1, 1], f32, kind="ExternalOutput").ap()
    o_dbg = nc.dram_tensor("o_dbg", [128, 1024], f32, kind="ExternalOutput").ap()

    NC128 = NBLK * NCAND  # 128 candidates per pair

    try:
      with tile.TileContext(nc) as tc, ExitStack() as ctx:
        pool = ctx.enter_context(tc.tile_pool(name="p", bufs=1))
        psum = ctx.enter_context(tc.tile_pool(name="ps", bufs=1, space="PSUM"))
        nv = nc.vector
        ns = nc.scalar

        AIN = pool.tile([P, 784 * NBLK], f32); nc.sync.dma_start(AIN[:], ain)
        CV = pool.tile([P, NC128], f32); nc.sync.dma_start(CV[:], cdv)
        CI = pool.tile([P, NC128], u32); nc.sync.dma_start(CI[:], cdi)
        BBS = pool.tile([P, NC128], f32); nc.sync.dma_start(BBS[:], bbs)
        FM = pool.tile([112, 7 * B * D], f32); nc.sync.dma_start(FM[:], fmi)
        PJT = pool.tile([128, 2 * C], f32); nc.sync.dma_start(PJT[:], prj)
        LAB = pool.tile([P, 1], f32); nc.sync.dma_start(LAB[:], lab)
        LAB2 = pool.tile([C, B], f32); nc.sync.dma_start(LAB2[:], lab2)
        FC = pool.tile([C, D], f32); nc.sync.dma_start(FC[:], fc0)
        EYE = pool.tile([C, C], f32); nc.sync.dma_start(EYE[:], eye)
        I28 = pool.tile([128, 28], f32); nc.sync.dma_start(I28[:], i28)
        I128 = pool.tile([P, 128], f32); nc.sync.dma_start(I128[:], i128)
        MMB = pool.tile([128, 76], f32); nc.sync.dma_start(MMB[:], mmb)
        RNK = pool.tile([P, NCAND], f32); nc.sync.dma_start(RNK[:], rnk)
        IDN = pool.tile([128, 128], f32); nc.sync.dma_start(IDN[:], idn)

        # ---- A, counts ----
        A = pool.tile([P, 784], f32)
        nv.tensor_reduce(out=A[:], in_=AIN[:].rearrange("p (s k) -> p s k", k=NBLK),
                         axis=AX.X, op=ALU.add)
        CNT = pool.tile([P, 1], f32)
        nv.tensor_reduce(out=CNT[:], in_=A[:], axis=AX.X, op=ALU.add)
        ISZ = pool.tile([P, 1], u32)
        nv.tensor_scalar(out=ISZ[:], in0=CNT[:], scalar1=0.5, scalar2=None, op0=ALU.is_lt)
        DEN = pool.tile([P, 1], f32)
        nv.tensor_scalar(out=DEN[:], in0=CNT[:], scalar1=1.0, scalar2=None, op0=ALU.max)

        # ---- merge top-32 of 128 candidates ----
        CIF = pool.tile([P, NC128], f32)
        nv.tensor_copy(CIF[:], CI[:])
        nv.tensor_tensor(out=CIF[:], in0=CIF[:], in1=BBS[:], op=ALU.add)
        CVa = pool.tile([P, NC128], f32)
        nv.tensor_copy(CVa[:], CV[:])
        MV = pool.tile([P, NCAND], f32)
        MP = pool.tile([P, NCAND], u32)
        for r in range(4):
            nv.max(out=MV[:, r * 8:(r + 1) * 8], in_=CVa[:])
            nv.max_index(out=MP[:, r * 8:(r + 1) * 8],
                         in_max=MV[:, r * 8:(r + 1) * 8], in_values=CVa[:])
            nv.match_replace(out=CVa[:], in_to_replace=MV[:, r * 8:(r + 1) * 8],
                             in_values=CVa[:], imm_value=-1.0)
        MPF = pool.tile([P, NCAND], f32)
        nv.tensor_copy(MPF[:], MP[:])
        # gather global idx at positions
        EQP = pool.tile([P, NCAND * 128], f32)
        EQP_v = EQP[:].rearrange("p (k q) -> p k q", q=128)
        nv.tensor_tensor(out=EQP_v, in0=MPF[:].unsqueeze(2).broadcast_to([P, NCAND, 128]),
                         in1=I128[:].unsqueeze(1).broadcast_to([P, NCAND, 128]),
                         op=ALU.is_equal)
        nv.tensor_tensor(out=EQP_v, in0=EQP_v,
                         in1=CIF[:].unsqueeze(1).broadcast_to([P, NCAND, 128]), op=ALU.mult)
        GIX = pool.tile([P, NCAND], f32)
        nv.tensor_reduce(out=GIX[:], in_=EQP_v, axis=AX.X, op=ALU.max)

        if stage <= 1:
            DBG = pool.tile([P, 64], f32)
            nv.tensor_copy(DBG[:, 0:32], GIX[:])
            nv.tensor_copy(DBG[:, 32:64], MPF[:])
            nc.sync.dma_start(o_dbg[0:P, 0:64], DBG[:])
        # ---- interpolation coefficients ----
        def ts(dst, src, s1, s2, op0, op1=None):
            nv.tensor_scalar(out=dst, in0=src, scalar1=s1, scalar2=s2, op0=op0,
                             **({"op1": op1} if op1 is not None else {}))

        if stage <= 1:
            OUTZ = pool.tile([1, 1], f32)
            nv.memset(OUTZ[:], 0.0)
            nc.sync.dma_start(o_loss, OUTZ[:])
            raise _StageDone()

        i32 = mybir.dt.int32

        def floor_pos(XX, pfx):
            """floor(x) for x>=0: round-to-nearest (f32->i32->f32 copy) then
            subtract 1 where round went up."""
            RI = pool.tile([P, NCAND], i32, name=f"{pfx}_ri", tag=f"{pfx}_ri")
            nv.tensor_copy(RI[:], XX[:])
            RF = pool.tile([P, NCAND], f32, name=f"{pfx}_rf", tag=f"{pfx}_rf")
            nv.tensor_copy(RF[:], RI[:])
            GT = pool.tile([P, NCAND], f32, name=f"{pfx}_gt", tag=f"{pfx}_gt")
            nv.tensor_tensor(out=GT[:], in0=RF[:], in1=XX[:], op=ALU.is_gt)
            nv.tensor_tensor(out=RF[:], in0=RF[:], in1=GT[:], op=ALU.subtract)
            return RF

        TT = pool.tile([P, NCAND], f32)
        ts(TT[:], GIX[:], 1.0 / 448.0, None, ALU.mult)
        HH = floor_pos(TT, "fh")
        WW = pool.tile([P, NCAND], f32)
        nv.scalar_tensor_tensor(out=WW[:], in0=HH[:], scalar=-448.0, in1=GIX[:],
                                op0=ALU.mult, op1=ALU.add)

        def coeffs(XX, pfx):
            U = pool.tile([P, NCAND], f32, name=f"{pfx}_u", tag=f"{pfx}_u")
            ts(U[:], XX[:], 8.5, 1.0 / 16.0, ALU.add, ALU.mult)
            FL = floor_pos(U, f"{pfx}_flr")
            F = pool.tile([P, NCAND], f32, name=f"{pfx}_f", tag=f"{pfx}_f")
            nv.tensor_tensor(out=F[:], in0=U[:], in1=FL[:], op=ALU.subtract)
            X0 = pool.tile([P, NCAND], f32, name=f"{pfx}_x0", tag=f"{pfx}_x0")
            ts(X0[:], FL[:], 1.0, None, ALU.subtract)
            ts(X0[:], X0[:], 0.0, 27.0, ALU.max, ALU.min)
            X1 = pool.tile([P, NCAND], f32, name=f"{pfx}_x1", tag=f"{pfx}_x1")
            ts(X1[:], FL[:], 0.0, 27.0, ALU.max, ALU.min)
            W1 = F
            W0 = pool.tile([P, NCAND], f32, name=f"{pfx}_w0", tag=f"{pfx}_w0")
            ts(W0[:], F[:], -1.0, 1.0, ALU.mult, ALU.add)
            return X0, X1, W0, W1

        I0, I1, WH0, WH1 = coeffs(HH, "ch")
        J0, J1, WWA, WWB = coeffs(WW, "cw")
        WW0 = pool.tile([P, NCAND], f32)
        nv.tensor_tensor(out=WW0[:], in0=WWA[:], in1=RNK[:], op=ALU.mult)
        WW1 = pool.tile([P, NCAND], f32)
        nv.tensor_tensor(out=WW1[:], in0=WWB[:], in1=RNK[:], op=ALU.mult)

        if stage == 2:
            DBG2 = pool.tile([P, 128], f32)
            for i, t in enumerate([I0, I1, WH0, WH1]):
                nv.tensor_copy(DBG2[:, i * 32:(i + 1) * 32], t[:])
            nc.sync.dma_start(o_dbg[0:P, 0:128], DBG2[:])
        # ---- stage (pair,k)-flatten and G build ----
        STG = pool.tile([P, NCAND * 8], f32)
        STG_v = STG[:].rearrange("p (k a) -> p k a", a=8)
        for idx, arr in enumerate([I0, I1, WH0, WH1, J0, J1, WW0, WW1]):
            nv.tensor_copy(STG_v[:, :, idx:idx + 1], arr[:].unsqueeze(2))

        if stage == 2:
            OUTZ = pool.tile([1, 1], f32)
            nv.memset(OUTZ[:], 0.0)
            nc.sync.dma_start(o_loss, OUTZ[:])
            raise _StageDone()

        FLT = pool.tile([128, 80], f32)
        for g in range(10):
            nc.sync.dma_start(
                FLT[:, g * 8:(g + 1) * 8],
                STG[g * 4:(g + 1) * 4, :].rearrange("p (k a) -> p k a", a=8))

        G = pool.tile([P, 784], f32)
        GpsA = psum.tile([P, 392], f32)
        GpsB = psum.tile([P, 392], f32)
        for g in range(10):
            col = lambda i: FLT[:, g * 8 + i:g * 8 + i + 1]
            EQR0 = pool.tile([128, 28], f32, tag="eqr", bufs=2)
            nv.tensor_scalar(out=EQR0[:], in0=I28[:], scalar1=col(0), scalar2=None,
                             op0=ALU.is_equal)
            RQ = pool.tile([128, 28], f32, tag="rq", bufs=2)
            nv.tensor_scalar(out=RQ[:], in0=EQR0[:], scalar1=col(2), scalar2=None,
                             op0=ALU.mult)
            EQR1 = pool.tile([128, 28], f32, tag="eqr2", bufs=2)
            nv.tensor_scalar(out=EQR1[:], in0=I28[:], scalar1=col(1), scalar2=None,
                             op0=ALU.is_equal)
            nv.scalar_tensor_tensor(out=RQ[:], in0=EQR1[:], scalar=col(3), in1=RQ[:],
                                    op0=ALU.mult, op1=ALU.add)
            EQC0 = pool.tile([128, 28], f32, tag="eqr", bufs=2)
            nv.tensor_scalar(out=EQC0[:], in0=I28[:], scalar1=col(4), scalar2=None,
                             op0=ALU.is_equal)
            CQ = pool.tile([128, 28], f32, tag="cq", bufs=2)
            nv.tensor_scalar(out=CQ[:], in0=EQC0[:], scalar1=col(6), scalar2=None,
                             op0=ALU.mult)
            EQC1 = pool.tile([128, 28], f32, tag="eqr2", bufs=2)
            nv.tensor_scalar(out=EQC1[:], in0=I28[:], scalar1=col(5), scalar2=None,
                             op0=ALU.is_equal)
            nv.scalar_tensor_tensor(out=CQ[:], in0=EQC1[:], scalar=col(7), in1=CQ[:],
                                    op0=ALU.mult, op1=ALU.add)
            RHS = pool.tile([128, 784], f32, tag="rhs", bufs=2)
            nv.tensor_tensor(out=RHS[:].rearrange("p (a b) -> p a b", b=28),
                             in0=RQ[:].unsqueeze(2).broadcast_to([128, 28, 28]),
                             in1=CQ[:].unsqueeze(1).broadcast_to([128, 28, 28]),
                             op=ALU.mult)
            # band-membership lhsT: col j of MMB[:, 36-4g : 76-4g] is
            # one-hot(q//32 == j-4g) -> group g's 4 pairs land on rows 4g..4g+3
            lhsT_g = MMB[:, 36 - 4 * g:76 - 4 * g]
            nc.tensor.matmul(GpsA[:], lhsT=lhsT_g, rhs=RHS[:, 0:392],
                             start=(g == 0), stop=(g == 9))
            nc.tensor.matmul(GpsB[:], lhsT=lhsT_g, rhs=RHS[:, 392:784],
                             start=(g == 0), stop=(g == 9))
        ns.copy(G[:, 0:392], GpsA[:])
        ns.copy(G[:, 392:784], GpsB[:])

        if stage == 3:
            nc.sync.dma_start(o_dbg[0:P, 0:784], G[:])
        if stage == 35:
            nc.sync.dma_start(o_dbg[0:128, 0:80], FLT[:])
        # ---- coef + fsm ----
        if stage in (3, 35):
            OUTZ = pool.tile([1, 1], f32)
            nv.memset(OUTZ[:], 0.0)
            nc.sync.dma_start(o_loss, OUTZ[:])
            raise _StageDone()

        RDEN = pool.tile([P, 1], f32)
        nv.reciprocal(RDEN[:], DEN[:])
        AMN = pool.tile([P, 784], f32)
        nv.tensor_scalar(out=AMN[:], in0=A[:], scalar1=RDEN[:], scalar2=None, op0=ALU.mult)
        COEF = pool.tile([P, 784], f32)
        nv.select(COEF[:], ISZ[:].broadcast_to([P, 784]), G[:], AMN[:])
        nv.tensor_scalar(out=COEF[:], in0=COEF[:], scalar1=LAB[:], scalar2=None, op0=ALU.mult)

        CT = pool.tile([RB, 7 * P], f32)
        for u in range(7):
            TPS = psum.tile([RB, P], f32, tag="tps", bufs=2)
            nc.tensor.transpose(TPS[:], COEF[:, u * RB:(u + 1) * RB], IDN[:P, :P])
            ns.copy(CT[:, u * P:(u + 1) * P], TPS[:])

        FSM = pool.tile([C, B * D], f32)
        for b2 in range(B):
            FSps = psum.tile([C, D], f32, tag="fsps")
            for u in range(7):
                nc.tensor.matmul(FSps[:], lhsT=CT[:, u * P + b2 * C:u * P + (b2 + 1) * C],
                                 rhs=FM[:, u * (B * D) + b2 * D:u * (B * D) + (b2 + 1) * D],
                                 start=(u == 0), stop=(u == 6))
            ns.copy(FSM[:, b2 * D:(b2 + 1) * D], FSps[:])

        if stage == 4:
            nc.sync.dma_start(o_dbg[0:C, 0:B * D], FSM[:])
        # ---- scan ----
        if stage == 4:
            OUTZ = pool.tile([1, 1], f32)
            nv.memset(OUTZ[:], 0.0)
            nc.sync.dma_start(o_loss, OUTZ[:])
            raise _StageDone()

        ONES20 = pool.tile([C, 1], f32)
        nv.memset(ONES20[:], 1.0)
        LC = pool.tile([1, 1], f32); nv.memset(LC[:], 0.0)
        CCF = pool.tile([1, 1], f32); nv.memset(CCF[:], 0.0)
        SCR = pool.tile([C, D], f32, tag="scr")
        SCR2 = pool.tile([C, C], f32, tag="scr2")

        def l2norm_div(dst, src):
            nn2 = pool.tile([C, 1], f32, tag="nn2")
            nv.tensor_tensor(out=SCR[:], in0=src, in1=src, op=ALU.mult)
            nv.tensor_reduce(out=nn2[:], in_=SCR[:], axis=AX.X, op=ALU.add)
            nr = pool.tile([C, 1], f32, tag="nr")
            ns.activation(nr[:], nn2[:], AFT.Sqrt)
            nv.tensor_scalar(out=nr[:], in0=nr[:], scalar1=1e-12, scalar2=None, op0=ALU.max)
            rn = pool.tile([C, 1], f32, tag="rn")
            nv.reciprocal(rn[:], nr[:])
            nv.tensor_scalar(out=dst, in0=src, scalar1=rn[:], scalar2=None, op0=ALU.mult)

        for b2 in range(B):
            FSMb = FSM[:, b2 * D:(b2 + 1) * D]
            presb = LAB2[:, b2:b2 + 1]

            FSMN = pool.tile([C, D], f32, tag="fsmn")
            l2norm_div(FSMN[:], FSMb)
            FCN = pool.tile([C, D], f32, tag="fcn")
            l2norm_div(FCN[:], FC[:])

            # transposes of fsm (raw), fsm_n, fc_n -> [128, C] chunks
            TRS = {}
            for nm, srct in (("fsm", FSMb), ("fsmn", FSMN[:]), ("fcn", FCN[:])):
                dst = pool.tile([128, 2 * C], f32, tag=f"tr_{nm}", name=f"tr_{nm}_{b2}")
                for h2 in range(2):
                    TPS4 = psum.tile([128, C], f32, tag="tps", bufs=2)
                    nc.tensor.transpose(TPS4[:], srct[:, h2 * 128:(h2 + 1) * 128],
                                        IDN[:C, :C])
                    ns.copy(dst[:, h2 * C:(h2 + 1) * C], TPS4[:])
                TRS[nm] = dst

            COSps = psum.tile([C, C], f32, tag="cosps")
            for h2 in range(2):
                nc.tensor.matmul(COSps[:], lhsT=TRS["fsmn"][:, h2 * C:(h2 + 1) * C],
                                 rhs=TRS["fcn"][:, h2 * C:(h2 + 1) * C],
                                 start=(h2 == 0), stop=(h2 == 1))
            COSC = pool.tile([C, C], f32, tag="cosc")
            ns.activation(COSC[:], COSps[:], AFT.Abs)
            nv.tensor_scalar(out=COSC[:], in0=COSC[:], scalar1=1e-5, scalar2=1.0 - 1e-5,
                             op0=ALU.max, op1=ALU.min)
            LGC = pool.tile([C, C], f32, tag="lgc")
            ns.activation(LGC[:], COSC[:], AFT.Ln)
            OM = pool.tile([C, C], f32, tag="om")
            nv.tensor_scalar(out=OM[:], in0=COSC[:], scalar1=-1.0, scalar2=1.0,
                             op0=ALU.mult, op1=ALU.add)
            LOM = pool.tile([C, C], f32, tag="lom")
            ns.activation(LOM[:], OM[:], AFT.Ln)

            IDM = pool.tile([C, C], f32, tag="idm")
            nv.tensor_scalar(out=IDM[:], in0=EYE[:], scalar1=presb, scalar2=None, op0=ALU.mult)
            DIF = pool.tile([C, C], f32, tag="dif")
            nv.tensor_tensor(out=DIF[:], in0=LGC[:], in1=LOM[:], op=ALU.subtract)
            CCFD = pool.tile([C, 1], f32, tag="ccfd")
            nv.tensor_tensor(out=SCR2[:], in0=IDM[:], in1=DIF[:], op=ALU.mult)
            nv.tensor_reduce(out=CCFD[:], in_=SCR2[:], axis=AX.X, op=ALU.add)
            R1 = pool.tile([C, 1], f32, tag="r1")
            nv.tensor_reduce(out=R1[:], in_=LOM[:], axis=AX.X, op=ALU.add)
            nv.tensor_tensor(out=CCFD[:], in0=CCFD[:], in1=R1[:], op=ALU.add)

            COSM = pool.tile([C, C], f32, tag="cosm")
            nv.scalar_tensor_tensor(out=COSM[:], in0=EYE[:], scalar=-1e9, in1=COSC[:],
                                    op0=ALU.mult, op1=ALU.add)
            OFF = pool.tile([C, 1], f32, tag="off")
            nv.tensor_reduce(out=OFF[:], in_=COSM[:], axis=AX.X, op=ALU.max)
            QUAL = pool.tile([C, 1], f32, tag="qual")
            nv.tensor_scalar(out=QUAL[:], in0=OFF[:], scalar1=0.6, scalar2=None, op0=ALU.is_lt)
            nv.tensor_tensor(out=QUAL[:], in0=QUAL[:], in1=presb, op=ALU.mult)

            LOGps = psum.tile([C, C], f32, tag="cosps")
            for h2 in range(2):
                nc.tensor.matmul(LOGps[:], lhsT=TRS["fsm"][:, h2 * C:(h2 + 1) * C],
                                 rhs=PJT[:, h2 * C:(h2 + 1) * C],
                                 start=(h2 == 0), stop=(h2 == 1))
            MX = pool.tile([C, 1], f32, tag="mx")
            nv.tensor_reduce(out=MX[:], in_=LOGps, axis=AX.X, op=ALU.max)
            XT = pool.tile([C, C], f32, tag="xt")
            nv.tensor_scalar(out=XT[:], in0=LOGps, scalar1=MX[:], scalar2=None,
                             op0=ALU.subtract)
            ET = pool.tile([C, C], f32, tag="et")
            ns.activation(ET[:], XT[:], AFT.Exp)
            SM = pool.tile([C, 1], f32, tag="sm")
            nv.tensor_reduce(out=SM[:], in_=ET[:], axis=AX.X, op=ALU.add)
            LGS = pool.tile([C, 1], f32, tag="lgs")
            ns.activation(LGS[:], SM[:], AFT.Ln)
            LGP = pool.tile([C, C], f32, tag="lgp")
            nv.tensor_scalar(out=LGP[:], in0=XT[:], scalar1=LGS[:], scalar2=-100.0,
                             op0=ALU.subtract, op1=ALU.max)
            SME = pool.tile([C, C], f32, tag="sme")
            nv.tensor_tensor(out=SME[:], in0=SM[:].broadcast_to([C, C]), in1=ET[:],
                             op=ALU.subtract)
            LSME = pool.tile([C, C], f32, tag="lsme")
            ns.activation(LSME[:], SME[:], AFT.Ln)
            L1P = pool.tile([C, C], f32, tag="l1p")
            nv.tensor_scalar(out=L1P[:], in0=LSME[:], scalar1=LGS[:], scalar2=-100.0,
                             op0=ALU.subtract, op1=ALU.max)

            DD = pool.tile([C, C], f32, tag="dd")
            nv.tensor_tensor(out=DD[:], in0=LGP[:], in1=L1P[:], op=ALU.subtract)
            DDG = pool.tile([C, 1], f32, tag="ddg")
            nv.tensor_tensor(out=SCR2[:], in0=EYE[:], in1=DD[:], op=ALU.mult)
            nv.tensor_reduce(out=DDG[:], in_=SCR2[:], axis=AX.X, op=ALU.add)
            RSM = pool.tile([C, 1], f32, tag="rsm")
            nv.tensor_reduce(out=RSM[:], in_=L1P[:], axis=AX.X, op=ALU.add)
            TERM = pool.tile([C, 1], f32, tag="term")
            nv.tensor_tensor(out=TERM[:], in0=DDG[:], in1=RSM[:], op=ALU.add)
            nv.tensor_scalar(out=TERM[:], in0=TERM[:], scalar1=-1.0 / C, scalar2=None,
                             op0=ALU.mult)
            CONTR = pool.tile([C, 1], f32, tag="contr")
            nv.tensor_tensor(out=CONTR[:], in0=TERM[:], in1=QUAL[:], op=ALU.mult)

            PR = pool.tile([C, 3], f32, tag="pr")
            nv.tensor_copy(PR[:, 0:1], QUAL[:])
            nv.tensor_copy(PR[:, 1:2], CONTR[:])
            nv.tensor_copy(PR[:, 2:3], CCFD[:])
            REDps = psum.tile([1, 3], f32, tag="redps")
            nc.tensor.matmul(REDps[:], lhsT=ONES20[:], rhs=PR[:], start=True, stop=True)
            RED = pool.tile([1, 3], f32, tag="red")
            ns.copy(RED[:], REDps[:])

            # loss_cls = (loss_cls + S) / max(n, 1)   (divide-by-1 when n==0)
            nv.tensor_tensor(out=LC[:], in0=LC[:], in1=RED[:, 1:2], op=ALU.add)
            NB1 = pool.tile([1, 1], f32, tag="nb1")
            nv.tensor_scalar(out=NB1[:], in0=RED[:, 0:1], scalar1=1.0, scalar2=None,
                             op0=ALU.max)
            RNB = pool.tile([1, 1], f32, tag="rnb")
            nv.reciprocal(RNB[:], NB1[:])
            nv.tensor_scalar(out=LC[:], in0=LC[:], scalar1=RNB[:], scalar2=None,
                             op0=ALU.mult)
            # loss_ccf += -(1/400) * ccf_sum
            nv.scalar_tensor_tensor(out=CCF[:], in0=RED[:, 2:3], scalar=-1.0 / (C * C),
                                    in1=CCF[:], op0=ALU.mult, op1=ALU.add)

            # fc = fc + 0.05 * qual * (fsm - fc)
            DFC = pool.tile([C, D], f32, tag="dfc")
            nv.tensor_tensor(out=DFC[:], in0=FSMb, in1=FC[:], op=ALU.subtract)
            Q05 = pool.tile([C, 1], f32, tag="q05")
            nv.tensor_scalar(out=Q05[:], in0=QUAL[:], scalar1=0.05, scalar2=None,
                             op0=ALU.mult)
            nv.scalar_tensor_tensor(out=FC[:], in0=DFC[:], scalar=Q05[:], in1=FC[:],
                                    op0=ALU.mult, op1=ALU.add)

        OUT = pool.tile([1, 1], f32)
        nv.tensor_tensor(out=OUT[:], in0=LC[:], in1=CCF[:], op=ALU.add)
        nc.sync.dma_start(o_loss, OUT[:])
    except _StageDone:
        pass

    nc.compile()
    return nc


# --------------------------------------------------------------------------
# Fast path (no top-k: valid when every present class has count > 0).
# --------------------------------------------------------------------------

bf16 = mybir.dt.bfloat16
f16 = mybir.dt.float16


def _emit_tree(nc, pool, src, n, width, op, pfx, dt=None, part=None):
    """Binary-tree reduce over n leaves of `width` cols each -> [P, width]."""
    nv = nc.vector
    dt = bf16 if dt is None else dt
    part = RB if part is None else part
    cur = src
    lvl = 0
    while n > 1:
        h = n // 2
        odd = n - 2 * h
        dst = pool.tile([part, h * width], dt, name=f"{pfx}_l{lvl}")
        nv.tensor_tensor(out=dst[:], in0=cur[:, :h * width],
                         in1=cur[:, h * width:2 * h * width], op=op)
        if odd:
            nv.tensor_tensor(out=dst[:, :width], in0=dst[:, :width],
                             in1=cur[:, 2 * h * width:(2 * h + 1) * width], op=op)
        cur, n, lvl = dst, h, lvl + 1
    return cur


def _build_a_fast(hig, low, bg, CP):
    nc = bacc.Bacc("TRN2", target_bir_lowering=False, debug=False, num_devices=8)

    camv = nc.dram_tensor("camv", [CP, NPIX], bf16, kind="ExternalInput").ap()
    wrt = nc.dram_tensor("wrt", [RB, 28], bf16, kind="ExternalInput").ap()
    wct = nc.dram_tensor("wct", [RB, 4 * 28], f16, kind="ExternalInput").ap()
    o_a = nc.dram_tensor("o_a", [28, CP * 28], f32, kind="ExternalOutput").ap()

    thmax = float(max(hig, low, bg))
    # class groups of <=4 (PSUM bank = 512 f32 = 4 classes x 4 u x 28)
    grps = []
    c0 = 0
    while c0 < CP:
        n = min(4, CP - c0)
        grps.append((c0, n))
        c0 += n
    ch = (CP + 1) // 2  # class-split DMA halves

    with tile.TileContext(nc) as tc, ExitStack() as ctx:
        pool = ctx.enter_context(tc.tile_pool(name="p", bufs=1))
        psum = ctx.enter_context(tc.tile_pool(name="ps", bufs=1, space="PSUM"))
        nv = nc.vector
        ns = nc.scalar

        VP = pool.tile([RB, CP * W], bf16)
        # class-quarters so partial max trees overlap the later DMA chunks
        qs = []
        q0 = 0
        while q0 < CP:
            qn = min(max(1, (CP + 3) // 4), CP - q0)
            qs.append((q0, qn))
            q0 += qn
        for (q0_, qn_) in qs:
            nc.sync.dma_start(VP[:, q0_ * W:(q0_ + qn_) * W],
                              camv[q0_:q0_ + qn_].rearrange("c (r w) -> r c w",
                                                            w=W))
        WR = pool.tile([RB, 28], bf16)
        nc.sync.dma_start(WR[:], wrt)
        WC = pool.tile([RB, 4 * 28], f16)
        nc.sync.dma_start(WC[:], wct)

        # ---- per-pixel keep-gate (bf16, w innermost so TTs hit 2x mode) ----
        parts = [_emit_tree(nc, pool, VP[:, a * W:(a + n) * W], n, W, ALU.max,
                            f"t1q{i}") for i, (a, n) in enumerate(qs)]
        while len(parts) > 1:
            nxt = []
            for i in range(0, len(parts) - 1, 2):
                t = pool.tile([RB, W], bf16, name=f"t1m{len(parts)}_{i}")
                nv.tensor_tensor(out=t[:], in0=parts[i][:], in1=parts[i + 1][:],
                                 op=ALU.max)
                nxt.append(t)
            if len(parts) % 2:
                nxt.append(parts[-1])
            parts = nxt
        T1 = parts[0]
        T13 = pool.tile([RB, W], bf16)
        nv.tensor_scalar(out=T13[:], in0=T1[:], scalar1=-MARGIN, scalar2=None,
                         op0=ALU.add)
        NG = pool.tile([RB, CP * W], bf16)
        NG_cw = NG[:].rearrange("p (c w) -> p c w", w=W)
        V_cw = VP[:].rearrange("p (c w) -> p c w", w=W)
        nv.tensor_tensor(out=NG_cw, in0=V_cw,
                         in1=T13[:].unsqueeze(1).broadcast_to([RB, CP, W]),
                         op=ALU.is_gt)
        NGS = _emit_tree(nc, pool, NG, CP, W, ALU.add, "ngs")

        # keep iff t1 >= thmax and (exactly one class above t1-0.3 or t1 <= hig)
        LEH = pool.tile([RB, W], bf16)
        nv.tensor_scalar(out=LEH[:], in0=T1[:], scalar1=float(hig),
                         scalar2=None, op0=ALU.is_le)
        K1 = pool.tile([RB, W], bf16)
        nv.tensor_scalar(out=K1[:], in0=T1[:], scalar1=thmax,
                         scalar2=None, op0=ALU.is_ge)
        MOK = pool.tile([RB, W], bf16)
        nv.tensor_scalar(out=MOK[:], in0=NGS[:], scalar1=1.5, scalar2=None,
                         op0=ALU.is_lt)
        nv.tensor_tensor(out=MOK[:], in0=MOK[:], in1=LEH[:], op=ALU.max)
        KEEP = pool.tile([RB, W], bf16)
        nv.tensor_tensor(out=KEEP[:], in0=K1[:], in1=MOK[:], op=ALU.mult)
        # threshold map: t1 where kept else 2.0 (cam < 1, so M == 0 there).
        # Kept pixels have margin >= 0.3 -> no tie at the max -> M is one-hot.
        # KEEP is exactly 0/1 so this select-by-arithmetic is exact in bf16.
        T1K = pool.tile([RB, W], bf16)
        nv.tensor_tensor(out=T1K[:], in0=T1[:], in1=KEEP[:], op=ALU.mult)
        NK2 = pool.tile([RB, W], bf16)
        nv.tensor_scalar(out=NK2[:], in0=KEEP[:], scalar1=-2.0,
                         scalar2=2.0, op0=ALU.mult, op1=ALU.add)
        T1X = pool.tile([RB, W], bf16)
        nv.tensor_tensor(out=T1X[:], in0=T1K[:], in1=NK2[:], op=ALU.add)

        # ---- M chunks + PE bilinear downsample (exact: weights are k/32) ----
        M = pool.tile([RB, CP * W], bf16)
        M_cw = M[:].rearrange("p (c w) -> p c w", w=W)
        Yps = [psum.tile([RB, n * 4 * 28], f32, name=f"yps{g}")
               for g, (c0, n) in enumerate(grps)]
        Asb = pool.tile([28, CP * 28], f32)
        Ysb = [pool.tile([RB, n * 4 * 28], f16, name=f"ysb{g}")
               for g, (c0, n) in enumerate(grps)]
        Aps = psum.tile([28, CP * 28], f32)
        # stage 1 groups back-to-back on PE; copies trail on Act/DVE; then
        # stage 2 groups (so PE never waits a copy mid-stream)
        for g, (c0, n) in enumerate(grps):
            T1X_b = T1X[:].unsqueeze(1).broadcast_to([RB, n, W])
            nv.tensor_tensor(out=M_cw[:, c0:c0 + n, :],
                             in0=V_cw[:, c0:c0 + n, :], in1=T1X_b,
                             op=ALU.is_ge)
            for cr in range(n):
                c = c0 + cr
                for u in range(4):
                    nc.tensor.matmul(
                        Yps[g][:, (cr * 4 + u) * 28:(cr * 4 + u + 1) * 28],
                        lhsT=M[:, c * W + u * RB:c * W + (u + 1) * RB],
                        rhs=WR[:], start=True, stop=True)
            if g % 2 == 0:
                ns.copy(Ysb[g][:], Yps[g][:])
            else:
                nv.tensor_copy(Ysb[g][:], Yps[g][:])
        for g, (c0, n) in enumerate(grps):
            for cr in range(n):
                c = c0 + cr
                for u in range(4):
                    nc.tensor.matmul(
                        Aps[:, c * 28:(c + 1) * 28],
                        lhsT=Ysb[g][:, (cr * 4 + u) * 28:(cr * 4 + u + 1) * 28],
                        rhs=WC[:, u * 28:(u + 1) * 28],
                        start=(u == 0), stop=(u == 3))
            # ship this group's A block as soon as its stage-2 finishes
            if g % 2 == 0:
                ns.copy(Asb[:, c0 * 28:(c0 + n) * 28],
                        Aps[:, c0 * 28:(c0 + n) * 28])
            else:
                nv.tensor_copy(Asb[:, c0 * 28:(c0 + n) * 28],
                               Aps[:, c0 * 28:(c0 + n) * 28])
            nc.sync.dma_start(o_a[:, c0 * 28:(c0 + n) * 28],
                              Asb[:, c0 * 28:(c0 + n) * 28])

    nc.compile()
    return nc


def _build_b_fast():
    nc = bacc.Bacc("TRN2", target_bir_lowering=False, debug=False, num_devices=1)
    P = B * C  # 40

    # aint layout: [pix%112, k*280 + u*40 + pair]  (A^T partials, block-major)
    aint = nc.dram_tensor("aint", [112, NBLK * 7 * P], f32,
                          kind="ExternalInput").ap()
    fmi = nc.dram_tensor("fmi", [112, 7 * B * D], f32, kind="ExternalInput").ap()
    smt = nc.dram_tensor("smt", [128, 226], f32, kind="ExternalInput").ap()

    o_loss = nc.dram_tensor("o_loss", [1, 1], f32, kind="ExternalOutput").ap()
    o_cnt = nc.dram_tensor("o_cnt", [1, P], f32, kind="ExternalOutput").ap()

    L5 = float(np.log(1e-5))
    L1M = float(np.log1p(-1e-5))
    LNLO = float(np.log(1e-5))
    LNHI = float(np.log1p(-1e-5))

    with tile.TileContext(nc) as tc, ExitStack() as ctx:
        pool = ctx.enter_context(tc.tile_pool(name="p", bufs=1))
        psum = ctx.enter_context(tc.tile_pool(name="ps", bufs=1, space="PSUM"))
        nv = nc.vector
        ns = nc.scalar

        AIN = pool.tile([112, NBLK * 7 * P], f32)
        nc.sync.dma_start(AIN[:], aint)
        SM = pool.tile([128, 226], f32)
        nc.sync.dma_start(SM[:], smt)
        FM = pool.tile([112, 7 * B * D], f32)
        for fc in range(4):
            c0, c1 = fc * 1024, min((fc + 1) * 1024, 7 * B * D)
            nc.sync.dma_start(FM[:, c0:c1], fmi[:, c0:c1])
        PJT = SM[:, 0:40]            # [128, (dc,c2)] proj^T chunks
        ONES112 = SM[0:112, 40:41]
        EYE = SM[0:C, 41:61]
        LAB2 = SM[0:C, 61:63]
        BSEL = SM[0:P, 63:65]
        EYEBC = SM[0:P, 65:85]
        ONES20 = SM[0:C, 85:86]
        SH0 = SM[0:C, 86:126]
        SH1 = SM[0:C, 126:166]
        ONESM = SM[0:C, 166:186]
        ONES1R = SM[0:1, 186:226]    # [1, 40] ones

        # ---- early independent: n_b, 1/max(n_b,1), step-0 ccf constant ----
        NSps = psum.tile([1, 2], f32, name="nsps")
        nc.tensor.matmul(NSps[:], lhsT=ONES20, rhs=LAB2, start=True, stop=True)
        NS = pool.tile([1, 2], f32)
        nv.tensor_copy(NS[:], NSps[:])
        DN = pool.tile([1, 2], f32)
        nv.tensor_scalar(out=DN[:], in0=NS[:], scalar1=1.0, scalar2=None,
                         op0=ALU.max)
        RDN = pool.tile([1, 2], f32)
        nv.reciprocal(RDN[:], DN[:])
        CCF = pool.tile([1, 1], f32)
        nv.tensor_scalar(out=CCF[:], in0=NS[:, 0:1],
                         scalar1=-(L5 - L1M) / (C * C), scalar2=-L1M,
                         op0=ALU.mult, op1=ALU.add)

        # ---- raw coef^T = sum over 4 row-blocks (tree); scale LR rides later --
        H1 = pool.tile([112, 2 * 7 * P], f32)
        nv.tensor_tensor(out=H1[:], in0=AIN[:, :2 * 7 * P],
                         in1=AIN[:, 2 * 7 * P:], op=ALU.add)
        CTR = pool.tile([112, 7 * P], f32)
        nv.tensor_tensor(out=CTR[:], in0=H1[:, :7 * P], in1=H1[:, 7 * P:],
                         op=ALU.add)

        # ---- counts -> LR = label/max(cnt,1) as a [40,1] column via PE ----
        CNTps = psum.tile([1, 7 * P], f32, name="cntps")
        nc.tensor.matmul(CNTps[:], lhsT=ONES112, rhs=CTR[:], start=True,
                         stop=True)
        CNTR = pool.tile([1, 7 * P], f32)
        nv.tensor_copy(CNTR[:], CNTps[:])
        CNT = _emit_tree(nc, pool, CNTR, 7, P, ALU.add, "cnt", dt=f32, part=1)
        nc.sync.dma_start(o_cnt, CNT[:])
        DENR = pool.tile([1, P], f32)
        nv.tensor_scalar(out=DENR[:], in0=CNT[:], scalar1=1.0, scalar2=None,
                         op0=ALU.max)
        RDR = pool.tile([1, P], f32)
        nv.reciprocal(RDR[:], DENR[:])
        LRps = psum.tile([P, P], f32, name="lrps")
        nc.tensor.matmul(LRps[:], lhsT=RDR[:], rhs=ONES1R, start=True, stop=True)
        LR40 = pool.tile([P, 1], f32)
        nv.tensor_copy(LR40[:], LRps[:, 0:1])

        # ---- fsm^T (raw scale) = fmap_ds^T @ coef_raw^T ----
        FTps = [psum.tile([128, P], f32, name=f"ftps{dc}", tag="ftps", bufs=2)
                for dc in range(2)]
        for dc in range(2):
            for b2 in range(B):
                for u in range(7):
                    nc.tensor.matmul(
                        FTps[dc][:, b2 * C:(b2 + 1) * C],
                        lhsT=FM[:, u * (B * D) + b2 * D + dc * 128:
                                u * (B * D) + b2 * D + (dc + 1) * 128],
                        rhs=CTR[:, u * P + b2 * C:u * P + (b2 + 1) * C],
                        start=(u == 0), stop=(u == 6))
        FSMT = pool.tile([128, 2 * P], f32)   # [d, (dc, b, c)]
        nv.tensor_copy(FSMT[:, 0:P], FTps[0][:])
        nv.tensor_copy(FSMT[:, P:2 * P], FTps[1][:])

        # ---- Gram diagonals (raw norms) + raw cos dot + logits ----
        SMLps = psum.tile([C, 4 * C], f32, name="smlps")
        for b2 in range(B):
            for dc in range(2):
                nc.tensor.matmul(
                    SMLps[:, b2 * C:(b2 + 1) * C],
                    lhsT=FSMT[:, dc * P + b2 * C:dc * P + (b2 + 1) * C],
                    rhs=FSMT[:, dc * P + b2 * C:dc * P + (b2 + 1) * C],
                    start=(dc == 0), stop=(dc == 1))
        RAWps = SMLps[:, 2 * C:3 * C]
        for dc in range(2):
            nc.tensor.matmul(RAWps,
                             lhsT=FSMT[:, dc * P + C:(dc + 1) * P],
                             rhs=FSMT[:, dc * P:dc * P + C],
                             start=(dc == 0), stop=(dc == 1))
        MMps = psum.tile([P, C], f32, name="mmps")
        LOGps = MMps[:, 0:C]
        for dc in range(2):
            nc.tensor.matmul(LOGps, lhsT=FSMT[:, dc * P:(dc + 1) * P],
                             rhs=PJT[:, dc * C:(dc + 1) * C],
                             start=(dc == 0), stop=(dc == 1))

        SCRD = pool.tile([C, C], f32)
        NRM2B = pool.tile([C, 2], f32)
        for b2 in range(B):
            nv.tensor_tensor(out=SCRD[:], in0=SMLps[:, b2 * C:(b2 + 1) * C],
                             in1=EYE, op=ALU.mult)
            nv.tensor_reduce(out=NRM2B[:, b2:b2 + 1], in_=SCRD[:], axis=AX.X,
                             op=ALU.add)
        NRM2G = pool.tile([C, 2], f32)
        nv.tensor_scalar(out=NRM2G[:], in0=NRM2B[:], scalar1=1e-24, scalar2=None,
                         op0=ALU.max)
        LN2 = pool.tile([C, 2], f32)
        ns.activation(LN2[:], NRM2G[:], AFT.Ln)

        # ---- cos in log space: ln|dot| - ln||a|| - ln||b|| (+absent -> -50) --
        hp = ExitStack()
        hp.enter_context(tc.high_priority())
        ABSR = pool.tile([C, C], f32)
        nv.tensor_scalar(out=ABSR[:], in0=RAWps, scalar1=-1.0, scalar2=None,
                         op0=ALU.mult)
        nv.tensor_tensor(out=ABSR[:], in0=ABSR[:], in1=RAWps, op=ALU.max)
        nv.tensor_scalar(out=ABSR[:], in0=ABSR[:], scalar1=1e-30, scalar2=None,
                         op0=ALU.max)
        LNC = pool.tile([C, C], f32)
        ns.activation(LNC[:], ABSR[:], AFT.Ln)
        # row term: -0.5*ln n1_c ; column term via PE: -0.5*ln n0_j - 50*(1-p0_j)
        COLV = pool.tile([C, 1], f32)
        nv.tensor_scalar(out=COLV[:], in0=LAB2[:, 0:1], scalar1=50.0,
                         scalar2=-50.0, op0=ALU.mult, op1=ALU.add)
        nv.scalar_tensor_tensor(out=COLV[:], in0=LN2[:, 0:1], scalar=-0.5,
                                in1=COLV[:], op0=ALU.mult, op1=ALU.add)
        DIAGC = pool.tile([C, C], f32)
        nv.tensor_scalar(out=DIAGC[:], in0=EYE, scalar1=COLV[:], scalar2=None,
                         op0=ALU.mult)
        CSMps = SMLps[:, 3 * C:4 * C]
        nc.tensor.matmul(CSMps, lhsT=ONESM, rhs=DIAGC[:], start=True, stop=True)
        RV = pool.tile([C, 1], f32)
        nv.tensor_scalar(out=RV[:], in0=LN2[:, 1:2], scalar1=-0.5, scalar2=None,
                         op0=ALU.mult)
        nv.tensor_scalar(out=LNC[:], in0=LNC[:], scalar1=RV[:], scalar2=None,
                         op0=ALU.add)
        nv.tensor_tensor(out=LNC[:], in0=LNC[:], in1=CSMps, op=ALU.add)
        nv.tensor_scalar(out=LNC[:], in0=LNC[:], scalar1=LNLO, scalar2=LNHI,
                         op0=ALU.max, op1=ALU.min)
        COSC = pool.tile([C, C], f32)
        ns.activation(COSC[:], LNC[:], AFT.Exp)
        hp.close()

        # ---- softmax-BCE per (b,c) row (logits get the true LR scale) ----
        LOG = pool.tile([P, C], f32)
        nv.tensor_scalar(out=LOG[:], in0=LOGps, scalar1=LR40[:], scalar2=None,
                         op0=ALU.mult)
        MX = pool.tile([P, 1], f32)
        nv.tensor_reduce(out=MX[:], in_=LOG[:], axis=AX.X, op=ALU.max)
        XT = pool.tile([P, C], f32)
        nv.tensor_scalar(out=XT[:], in0=LOG[:], scalar1=MX[:], scalar2=None,
                         op0=ALU.subtract)
        ET = pool.tile([P, C], f32)
        SMR = pool.tile([P, 1], f32)
        ns.activation(ET[:], XT[:], AFT.Exp, accum_out=SMR[:])
        LGS = pool.tile([P, 1], f32)
        ns.activation(LGS[:], SMR[:], AFT.Ln)
        LGP = pool.tile([P, C], f32)
        nv.tensor_scalar(out=LGP[:], in0=XT[:], scalar1=LGS[:], scalar2=-100.0,
                         op0=ALU.subtract, op1=ALU.max)
        SME = pool.tile([P, C], f32)
        nv.scalar_tensor_tensor(out=SME[:], in0=ET[:], scalar=-1.0,
                                in1=SMR[:].broadcast_to([P, C]),
                                op0=ALU.mult, op1=ALU.add)
        LSME = pool.tile([P, C], f32)
        ns.activation(LSME[:], SME[:], AFT.Ln)
        L1P = pool.tile([P, C], f32)
        nv.tensor_scalar(out=L1P[:], in0=LSME[:], scalar1=LGS[:], scalar2=-100.0,
                         op0=ALU.subtract, op1=ALU.max)
        DD = pool.tile([P, C], f32)
        nv.tensor_tensor(out=DD[:], in0=LGP[:], in1=L1P[:], op=ALU.subtract)
        SCRP = pool.tile([P, C], f32)
        nv.tensor_tensor(out=SCRP[:], in0=DD[:], in1=EYEBC, op=ALU.mult)
        DDG = pool.tile([P, 1], f32)
        nv.tensor_reduce(out=DDG[:], in_=SCRP[:], axis=AX.X, op=ALU.add)
        RSM = pool.tile([P, 1], f32)
        nv.tensor_reduce(out=RSM[:], in_=L1P[:], axis=AX.X, op=ALU.add)
        TERM = pool.tile([P, 1], f32)
        nv.tensor_tensor(out=TERM[:], in0=DDG[:], in1=RSM[:], op=ALU.add)
        nv.tensor_scalar(out=TERM[:], in0=TERM[:], scalar1=-1.0 / C, scalar2=None,
                         op0=ALU.mult)

        # ---- qualify: b0 = present_0; b1 = present_1 & offdiag max < 0.6 ----
        COSM = pool.tile([C, C], f32)
        nv.scalar_tensor_tensor(out=COSM[:], in0=EYE, scalar=-1e9, in1=COSC[:],
                                op0=ALU.mult, op1=ALU.add)
        OFF = pool.tile([C, 1], f32)
        nv.tensor_reduce(out=OFF[:], in_=COSM[:], axis=AX.X, op=ALU.max)
        QB1 = pool.tile([C, 1], f32)
        nv.tensor_scalar(out=QB1[:], in0=OFF[:], scalar1=0.6, scalar2=None,
                         op0=ALU.is_lt)
        nv.tensor_tensor(out=QB1[:], in0=QB1[:], in1=LAB2[:, 1:2], op=ALU.mult)
        QRps = psum.tile([P, 6], f32, name="qrps")
        Q40ps = QRps[:, 0:1]
        nc.tensor.matmul(Q40ps, lhsT=SH0, rhs=LAB2[:, 0:1], start=True,
                         stop=False)
        nc.tensor.matmul(Q40ps, lhsT=SH1, rhs=QB1[:], start=False, stop=True)
        CONTR = pool.tile([P, 1], f32)
        nv.tensor_tensor(out=CONTR[:], in0=TERM[:], in1=Q40ps, op=ALU.mult)

        # ---- ccf step 1 (LGC == clipped LNC already) ----
        OM = pool.tile([C, C], f32)
        nv.tensor_scalar(out=OM[:], in0=COSC[:], scalar1=-1.0, scalar2=1.0,
                         op0=ALU.mult, op1=ALU.add)
        LOM = pool.tile([C, C], f32)
        R1 = pool.tile([C, 1], f32)
        ns.activation(LOM[:], OM[:], AFT.Ln, accum_out=R1[:])
        DIF = pool.tile([C, C], f32)
        nv.tensor_tensor(out=DIF[:], in0=LNC[:], in1=LOM[:], op=ALU.subtract)
        nv.tensor_scalar(out=DIF[:], in0=DIF[:], scalar1=LAB2[:, 1:2],
                         scalar2=None, op0=ALU.mult)
        CC1 = pool.tile([C, 1], f32)
        nv.tensor_tensor(out=SCRD[:], in0=DIF[:], in1=EYE, op=ALU.mult)
        nv.tensor_reduce(out=CC1[:], in_=SCRD[:], axis=AX.X, op=ALU.add)
        nv.tensor_tensor(out=CC1[:], in0=CC1[:], in1=R1[:], op=ALU.add)

        # ---- partition sums via PE, final scalar chain ----
        REDps = QRps[0:1, 1:6]
        nc.tensor.matmul(REDps[:, 2:4], lhsT=CONTR[:], rhs=BSEL, start=True,
                         stop=True)
        nc.tensor.matmul(REDps[:, 4:5], lhsT=ONES20, rhs=CC1[:], start=True,
                         stop=True)
        RED = pool.tile([1, 5], f32)
        nv.tensor_copy(RED[:, 2:5], REDps[:, 2:5])
        LC = pool.tile([1, 1], f32)
        nv.tensor_tensor(out=LC[:], in0=RED[:, 2:3], in1=RDN[:, 0:1], op=ALU.mult)
        nv.tensor_tensor(out=LC[:], in0=LC[:], in1=RED[:, 3:4], op=ALU.add)
        nv.tensor_tensor(out=LC[:], in0=LC[:], in1=RDN[:, 1:2], op=ALU.mult)
        CC1S = pool.tile([1, 1], f32)
        nv.tensor_scalar(out=CC1S[:], in0=RED[:, 4:5], scalar1=-1.0 / (C * C),
                         scalar2=None, op0=ALU.mult)
        OUT = pool.tile([1, 1], f32)
        nv.tensor_tensor(out=OUT[:], in0=LC[:], in1=CCF[:], op=ALU.add)
        nv.tensor_tensor(out=OUT[:], in0=OUT[:], in1=CC1S[:], op=ALU.add)
        nc.sync.dma_start(o_loss, OUT[:])

    nc.compile()
    return nc


def _marshal_a_fast(cam, CP, idxs):
    bf = mybir.dt.np(bf16)
    fh = mybir.dt.np(f16)
    in_maps = []
    for core in range(8):
        b, blk = core // NBLK, core % NBLK
        idx = idxs[b]
        camv = np.zeros((CP, NPIX), bf)
        if len(idx):
            camv[:len(idx)] = cam[b, idx, blk * RB:(blk + 1) * RB, :].reshape(
                len(idx), NPIX).astype(bf)
        wct = np.ascontiguousarray(
            W1D.reshape(4, RB, 28).transpose(1, 0, 2).reshape(RB, 4 * 28))
        in_maps.append({
            "camv": camv,
            "wrt": np.ascontiguousarray(W1D[blk * RB:(blk + 1) * RB, :]).astype(bf),
            "wct": wct.astype(fh),
        })
    return in_maps


def _marshal_b_fast(res_a, fmap, cls_label, proj_weight, CP, idxs):
    P = B * C
    a8 = np.stack([res_a[k]["o_a"] for k in range(8)])          # [8, 28, CP*28]
    a8 = a8.reshape(B, NBLK, 28, CP, 28)
    afull = np.zeros((P, NBLK, 784), np.float32)
    for b in range(B):
        idx = idxs[b]
        if len(idx):
            # [blk, 28i, slot, 28j] -> [slot, blk, (i,j)]
            afull[b * C + idx] = a8[b, :, :, :len(idx), :].transpose(
                2, 0, 1, 3).reshape(len(idx), NBLK, 784)
    # aint[p, k*280 + u*40 + pair] = afull[pair, k, u*112 + p]
    aint = np.ascontiguousarray(
        afull.reshape(P, NBLK, 7, 112).transpose(3, 1, 2, 0)
    ).reshape(112, NBLK * 7 * P)

    fm = np.asarray(fmap, np.float32).reshape(B, D, 784)
    # fmi[p, u*512 + b*256 + d] = fmap[b, d, u*112 + p]
    fmi = np.ascontiguousarray(
        fm.transpose(2, 0, 1).reshape(7, 112, B, D).transpose(1, 0, 2, 3)
    ).reshape(112, 7 * B * D)

    lab = np.asarray(cls_label, np.float32)
    smt = np.zeros((128, 226), np.float32)
    smt[:, 0:40] = np.ascontiguousarray(
        np.asarray(proj_weight, np.float32).T
    ).reshape(2, 128, C).transpose(1, 0, 2).reshape(128, 2 * C)
    smt[0:112, 40:41] = 1.0
    smt[0:C, 41:61] = np.eye(C, dtype=np.float32)
    smt[0:C, 61:63] = lab.T
    smt[0:P, 63:65] = (np.arange(P)[:, None] // C ==
                       np.arange(2)[None, :]).astype(np.float32)
    smt[0:P, 65:85] = np.tile(np.eye(C, dtype=np.float32), (B, 1))
    smt[0:C, 85:86] = 1.0
    smt[0:C, 86:126] = np.eye(C, P, dtype=np.float32)
    smt[0:C, 126:166] = np.eye(C, P, k=C, dtype=np.float32)
    smt[0:C, 166:186] = 1.0
    smt[0:1, 186:226] = 1.0
    return {"aint": aint, "fmi": fmi, "smt": smt}


# --------------------------------------------------------------------------
# Host marshaling + driver
# --------------------------------------------------------------------------

_CACHE = {}


def _get_programs(hig, low, bg, CP):
    stage = int(os.environ.get("BASSK_B_STAGE", "99"))
    key = ("slow", float(hig), float(low), float(bg), stage, CP)
    if key not in _CACHE:
        _CACHE[key] = (_build_a(hig, low, bg, CP), _build_b(stage))
    return _CACHE[key]


def _get_programs_fast(hig, low, bg, CP):
    key = (float(hig), float(low), float(bg), CP)
    if key not in _CACHE:
        _CACHE[key] = (_build_a_fast(hig, low, bg, CP), _build_b_fast())
    return _CACHE[key]


def _marshal_a(cam, cls_label, CP, idxs):
    eye128 = np.eye(128, dtype=np.float32)
    clst = np.tile((np.arange(CP, dtype=np.float32) + 1.0)[None, :], (RB, 1))
    iodt = np.tile((float(CP) - np.arange(CP, dtype=np.float32))[None, :], (RB, 1))
    wct = np.ascontiguousarray(
        W1D.reshape(4, RB, 28).transpose(1, 0, 2).reshape(RB, 4 * 28))
    in_maps = []
    for core in range(8):
        b, blk = core // NBLK, core % NBLK
        idx = idxs[b]
        camv = np.zeros((CP, NPIX), np.float32)
        if len(idx):
            camv[:len(idx)] = cam[b, idx, blk * RB:(blk + 1) * RB, :].reshape(
                len(idx), NPIX)
        labt = np.tile((np.arange(CP) < len(idx)).astype(np.float32)[None, :],
                       (RB, 1))
        in_maps.append({
            "camv": camv,
            "labt": labt,
            "clst": clst,
            "iodt": iodt,
            "wrt": np.ascontiguousarray(W1D[blk * RB:(blk + 1) * RB, :]),
            "wct": wct,
            "idn": eye128,
        })
    return in_maps


def _marshal_b(res_a, fmap, cls_label, proj_weight, feature_contrast, CP, idxs):
    P = B * C
    ntk = (CP + 7) // 8
    # scatter packed per-slot A partials back to global classes
    a8 = np.stack([res_a[k]["o_a"] for k in range(8)])          # [8, 28, CP*28]
    a8 = a8.reshape(B, NBLK, 28, CP, 28)
    afull = np.zeros((B, C, 28, 28, NBLK), np.float32)
    for b in range(B):
        idx = idxs[b]
        if len(idx):
            # [blk, 28, slot, 28] -> [slot, 28, 28, blk]
            afull[b, idx] = a8[b, :, :, :len(idx), :].transpose(2, 1, 3, 0)
    ain = np.ascontiguousarray(afull).reshape(P, 784 * NBLK)

    cand_v = np.zeros((P, NBLK * NCAND), np.float32)
    cand_i = np.zeros((P, NBLK * NCAND), np.uint32)
    for core in range(8):
        b, blk = core // NBLK, core % NBLK
        tks = [res_a[core][f"o_tk{t}"] for t in range(ntk)]
        for j, c in enumerate(idxs[b]):
            tk = tks[j // 8]
            rb = (j % 8) * 16
            vals = np.concatenate([tk[rb + 14, 0:16], tk[rb + 15, 0:16]])
            gidx = np.concatenate([tk[rb + 14, 16:32], tk[rb + 15, 16:32]])
            cand_v[b * C + c, blk * NCAND:(blk + 1) * NCAND] = vals.view(np.float32)
            cand_i[b * C + c, blk * NCAND:(blk + 1) * NCAND] = gidx

    bbs = np.zeros((P, NBLK * NCAND), np.float32)
    for blk in range(NBLK):
        bbs[:, blk * NCAND:(blk + 1) * NCAND] = blk * RB * W

    # pre-transposed fmap: fmt[sp, u*(B*D) + b*D + d] = fmap[b, d, u*112+sp]
    fm = np.asarray(fmap, np.float32).reshape(B, D, 7, 112)
    fmi = np.ascontiguousarray(fm.transpose(3, 2, 0, 1)).reshape(112, 7 * B * D)

    rnk = np.zeros((P, NCAND), np.float32)
    rnk[:, :K_TOP] = 1.0 / K_TOP

    return {
        "ain": ain,
        "cdv": cand_v,
        "cdi": cand_i,
        "bbs": bbs,
        "fmi": fmi,
        "prj": np.ascontiguousarray(
            np.asarray(proj_weight, np.float32).T.reshape(2, 128, C)
            .transpose(1, 0, 2)).reshape(128, 2 * C),
        "lab": np.asarray(cls_label, np.float32).reshape(P, 1),
        "lab2": np.ascontiguousarray(np.asarray(cls_label, np.float32).T),
        "fc0": np.asarray(feature_contrast, np.float32),
        "eye": np.eye(C, dtype=np.float32),
        "i28": np.tile(np.arange(28, dtype=np.float32)[None, :], (128, 1)),
        "i128": np.tile(np.arange(128, dtype=np.float32)[None, :], (P, 1)),
        "mmb": (np.arange(128)[:, None] // NCAND ==
                np.arange(76)[None, :] - 36).astype(np.float32),
        "rnk": rnk,
        "idn": np.eye(128, dtype=np.float32),
    }


LAST_EXEC_NS = {}


def _run(nc, in_maps, core_ids, tag="k"):
    if os.environ.get("BASSK_SIM") == "1":
        from concourse.bass_interp import CoreSim, MultiCoreSim
        if len(core_ids) == 1:
            sim = CoreSim(nc, trace=False, require_finite=False)
            sims = [sim]
        else:
            msim = MultiCoreSim(nc, num_cores=len(core_ids), trace=False,
                                require_finite=False)
            sims = [msim.cores[i] for i in core_ids]
            sim = msim
        for s, m in zip(sims, in_maps):
            for name, arr in m.items():
                s.tensor(name)[:] = arr
        sim.simulate(check_with_hw=False)
        outs = []
        for s in sims:
            d = {}
            for alloc in nc.m.functions[0].allocations:
                if getattr(alloc, "kind", None) == "ExternalOutput":
                    nm = alloc.memorylocations[0].name
                    d[nm] = np.array(s.tensor(nm))
            outs.append(d)
        return outs
    trace = os.environ.get("BASSK_TRACE") == "1"
    if trace:
        try:
            from antenv.axon_hooks import get_axon_ntff_profile_hook  # noqa: F401
        except Exception:
            trace = False
    res = run_bass_kernel_spmd(nc, in_maps, core_ids, trace=trace)
    if res.exec_time_ns is not None:
        LAST_EXEC_NS[tag] = res.exec_time_ns
    return res.results


def _kernel_slow(fmap, cam, cls_label, proj_weight, feature_contrast,
                 hig_thre, low_thre, bg_thre, idxs, CP):
    nca, ncb = _get_programs(float(hig_thre), float(low_thre), float(bg_thre), CP)
    res_a = _run(nca, _marshal_a(cam, cls_label, CP, idxs), list(range(8)), tag="A")
    in_b = _marshal_b(res_a, fmap, cls_label, proj_weight, feature_contrast, CP, idxs)
    res_b = _run(ncb, [in_b], [0], tag="B")
    return np.float32(res_b[0]["o_loss"].reshape(-1)[0])


def kernel(fmap, cam, cls_label, proj_weight, feature_contrast,
           hig_thre, low_thre, bg_thre):
    fmap = np.asarray(fmap, np.float32)
    cam = np.asarray(cam, np.float32)
    lab = np.asarray(cls_label, np.float32)
    idxs = [np.where(lab[b] > 0.5)[0] for b in range(B)]
    cp_act = max((len(i) for i in idxs), default=0)
    CP = min(C, max(4, ((cp_act + 3) // 4) * 4))

    fc_zero = not np.any(np.asarray(feature_contrast, np.float32))
    if fc_zero and os.environ.get("BASSK_FORCE_SLOW") != "1":
        CPF = max(1, cp_act)
        nca, ncb = _get_programs_fast(float(hig_thre), float(low_thre),
                                      float(bg_thre), CPF)
        res_a = _run(nca, _marshal_a_fast(cam, CPF, idxs), list(range(8)), tag="A")
        in_b = _marshal_b_fast(res_a, fmap, cls_label, proj_weight, CPF, idxs)
        res_b = _run(ncb, [in_b], [0], tag="B")
        cnt = res_b[0]["o_cnt"].reshape(B, C)
        # fast path assumed every present class has masked pixels; verify.
        if not np.any((lab > 0.5) & (cnt < 0.5)):
            loss = np.float32(res_b[0]["o_loss"].reshape(-1)[0])
            return np.asarray(loss, dtype=np.float32).reshape(())
    loss = _kernel_slow(fmap, cam, cls_label, proj_weight, feature_contrast,
                        hig_thre, low_thre, bg_thre, idxs, CP)
    return np.asarray(loss, dtype=np.float32).reshape(())



# revision 29
# speedup vs baseline: 1.0237x; 1.0237x over previous
"""Trainium2 Bass kernel for nn_CPCLoss (self-contained).

Strategy (8 NeuronCores, full inputs in / full output out):
  NEFF-A, SPMD on 8 cores — core k = (batch b=k//4, row-block blk=k%4 of 112
  dst rows). Each core reads its cam shard [20, 112, 448] and computes:
    * per-pixel top1/second/argmax over classes -> pseudo-label class map
    * A_partial[c] = Wr_blk^T @ onehot(q==c+1) @ Wc  (28x28 bilinear-downsample
      coefficient grid per class; Wr/Wc are the static jax.image.resize
      bilinear matrices) via PE matmuls
    * exact per-class top-256 (values+indices) over the 50176 shard pixels via
      the gpsimd topk instruction; top-32 shipped as merge candidates
  Host only reshapes/concats partials (no arithmetic).
  NEFF-B, 1 core — sums partials, merges exact top-25 per (b,c), builds the
  bilinear gather matrix G, selects coef = count==0 ? G/25 : A/max(count,1),
  fsm = coef @ fmap^T, then runs the 2-step EMA memory-bank scan and emits the
  scalar loss.
"""
import os
import sys

os.environ.setdefault("MYCRO_LOCAL_CACHE", "1")
if "/opt/trn_rl_repo" not in sys.path:
    sys.path.insert(0, "/opt/trn_rl_repo")

from contextlib import ExitStack

import numpy as np

from concourse import bacc, bass_isa, mybir, tile
from concourse.bass_utils import run_bass_kernel_spmd


class _StageDone(Exception):
    pass

f32 = mybir.dt.float32
u32 = mybir.dt.uint32
ALU = mybir.AluOpType
AFT = mybir.ActivationFunctionType
AX = mybir.AxisListType

B, C, D = 2, 20, 256
H = W = 448
FH = FW = 28
K_TOP = 25
NBLK = 4
RB = H // NBLK            # 112
NPIX = RB * W             # 50176
NCAND = 32                # candidates shipped per (core, class)
MARGIN = 0.3


def _make_w1d():
    scale = FH / H
    w = np.zeros((H, FH), dtype=np.float64)
    for x in range(H):
        s = (x + 0.5) * scale - 0.5
        i0 = int(np.floor(s))
        f = s - i0
        for i, wt in ((i0, 1.0 - f), (i0 + 1, f)):
            if 0 <= i < FH:
                w[x, i] += wt
        w[x] /= w[x].sum()
    return w.astype(np.float32)


W1D = _make_w1d()


def _emit_topk(nc, out_ap, in_ap, tokens):
    g = nc.gpsimd
    return g.add_instruction(bass_isa.InstTopk(
        name=f"I-{nc.next_id()}",
        ins=[g.lower_ap(in_ap, for_isa=True)],
        outs=[g.lower_ap(out_ap, for_isa=True)],
        _tokens=tokens, _n=NPIX, _k=256))


# --------------------------------------------------------------------------
# NEFF-A
# --------------------------------------------------------------------------

def _build_a(hig, low, bg, CP=C):
    nc = bacc.Bacc("TRN2", target_bir_lowering=False, debug=False, num_devices=8)

    camv = nc.dram_tensor("camv", [CP, NPIX], f32, kind="ExternalInput").ap()
    labt = nc.dram_tensor("labt", [RB, CP], f32, kind="ExternalInput").ap()
    clst = nc.dram_tensor("clst", [RB, CP], f32, kind="ExternalInput").ap()
    iodt = nc.dram_tensor("iodt", [RB, CP], f32, kind="ExternalInput").ap()
    wrt = nc.dram_tensor("wrt", [RB, 28], f32, kind="ExternalInput").ap()
    wct = nc.dram_tensor("wct", [RB, 4 * 28], f32, kind="ExternalInput").ap()
    idn = nc.dram_tensor("idn", [128, 128], f32, kind="ExternalInput").ap()

    o_a = nc.dram_tensor("o_a", [28, CP * 28], f32, kind="ExternalOutput").ap()
    ntk = (CP + 7) // 8
    tok = [min(8, CP - 8 * t) for t in range(ntk)]
    o_tk = [nc.dram_tensor(f"o_tk{t}", [16 * tok[t], 32], u32,
                           kind="ExternalOutput").ap() for t in range(ntk)]

    thmax = float(max(hig, low, bg))

    with tile.TileContext(nc) as tc, ExitStack() as ctx:
        pool = ctx.enter_context(tc.tile_pool(name="p", bufs=1))
        psum = ctx.enter_context(tc.tile_pool(name="ps", bufs=1, space="PSUM"))
        nv = nc.vector

        VP = pool.tile([RB, CP * W], f32)
        nc.sync.dma_start(VP[:], camv.rearrange("c (r w) -> r c w", w=W))
        VT = []
        for t in range(ntk):
            vt = pool.tile([16 * tok[t], NPIX // 16], f32, name=f"VT{t}")
            nc.sync.dma_start(vt[:], camv[8 * t:8 * t + tok[t]]
                              .rearrange("c (g f) -> (c g) f", f=NPIX // 16))
            VT.append(vt)

        LB = pool.tile([RB, CP], f32); nc.sync.dma_start(LB[:], labt)
        CL = pool.tile([RB, CP], f32); nc.sync.dma_start(CL[:], clst)
        IO = pool.tile([RB, CP], f32); nc.sync.dma_start(IO[:], iodt)
        WR = pool.tile([RB, 28], f32); nc.sync.dma_start(WR[:], wrt)
        WC = pool.tile([RB, 4 * 28], f32); nc.sync.dma_start(WC[:], wct)
        IDN = pool.tile([128, 128], f32); nc.sync.dma_start(IDN[:], idn)

        # ---- pseudo-label phase ----
        V_cw = VP[:].rearrange("p (c w) -> p c w", w=W)
        V_wc = VP[:].rearrange("p (c w) -> p w c", w=W)
        LB_b = LB[:].unsqueeze(2).broadcast_to([RB, CP, W])
        nv.tensor_tensor(out=V_cw, in0=V_cw, in1=LB_b, op=ALU.mult)  # valid in-place

        T1 = pool.tile([RB, W], f32)
        nv.tensor_reduce(out=T1[:], in_=V_wc, axis=AX.X, op=ALU.max)

        GE = pool.tile([RB, CP * W], f32)
        GE_cw = GE[:].rearrange("p (c w) -> p c w", w=W)
        T1_b = T1[:].unsqueeze(1).broadcast_to([RB, CP, W])
        nv.tensor_tensor(out=GE_cw, in0=V_cw, in1=T1_b, op=ALU.is_ge)

        EN = pool.tile([RB, CP * W], f32, tag="scr")
        EN_cw = EN[:].rearrange("p (c w) -> p c w", w=W)
        IO_b = IO[:].unsqueeze(2).broadcast_to([RB, CP, W])
        nv.tensor_tensor(out=EN_cw, in0=GE_cw, in1=IO_b, op=ALU.mult)
        AM = pool.tile([RB, W], f32)
        nv.tensor_reduce(out=AM[:], in_=EN[:].rearrange("p (c w) -> p w c", w=W),
                         axis=AX.X, op=ALU.max)

        MK = pool.tile([RB, CP * W], f32, tag="scr")
        MK_cw = MK[:].rearrange("p (c w) -> p c w", w=W)
        nv.scalar_tensor_tensor(out=MK_cw, in0=GE_cw, scalar=-1e9, in1=V_cw,
                                op0=ALU.mult, op1=ALU.add)
        SC = pool.tile([RB, W], f32)
        nv.tensor_reduce(out=SC[:], in_=MK[:].rearrange("p (c w) -> p w c", w=W),
                         axis=AX.X, op=ALU.max)

        # keep iff top1 >= max(hig,low,bg) and (margin >= 0.3 or top1 <= hig)
        KG = pool.tile([RB, W], f32)
        nv.tensor_scalar(out=KG[:], in0=T1[:], scalar1=thmax, scalar2=None, op0=ALU.is_ge)
        MGOK = pool.tile([RB, W], f32)
        nv.tensor_tensor(out=MGOK[:], in0=T1[:], in1=SC[:], op=ALU.subtract)
        nv.tensor_scalar(out=MGOK[:], in0=MGOK[:], scalar1=MARGIN, scalar2=None, op0=ALU.is_ge)
        LEH = pool.tile([RB, W], f32)
        nv.tensor_scalar(out=LEH[:], in0=T1[:], scalar1=float(hig), scalar2=None, op0=ALU.is_le)
        nv.tensor_tensor(out=MGOK[:], in0=MGOK[:], in1=LEH[:], op=ALU.max)
        nv.tensor_tensor(out=KG[:], in0=KG[:], in1=MGOK[:], op=ALU.mult)
        Q = pool.tile([RB, W], f32)
        nv.tensor_scalar(out=Q[:], in0=AM[:], scalar1=-1.0, scalar2=float(CP + 1),
                         op0=ALU.mult, op1=ALU.add)
        nv.tensor_tensor(out=Q[:], in0=Q[:], in1=KG[:], op=ALU.mult)

        # ---- q transpose + one-hot EQT + matmuls for A ----
        QT = pool.tile([RB, 4 * RB], f32)
        for u in range(4):
            QTP = psum.tile([RB, RB], f32, tag="qtp")
            nc.tensor.transpose(QTP[:], Q[:, u * RB:(u + 1) * RB], IDN[:RB, :RB])
            nc.scalar.copy(QT[:, u * RB:(u + 1) * RB], QTP[:])

        EQT = pool.tile([RB, 4 * CP * RB], f32)
        for u in range(4):
            sl = EQT[:, u * CP * RB:(u + 1) * CP * RB]
            sl_cw = sl.rearrange("p (c r) -> p c r", r=RB)
            QT_b = QT[:, u * RB:(u + 1) * RB].unsqueeze(1).broadcast_to([RB, CP, RB])
            CL_b = CL[:].unsqueeze(2).broadcast_to([RB, CP, RB])
            nv.tensor_tensor(out=sl_cw, in0=QT_b, in1=CL_b, op=ALU.is_equal)
        # PSUM bank = 512 f32: hold 5 classes (140 cols) per bank-tile
        ngrp = (CP + 4) // 5
        T0sb = pool.tile([RB, CP * 28], f32)
        Asb = pool.tile([28, CP * 28], f32)
        T0ps = [psum.tile([RB, 5 * 28], f32, name=f"t0ps{i}", tag="accps", bufs=4)
                for i in range(ngrp)]
        Aps = [psum.tile([28, 5 * 28], f32, name=f"aps{i}", tag="accps", bufs=4)
               for i in range(ngrp)]
        for c in range(CP):
            grp, off = c // 5, (c % 5) * 28
            for u in range(4):
                nc.tensor.matmul(
                    T0ps[grp][:, off:off + 28],
                    lhsT=EQT[:, u * CP * RB + c * RB:u * CP * RB + (c + 1) * RB],
                    rhs=WC[:, u * 28:(u + 1) * 28],
                    start=(u == 0), stop=(u == 3))
        for i in range(ngrp):
            w0 = i * 140
            w1 = min(w0 + 140, CP * 28)
            nc.scalar.copy(T0sb[:, w0:w1], T0ps[i][:, 0:w1 - w0])
        for c in range(CP):
            grp, off = c // 5, (c % 5) * 28
            nc.tensor.matmul(Aps[grp][:, off:off + 28], lhsT=WR[:],
                             rhs=T0sb[:, c * 28:(c + 1) * 28], start=True, stop=True)
        for i in range(ngrp):
            w0 = i * 140
            w1 = min(w0 + 140, CP * 28)
            nc.scalar.copy(Asb[:, w0:w1], Aps[i][:, 0:w1 - w0])
        nc.sync.dma_start(o_a, Asb[:])

        # ---- per-class topk ----
        for t in range(ntk):
            tkt = pool.tile([16 * tok[t], 32], u32, name=f"TK{t}")
            _emit_topk(nc, tkt[:], VT[t][:], tokens=tok[t])
            nc.sync.dma_start(o_tk[t], tkt[:])

    nc.compile()
    return nc


# --------------------------------------------------------------------------
# NEFF-B
# --------------------------------------------------------------------------

def _build_b(stage=99):
    nc = bacc.Bacc("TRN2", target_bir_lowering=False, debug=False, num_devices=1)
    P = B * C  # 40 (b,c) pairs

    ain = nc.dram_tensor("ain", [P, 784 * NBLK], f32, kind="ExternalInput").ap()
    cdv = nc.dram_tensor("cdv", [P, NBLK * NCAND], f32, kind="ExternalInput").ap()
    cdi = nc.dram_tensor("cdi", [P, NBLK * NCAND], u32, kind="ExternalInput").ap()
    bbs = nc.dram_tensor("bbs", [P, NBLK * NCAND], f32, kind="ExternalInput").ap()
    fmi = nc.dram_tensor("fmi", [112, 7 * B * D], f32, kind="ExternalInput").ap()
    prj = nc.dram_tensor("prj", [128, 2 * C], f32, kind="ExternalInput").ap()
    lab = nc.dram_tensor("lab", [P, 1], f32, kind="ExternalInput").ap()
    lab2 = nc.dram_tensor("lab2", [C, B], f32, kind="ExternalInput").ap()
    fc0 = nc.dram_tensor("fc0", [C, D], f32, kind="ExternalInput").ap()
    eye = nc.dram_tensor("eye", [C, C], f32, kind="ExternalInput").ap()
    i28 = nc.dram_tensor("i28", [128, 28], f32, kind="ExternalInput").ap()
    i128 = nc.dram_tensor("i128", [P, 128], f32, kind="ExternalInput").ap()
    mmb = nc.dram_tensor("mmb", [128, 76], f32, kind="ExternalInput").ap()
    rnk = nc.dram_tensor("rnk", [P, NCAND], f32, kind="ExternalInput").ap()
    idn = nc.dram_tensor("idn", [128, 128], f32, kind="ExternalInput").ap()

    o_loss = nc.dram_tensor("o_loss", [1, 1], f32, kind="ExternalOutput").ap()
    o_dbg = nc.dram_tensor("o_dbg", [128, 1024], f32, kind="ExternalOutput").ap()

    NC128 = NBLK * NCAND  # 128 candidates per pair

    try:
      with tile.TileContext(nc) as tc, ExitStack() as ctx:
        pool = ctx.enter_context(tc.tile_pool(name="p", bufs=1))
        psum = ctx.enter_context(tc.tile_pool(name="ps", bufs=1, space="PSUM"))
        nv = nc.vector
        ns = nc.scalar

        AIN = pool.tile([P, 784 * NBLK], f32); nc.sync.dma_start(AIN[:], ain)
        CV = pool.tile([P, NC128], f32); nc.sync.dma_start(CV[:], cdv)
        CI = pool.tile([P, NC128], u32); nc.sync.dma_start(CI[:], cdi)
        BBS = pool.tile([P, NC128], f32); nc.sync.dma_start(BBS[:], bbs)
        FM = pool.tile([112, 7 * B * D], f32); nc.sync.dma_start(FM[:], fmi)
        PJT = pool.tile([128, 2 * C], f32); nc.sync.dma_start(PJT[:], prj)
        LAB = pool.tile([P, 1], f32); nc.sync.dma_start(LAB[:], lab)
        LAB2 = pool.tile([C, B], f32); nc.sync.dma_start(LAB2[:], lab2)
        FC = pool.tile([C, D], f32); nc.sync.dma_start(FC[:], fc0)
        EYE = pool.tile([C, C], f32); nc.sync.dma_start(EYE[:], eye)
        I28 = pool.tile([128, 28], f32); nc.sync.dma_start(I28[:], i28)
        I128 = pool.tile([P, 128], f32); nc.sync.dma_start(I128[:], i128)
        MMB = pool.tile([128, 76], f32); nc.sync.dma_start(MMB[:], mmb)
        RNK = pool.tile([P, NCAND], f32); nc.sync.dma_start(RNK[:], rnk)
        IDN = pool.tile([128, 128], f32); nc.sync.dma_start(IDN[:], idn)

        # ---- A, counts ----
        A = pool.tile([P, 784], f32)
        nv.tensor_reduce(out=A[:], in_=AIN[:].rearrange("p (s k) -> p s k", k=NBLK),
                         axis=AX.X, op=ALU.add)
        CNT = pool.tile([P, 1], f32)
        nv.tensor_reduce(out=CNT[:], in_=A[:], axis=AX.X, op=ALU.add)
        ISZ = pool.tile([P, 1], u32)
        nv.tensor_scalar(out=ISZ[:], in0=CNT[:], scalar1=0.5, scalar2=None, op0=ALU.is_lt)
        DEN = pool.tile([P, 1], f32)
        nv.tensor_scalar(out=DEN[:], in0=CNT[:], scalar1=1.0, scalar2=None, op0=ALU.max)

        # ---- merge top-32 of 128 candidates ----
        CIF = pool.tile([P, NC128], f32)
        nv.tensor_copy(CIF[:], CI[:])
        nv.tensor_tensor(out=CIF[:], in0=CIF[:], in1=BBS[:], op=ALU.add)
        CVa = pool.tile([P, NC128], f32)
        nv.tensor_copy(CVa[:], CV[:])
        MV = pool.tile([P, NCAND], f32)
        MP = pool.tile([P, NCAND], u32)
        for r in range(4):
            nv.max(out=MV[:, r * 8:(r + 1) * 8], in_=CVa[:])
            nv.max_index(out=MP[:, r * 8:(r + 1) * 8],
                         in_max=MV[:, r * 8:(r + 1) * 8], in_values=CVa[:])
            nv.match_replace(out=CVa[:], in_to_replace=MV[:, r * 8:(r + 1) * 8],
                             in_values=CVa[:], imm_value=-1.0)
        MPF = pool.tile([P, NCAND], f32)
        nv.tensor_copy(MPF[:], MP[:])
        # gather global idx at positions
        EQP = pool.tile([P, NCAND * 128], f32)
        EQP_v = EQP[:].rearrange("p (k q) -> p k q", q=128)
        nv.tensor_tensor(out=EQP_v, in0=MPF[:].unsqueeze(2).broadcast_to([P, NCAND, 128]),
                         in1=I128[:].unsqueeze(1).broadcast_to([P, NCAND, 128]),
                         op=ALU.is_equal)
        nv.tensor_tensor(out=EQP_v, in0=EQP_v,
                         in1=CIF[:].unsqueeze(1).broadcast_to([P, NCAND, 128]), op=ALU.mult)
        GIX = pool.tile([P, NCAND], f32)
        nv.tensor_reduce(out=GIX[:], in_=EQP_v, axis=AX.X, op=ALU.max)

        if stage <= 1:
            DBG = pool.tile([P, 64], f32)
            nv.tensor_copy(DBG[:, 0:32], GIX[:])
            nv.tensor_copy(DBG[:, 32:64], MPF[:])
            nc.sync.dma_start(o_dbg[0:P, 0:64], DBG[:])
        # ---- interpolation coefficients ----
        def ts(dst, src, s1, s2, op0, op1=None):
            nv.tensor_scalar(out=dst, in0=src, scalar1=s1, scalar2=s2, op0=op0,
                             **({"op1": op1} if op1 is not None else {}))

        if stage <= 1:
            OUTZ = pool.tile([1, 1], f32)
            nv.memset(OUTZ[:], 0.0)
            nc.sync.dma_start(o_loss, OUTZ[:])
            raise _StageDone()

        i32 = mybir.dt.int32

        def floor_pos(XX, pfx):
            """floor(x) for x>=0: round-to-nearest (f32->i32->f32 copy) then
            subtract 1 where round went up."""
            RI = pool.tile([P, NCAND], i32, name=f"{pfx}_ri", tag=f"{pfx}_ri")
            nv.tensor_copy(RI[:], XX[:])
            RF = pool.tile([P, NCAND], f32, name=f"{pfx}_rf", tag=f"{pfx}_rf")
            nv.tensor_copy(RF[:], RI[:])
            GT = pool.tile([P, NCAND], f32, name=f"{pfx}_gt", tag=f"{pfx}_gt")
            nv.tensor_tensor(out=GT[:], in0=RF[:], in1=XX[:], op=ALU.is_gt)
            nv.tensor_tensor(out=RF[:], in0=RF[:], in1=GT[:], op=ALU.subtract)
            return RF

        TT = pool.tile([P, NCAND], f32)
        ts(TT[:], GIX[:], 1.0 / 448.0, None, ALU.mult)
        HH = floor_pos(TT, "fh")
        WW = pool.tile([P, NCAND], f32)
        nv.scalar_tensor_tensor(out=WW[:], in0=HH[:], scalar=-448.0, in1=GIX[:],
                                op0=ALU.mult, op1=ALU.add)

        def coeffs(XX, pfx):
            U = pool.tile([P, NCAND], f32, name=f"{pfx}_u", tag=f"{pfx}_u")
            ts(U[:], XX[:], 8.5, 1.0 / 16.0, ALU.add, ALU.mult)
            FL = floor_pos(U, f"{pfx}_flr")
            F = pool.tile([P, NCAND], f32, name=f"{pfx}_f", tag=f"{pfx}_f")
            nv.tensor_tensor(out=F[:], in0=U[:], in1=FL[:], op=ALU.subtract)
            X0 = pool.tile([P, NCAND], f32, name=f"{pfx}_x0", tag=f"{pfx}_x0")
            ts(X0[:], FL[:], 1.0, None, ALU.subtract)
            ts(X0[:], X0[:], 0.0, 27.0, ALU.max, ALU.min)
            X1 = pool.tile([P, NCAND], f32, name=f"{pfx}_x1", tag=f"{pfx}_x1")
            ts(X1[:], FL[:], 0.0, 27.0, ALU.max, ALU.min)
            W1 = F
            W0 = pool.tile([P, NCAND], f32, name=f"{pfx}_w0", tag=f"{pfx}_w0")
            ts(W0[:], F[:], -1.0, 1.0, ALU.mult, ALU.add)
            return X0, X1, W0, W1

        I0, I1, WH0, WH1 = coeffs(HH, "ch")
        J0, J1, WWA, WWB = coeffs(WW, "cw")
        WW0 = pool.tile([P, NCAND], f32)
        nv.tensor_tensor(out=WW0[:], in0=WWA[:], in1=RNK[:], op=ALU.mult)
        WW1 = pool.tile([P, NCAND], f32)
        nv.tensor_tensor(out=WW1[:], in0=WWB[:], in1=RNK[:], op=ALU.mult)

        if stage == 2:
            DBG2 = pool.tile([P, 128], f32)
            for i, t in enumerate([I0, I1, WH0, WH1]):
                nv.tensor_copy(DBG2[:, i * 32:(i + 1) * 32], t[:])
            nc.sync.dma_start(o_dbg[0:P, 0:128], DBG2[:])
        # ---- stage (pair,k)-flatten and G build ----
        STG = pool.tile([P, NCAND * 8], f32)
        STG_v = STG[:].rearrange("p (k a) -> p k a", a=8)
        for idx, arr in enumerate([I0, I1, WH0, WH1, J0, J1, WW0, WW1]):
            nv.tensor_copy(STG_v[:, :, idx:idx + 1], arr[:].unsqueeze(2))

        if stage == 2:
            OUTZ = pool.tile([1, 1], f32)
            nv.memset(OUTZ[:], 0.0)
            nc.sync.dma_start(o_loss, OUTZ[:])
            raise _StageDone()

        FLT = pool.tile([128, 80], f32)
        for g in range(10):
            nc.sync.dma_start(
                FLT[:, g * 8:(g + 1) * 8],
                STG[g * 4:(g + 1) * 4, :].rearrange("p (k a) -> p k a", a=8))

        G = pool.tile([P, 784], f32)
        GpsA = psum.tile([P, 392], f32)
        GpsB = psum.tile([P, 392], f32)
        for g in range(10):
            col = lambda i: FLT[:, g * 8 + i:g * 8 + i + 1]
            EQR0 = pool.tile([128, 28], f32, tag="eqr", bufs=2)
            nv.tensor_scalar(out=EQR0[:], in0=I28[:], scalar1=col(0), scalar2=None,
                             op0=ALU.is_equal)
            RQ = pool.tile([128, 28], f32, tag="rq", bufs=2)
            nv.tensor_scalar(out=RQ[:], in0=EQR0[:], scalar1=col(2), scalar2=None,
                             op0=ALU.mult)
            EQR1 = pool.tile([128, 28], f32, tag="eqr2", bufs=2)
            nv.tensor_scalar(out=EQR1[:], in0=I28[:], scalar1=col(1), scalar2=None,
                             op0=ALU.is_equal)
            nv.scalar_tensor_tensor(out=RQ[:], in0=EQR1[:], scalar=col(3), in1=RQ[:],
                                    op0=ALU.mult, op1=ALU.add)
            EQC0 = pool.tile([128, 28], f32, tag="eqr", bufs=2)
            nv.tensor_scalar(out=EQC0[:], in0=I28[:], scalar1=col(4), scalar2=None,
                             op0=ALU.is_equal)
            CQ = pool.tile([128, 28], f32, tag="cq", bufs=2)
            nv.tensor_scalar(out=CQ[:], in0=EQC0[:], scalar1=col(6), scalar2=None,
                             op0=ALU.mult)
            EQC1 = pool.tile([128, 28], f32, tag="eqr2", bufs=2)
            nv.tensor_scalar(out=EQC1[:], in0=I28[:], scalar1=col(5), scalar2=None,
                             op0=ALU.is_equal)
            nv.scalar_tensor_tensor(out=CQ[:], in0=EQC1[:], scalar=col(7), in1=CQ[:],
                                    op0=ALU.mult, op1=ALU.add)
            RHS = pool.tile([128, 784], f32, tag="rhs", bufs=2)
            nv.tensor_tensor(out=RHS[:].rearrange("p (a b) -> p a b", b=28),
                             in0=RQ[:].unsqueeze(2).broadcast_to([128, 28, 28]),
                             in1=CQ[:].unsqueeze(1).broadcast_to([128, 28, 28]),
                             op=ALU.mult)
            # band-membership lhsT: col j of MMB[:, 36-4g : 76-4g] is
            # one-hot(q//32 == j-4g) -> group g's 4 pairs land on rows 4g..4g+3
            lhsT_g = MMB[:, 36 - 4 * g:76 - 4 * g]
            nc.tensor.matmul(GpsA[:], lhsT=lhsT_g, rhs=RHS[:, 0:392],
                             start=(g == 0), stop=(g == 9))
            nc.tensor.matmul(GpsB[:], lhsT=lhsT_g, rhs=RHS[:, 392:784],
                             start=(g == 0), stop=(g == 9))
        ns.copy(G[:, 0:392], GpsA[:])
        ns.copy(G[:, 392:784], GpsB[:])

        if stage == 3:
            nc.sync.dma_start(o_dbg[0:P, 0:784], G[:])
        if stage == 35:
            nc.sync.dma_start(o_dbg[0:128, 0:80], FLT[:])
        # ---- coef + fsm ----
        if stage in (3, 35):
            OUTZ = pool.tile([1, 1], f32)
            nv.memset(OUTZ[:], 0.0)
            nc.sync.dma_start(o_loss, OUTZ[:])
            raise _StageDone()

        RDEN = pool.tile([P, 1], f32)
        nv.reciprocal(RDEN[:], DEN[:])
        AMN = pool.tile([P, 784], f32)
        nv.tensor_scalar(out=AMN[:], in0=A[:], scalar1=RDEN[:], scalar2=None, op0=ALU.mult)
        COEF = pool.tile([P, 784], f32)
        nv.select(COEF[:], ISZ[:].broadcast_to([P, 784]), G[:], AMN[:])
        nv.tensor_scalar(out=COEF[:], in0=COEF[:], scalar1=LAB[:], scalar2=None, op0=ALU.mult)

        CT = pool.tile([RB, 7 * P], f32)
        for u in range(7):
            TPS = psum.tile([RB, P], f32, tag="tps", bufs=2)
            nc.tensor.transpose(TPS[:], COEF[:, u * RB:(u + 1) * RB], IDN[:P, :P])
            ns.copy(CT[:, u * P:(u + 1) * P], TPS[:])

        FSM = pool.tile([C, B * D], f32)
        for b2 in range(B):
            FSps = psum.tile([C, D], f32, tag="fsps")
            for u in range(7):
                nc.tensor.matmul(FSps[:], lhsT=CT[:, u * P + b2 * C:u * P + (b2 + 1) * C],
                                 rhs=FM[:, u * (B * D) + b2 * D:u * (B * D) + (b2 + 1) * D],
                                 start=(u == 0), stop=(u == 6))
            ns.copy(FSM[:, b2 * D:(b2 + 1) * D], FSps[:])

        if stage == 4:
            nc.sync.dma_start(o_dbg[0:C, 0:B * D], FSM[:])
        # ---- scan ----
        if stage == 4:
            OUTZ = pool.tile([1, 1], f32)
            nv.memset(OUTZ[:], 0.0)
            nc.sync.dma_start(o_loss, OUTZ[:])
            raise _StageDone()

        ONES20 = pool.tile([C, 1], f32)
        nv.memset(ONES20[:], 1.0)
        LC = pool.tile([1, 1], f32); nv.memset(LC[:], 0.0)
        CCF = pool.tile([1, 1], f32); nv.memset(CCF[:], 0.0)
        SCR = pool.tile([C, D], f32, tag="scr")
        SCR2 = pool.tile([C, C], f32, tag="scr2")

        def l2norm_div(dst, src):
            nn2 = pool.tile([C, 1], f32, tag="nn2")
            nv.tensor_tensor(out=SCR[:], in0=src, in1=src, op=ALU.mult)
            nv.tensor_reduce(out=nn2[:], in_=SCR[:], axis=AX.X, op=ALU.add)
            nr = pool.tile([C, 1], f32, tag="nr")
            ns.activation(nr[:], nn2[:], AFT.Sqrt)
            nv.tensor_scalar(out=nr[:], in0=nr[:], scalar1=1e-12, scalar2=None, op0=ALU.max)
            rn = pool.tile([C, 1], f32, tag="rn")
            nv.reciprocal(rn[:], nr[:])
            nv.tensor_scalar(out=dst, in0=src, scalar1=rn[:], scalar2=None, op0=ALU.mult)

        for b2 in range(B):
            FSMb = FSM[:, b2 * D:(b2 + 1) * D]
            presb = LAB2[:, b2:b2 + 1]

            FSMN = pool.tile([C, D], f32, tag="fsmn")
            l2norm_div(FSMN[:], FSMb)
            FCN = pool.tile([C, D], f32, tag="fcn")
            l2norm_div(FCN[:], FC[:])

            # transposes of fsm (raw), fsm_n, fc_n -> [128, C] chunks
            TRS = {}
            for nm, srct in (("fsm", FSMb), ("fsmn", FSMN[:]), ("fcn", FCN[:])):
                dst = pool.tile([128, 2 * C], f32, tag=f"tr_{nm}", name=f"tr_{nm}_{b2}")
                for h2 in range(2):
                    TPS4 = psum.tile([128, C], f32, tag="tps", bufs=2)
                    nc.tensor.transpose(TPS4[:], srct[:, h2 * 128:(h2 + 1) * 128],
                                        IDN[:C, :C])
                    ns.copy(dst[:, h2 * C:(h2 + 1) * C], TPS4[:])
                TRS[nm] = dst

            COSps = psum.tile([C, C], f32, tag="cosps")
            for h2 in range(2):
                nc.tensor.matmul(COSps[:], lhsT=TRS["fsmn"][:, h2 * C:(h2 + 1) * C],
                                 rhs=TRS["fcn"][:, h2 * C:(h2 + 1) * C],
                                 start=(h2 == 0), stop=(h2 == 1))
            COSC = pool.tile([C, C], f32, tag="cosc")
            ns.activation(COSC[:], COSps[:], AFT.Abs)
            nv.tensor_scalar(out=COSC[:], in0=COSC[:], scalar1=1e-5, scalar2=1.0 - 1e-5,
                             op0=ALU.max, op1=ALU.min)
            LGC = pool.tile([C, C], f32, tag="lgc")
            ns.activation(LGC[:], COSC[:], AFT.Ln)
            OM = pool.tile([C, C], f32, tag="om")
            nv.tensor_scalar(out=OM[:], in0=COSC[:], scalar1=-1.0, scalar2=1.0,
                             op0=ALU.mult, op1=ALU.add)
            LOM = pool.tile([C, C], f32, tag="lom")
            ns.activation(LOM[:], OM[:], AFT.Ln)

            IDM = pool.tile([C, C], f32, tag="idm")
            nv.tensor_scalar(out=IDM[:], in0=EYE[:], scalar1=presb, scalar2=None, op0=ALU.mult)
            DIF = pool.tile([C, C], f32, tag="dif")
            nv.tensor_tensor(out=DIF[:], in0=LGC[:], in1=LOM[:], op=ALU.subtract)
            CCFD = pool.tile([C, 1], f32, tag="ccfd")
            nv.tensor_tensor(out=SCR2[:], in0=IDM[:], in1=DIF[:], op=ALU.mult)
            nv.tensor_reduce(out=CCFD[:], in_=SCR2[:], axis=AX.X, op=ALU.add)
            R1 = pool.tile([C, 1], f32, tag="r1")
            nv.tensor_reduce(out=R1[:], in_=LOM[:], axis=AX.X, op=ALU.add)
            nv.tensor_tensor(out=CCFD[:], in0=CCFD[:], in1=R1[:], op=ALU.add)

            COSM = pool.tile([C, C], f32, tag="cosm")
            nv.scalar_tensor_tensor(out=COSM[:], in0=EYE[:], scalar=-1e9, in1=COSC[:],
                                    op0=ALU.mult, op1=ALU.add)
            OFF = pool.tile([C, 1], f32, tag="off")
            nv.tensor_reduce(out=OFF[:], in_=COSM[:], axis=AX.X, op=ALU.max)
            QUAL = pool.tile([C, 1], f32, tag="qual")
            nv.tensor_scalar(out=QUAL[:], in0=OFF[:], scalar1=0.6, scalar2=None, op0=ALU.is_lt)
            nv.tensor_tensor(out=QUAL[:], in0=QUAL[:], in1=presb, op=ALU.mult)

            LOGps = psum.tile([C, C], f32, tag="cosps")
            for h2 in range(2):
                nc.tensor.matmul(LOGps[:], lhsT=TRS["fsm"][:, h2 * C:(h2 + 1) * C],
                                 rhs=PJT[:, h2 * C:(h2 + 1) * C],
                                 start=(h2 == 0), stop=(h2 == 1))
            MX = pool.tile([C, 1], f32, tag="mx")
            nv.tensor_reduce(out=MX[:], in_=LOGps, axis=AX.X, op=ALU.max)
            XT = pool.tile([C, C], f32, tag="xt")
            nv.tensor_scalar(out=XT[:], in0=LOGps, scalar1=MX[:], scalar2=None,
                             op0=ALU.subtract)
            ET = pool.tile([C, C], f32, tag="et")
            ns.activation(ET[:], XT[:], AFT.Exp)
            SM = pool.tile([C, 1], f32, tag="sm")
            nv.tensor_reduce(out=SM[:], in_=ET[:], axis=AX.X, op=ALU.add)
            LGS = pool.tile([C, 1], f32, tag="lgs")
            ns.activation(LGS[:], SM[:], AFT.Ln)
            LGP = pool.tile([C, C], f32, tag="lgp")
            nv.tensor_scalar(out=LGP[:], in0=XT[:], scalar1=LGS[:], scalar2=-100.0,
                             op0=ALU.subtract, op1=ALU.max)
            SME = pool.tile([C, C], f32, tag="sme")
            nv.tensor_tensor(out=SME[:], in0=SM[:].broadcast_to([C, C]), in1=ET[:],
                             op=ALU.subtract)
            LSME = pool.tile([C, C], f32, tag="lsme")
            ns.activation(LSME[:], SME[:], AFT.Ln)
            L1P = pool.tile([C, C], f32, tag="l1p")
            nv.tensor_scalar(out=L1P[:], in0=LSME[:], scalar1=LGS[:], scalar2=-100.0,
                             op0=ALU.subtract, op1=ALU.max)

            DD = pool.tile([C, C], f32, tag="dd")
            nv.tensor_tensor(out=DD[:], in0=LGP[:], in1=L1P[:], op=ALU.subtract)
            DDG = pool.tile([C, 1], f32, tag="ddg")
            nv.tensor_tensor(out=SCR2[:], in0=EYE[:], in1=DD[:], op=ALU.mult)
            nv.tensor_reduce(out=DDG[:], in_=SCR2[:], axis=AX.X, op=ALU.add)
            RSM = pool.tile([C, 1], f32, tag="rsm")
            nv.tensor_reduce(out=RSM[:], in_=L1P[:], axis=AX.X, op=ALU.add)
            TERM = pool.tile([C, 1], f32, tag="term")
            nv.tensor_tensor(out=TERM[:], in0=DDG[:], in1=RSM[:], op=ALU.add)
            nv.tensor_scalar(out=TERM[:], in0=TERM[:], scalar1=-1.0 / C, scalar2=None,
                             op0=ALU.mult)
            CONTR = pool.tile([C, 1], f32, tag="contr")
            nv.tensor_tensor(out=CONTR[:], in0=TERM[:], in1=QUAL[:], op=ALU.mult)

            PR = pool.tile([C, 3], f32, tag="pr")
            nv.tensor_copy(PR[:, 0:1], QUAL[:])
            nv.tensor_copy(PR[:, 1:2], CONTR[:])
            nv.tensor_copy(PR[:, 2:3], CCFD[:])
            REDps = psum.tile([1, 3], f32, tag="redps")
            nc.tensor.matmul(REDps[:], lhsT=ONES20[:], rhs=PR[:], start=True, stop=True)
            RED = pool.tile([1, 3], f32, tag="red")
            ns.copy(RED[:], REDps[:])

            # loss_cls = (loss_cls + S) / max(n, 1)   (divide-by-1 when n==0)
            nv.tensor_tensor(out=LC[:], in0=LC[:], in1=RED[:, 1:2], op=ALU.add)
            NB1 = pool.tile([1, 1], f32, tag="nb1")
            nv.tensor_scalar(out=NB1[:], in0=RED[:, 0:1], scalar1=1.0, scalar2=None,
                             op0=ALU.max)
            RNB = pool.tile([1, 1], f32, tag="rnb")
            nv.reciprocal(RNB[:], NB1[:])
            nv.tensor_scalar(out=LC[:], in0=LC[:], scalar1=RNB[:], scalar2=None,
                             op0=ALU.mult)
            # loss_ccf += -(1/400) * ccf_sum
            nv.scalar_tensor_tensor(out=CCF[:], in0=RED[:, 2:3], scalar=-1.0 / (C * C),
                                    in1=CCF[:], op0=ALU.mult, op1=ALU.add)

            # fc = fc + 0.05 * qual * (fsm - fc)
            DFC = pool.tile([C, D], f32, tag="dfc")
            nv.tensor_tensor(out=DFC[:], in0=FSMb, in1=FC[:], op=ALU.subtract)
            Q05 = pool.tile([C, 1], f32, tag="q05")
            nv.tensor_scalar(out=Q05[:], in0=QUAL[:], scalar1=0.05, scalar2=None,
                             op0=ALU.mult)
            nv.scalar_tensor_tensor(out=FC[:], in0=DFC[:], scalar=Q05[:], in1=FC[:],
                                    op0=ALU.mult, op1=ALU.add)

        OUT = pool.tile([1, 1], f32)
        nv.tensor_tensor(out=OUT[:], in0=LC[:], in1=CCF[:], op=ALU.add)
        nc.sync.dma_start(o_loss, OUT[:])
    except _StageDone:
        pass

    nc.compile()
    return nc


# --------------------------------------------------------------------------
# Fast path (no top-k: valid when every present class has count > 0).
# --------------------------------------------------------------------------

bf16 = mybir.dt.bfloat16
f16 = mybir.dt.float16


def _emit_tree(nc, pool, src, n, width, op, pfx, dt=None, part=None):
    """Binary-tree reduce over n leaves of `width` cols each -> [P, width]."""
    nv = nc.vector
    dt = bf16 if dt is None else dt
    part = RB if part is None else part
    cur = src
    lvl = 0
    while n > 1:
        h = n // 2
        odd = n - 2 * h
        dst = pool.tile([part, h * width], dt, name=f"{pfx}_l{lvl}")
        nv.tensor_tensor(out=dst[:], in0=cur[:, :h * width],
                         in1=cur[:, h * width:2 * h * width], op=op)
        if odd:
            nv.tensor_tensor(out=dst[:, :width], in0=dst[:, :width],
                             in1=cur[:, 2 * h * width:(2 * h + 1) * width], op=op)
        cur, n, lvl = dst, h, lvl + 1
    return cur


def _build_a_fast(hig, low, bg, CP):
    nc = bacc.Bacc("TRN2", target_bir_lowering=False, debug=False, num_devices=8)

    camv = nc.dram_tensor("camv", [CP, NPIX], bf16, kind="ExternalInput").ap()
    wrt = nc.dram_tensor("wrt", [RB, 28], bf16, kind="ExternalInput").ap()
    wct = nc.dram_tensor("wct", [RB, 4 * 28], f16, kind="ExternalInput").ap()
    o_a = nc.dram_tensor("o_a", [28, CP * 28], f32, kind="ExternalOutput").ap()

    thmax = float(max(hig, low, bg))
    # class groups of <=4 (PSUM bank = 512 f32 = 4 classes x 4 u x 28)
    grps = []
    c0 = 0
    while c0 < CP:
        n = min(4, CP - c0)
        grps.append((c0, n))
        c0 += n
    ch = (CP + 1) // 2  # class-split DMA halves

    with tile.TileContext(nc) as tc, ExitStack() as ctx:
        pool = ctx.enter_context(tc.tile_pool(name="p", bufs=1))
        psum = ctx.enter_context(tc.tile_pool(name="ps", bufs=1, space="PSUM"))
        nv = nc.vector
        ns = nc.scalar

        VP = pool.tile([RB, CP * W], bf16)
        # class-quarters so partial max trees overlap the later DMA chunks
        qs = []
        q0 = 0
        while q0 < CP:
            qn = min(max(1, (CP + 3) // 4), CP - q0)
            qs.append((q0, qn))
            q0 += qn
        for (q0_, qn_) in qs:
            nc.sync.dma_start(VP[:, q0_ * W:(q0_ + qn_) * W],
                              camv[q0_:q0_ + qn_].rearrange("c (r w) -> r c w",
                                                            w=W))
        WR = pool.tile([RB, 28], bf16)
        nc.sync.dma_start(WR[:], wrt)
        WC = pool.tile([RB, 4 * 28], f16)
        nc.sync.dma_start(WC[:], wct)

        # ---- per-pixel keep-gate (bf16, w innermost so TTs hit 2x mode) ----
        parts = [_emit_tree(nc, pool, VP[:, a * W:(a + n) * W], n, W, ALU.max,
                            f"t1q{i}") for i, (a, n) in enumerate(qs)]
        while len(parts) > 1:
            nxt = []
            for i in range(0, len(parts) - 1, 2):
                t = pool.tile([RB, W], bf16, name=f"t1m{len(parts)}_{i}")
                nv.tensor_tensor(out=t[:], in0=parts[i][:], in1=parts[i + 1][:],
                                 op=ALU.max)
                nxt.append(t)
            if len(parts) % 2:
                nxt.append(parts[-1])
            parts = nxt
        T1 = parts[0]
        T13 = pool.tile([RB, W], bf16)
        nv.tensor_scalar(out=T13[:], in0=T1[:], scalar1=-MARGIN, scalar2=None,
                         op0=ALU.add)
        NG = pool.tile([RB, CP * W], bf16)
        NG_cw = NG[:].rearrange("p (c w) -> p c w", w=W)
        V_cw = VP[:].rearrange("p (c w) -> p c w", w=W)
        nv.tensor_tensor(out=NG_cw, in0=V_cw,
                         in1=T13[:].unsqueeze(1).broadcast_to([RB, CP, W]),
                         op=ALU.is_gt)
        NGS = _emit_tree(nc, pool, NG, CP, W, ALU.add, "ngs")

        # keep iff t1 >= thmax and (exactly one class above t1-0.3 or t1 <= hig)
        LEH = pool.tile([RB, W], bf16)
        nv.tensor_scalar(out=LEH[:], in0=T1[:], scalar1=float(hig),
                         scalar2=None, op0=ALU.is_le)
        K1 = pool.tile([RB, W], bf16)
        nv.tensor_scalar(out=K1[:], in0=T1[:], scalar1=thmax,
                         scalar2=None, op0=ALU.is_ge)
        MOK = pool.tile([RB, W], bf16)
        nv.tensor_scalar(out=MOK[:], in0=NGS[:], scalar1=1.5, scalar2=None,
                         op0=ALU.is_lt)
        nv.tensor_tensor(out=MOK[:], in0=MOK[:], in1=LEH[:], op=ALU.max)
        KEEP = pool.tile([RB, W], bf16)
        nv.tensor_tensor(out=KEEP[:], in0=K1[:], in1=MOK[:], op=ALU.mult)
        # threshold map: t1 where kept else 2.0 (cam < 1, so M == 0 there).
        # Kept pixels have margin >= 0.3 -> no tie at the max -> M is one-hot.
        # KEEP is exactly 0/1 so this select-by-arithmetic is exact in bf16.
        T1K = pool.tile([RB, W], bf16)
        nv.tensor_tensor(out=T1K[:], in0=T1[:], in1=KEEP[:], op=ALU.mult)
        NK2 = pool.tile([RB, W], bf16)
        nv.tensor_scalar(out=NK2[:], in0=KEEP[:], scalar1=-2.0,
                         scalar2=2.0, op0=ALU.mult, op1=ALU.add)
        T1X = pool.tile([RB, W], bf16)
        nv.tensor_tensor(out=T1X[:], in0=T1K[:], in1=NK2[:], op=ALU.add)

        # ---- M chunks + PE bilinear downsample (exact: weights are k/32) ----
        M = pool.tile([RB, CP * W], bf16)
        M_cw = M[:].rearrange("p (c w) -> p c w", w=W)
        Yps = [psum.tile([RB, n * 4 * 28], f32, name=f"yps{g}")
               for g, (c0, n) in enumerate(grps)]
        Ysb = [pool.tile([RB, n * 4 * 28], f16, name=f"ysb{g}")
               for g, (c0, n) in enumerate(grps)]
        Aps = psum.tile([28, CP * 28], f32)
        # stage 1 groups back-to-back on PE; copies trail on Act/DVE; then
        # stage 2 groups (so PE never waits a copy mid-stream)
        for g, (c0, n) in enumerate(grps):
            T1X_b = T1X[:].unsqueeze(1).broadcast_to([RB, n, W])
            nv.tensor_tensor(out=M_cw[:, c0:c0 + n, :],
                             in0=V_cw[:, c0:c0 + n, :], in1=T1X_b,
                             op=ALU.is_ge)
            for cr in range(n):
                c = c0 + cr
                for u in range(4):
                    nc.tensor.matmul(
                        Yps[g][:, (cr * 4 + u) * 28:(cr * 4 + u + 1) * 28],
                        lhsT=M[:, c * W + u * RB:c * W + (u + 1) * RB],
                        rhs=WR[:], start=True, stop=True)
            if g % 2 == 0:
                ns.copy(Ysb[g][:], Yps[g][:])
            else:
                nv.tensor_copy(Ysb[g][:], Yps[g][:])
        for g, (c0, n) in enumerate(grps):
            for cr in range(n):
                c = c0 + cr
                for u in range(4):
                    nc.tensor.matmul(
                        Aps[:, c * 28:(c + 1) * 28],
                        lhsT=Ysb[g][:, (cr * 4 + u) * 28:(cr * 4 + u + 1) * 28],
                        rhs=WC[:, u * 28:(u + 1) * 28],
                        start=(u == 0), stop=(u == 3))
        Asb = pool.tile([28, CP * 28], f32)
        ns.copy(Asb[:], Aps[:])
        nc.sync.dma_start(o_a, Asb[:])


    nc.compile()
    return nc


def _build_b_fast():
    nc = bacc.Bacc("TRN2", target_bir_lowering=False, debug=False, num_devices=1)
    P = B * C  # 40

    # aint layout: [pix%112, k*280 + u*40 + pair]  (A^T partials, block-major)
    aint = nc.dram_tensor("aint", [112, NBLK * 7 * P], f32,
                          kind="ExternalInput").ap()
    fmi = nc.dram_tensor("fmi", [112, 7 * B * D], f32, kind="ExternalInput").ap()
    smt = nc.dram_tensor("smt", [128, 226], f32, kind="ExternalInput").ap()

    o_loss = nc.dram_tensor("o_loss", [1, 1], f32, kind="ExternalOutput").ap()
    o_cnt = nc.dram_tensor("o_cnt", [1, P], f32, kind="ExternalOutput").ap()

    L5 = float(np.log(1e-5))
    L1M = float(np.log1p(-1e-5))
    LNLO = float(np.log(1e-5))
    LNHI = float(np.log1p(-1e-5))

    with tile.TileContext(nc) as tc, ExitStack() as ctx:
        pool = ctx.enter_context(tc.tile_pool(name="p", bufs=1))
        psum = ctx.enter_context(tc.tile_pool(name="ps", bufs=1, space="PSUM"))
        nv = nc.vector
        ns = nc.scalar

        AIN = pool.tile([112, NBLK * 7 * P], f32)
        nc.sync.dma_start(AIN[:], aint)
        SM = pool.tile([128, 226], f32)
        nc.sync.dma_start(SM[:], smt)
        FM = pool.tile([112, 7 * B * D], f32)
        for fc in range(4):
            c0, c1 = fc * 1024, min((fc + 1) * 1024, 7 * B * D)
            nc.sync.dma_start(FM[:, c0:c1], fmi[:, c0:c1])
        PJT = SM[:, 0:40]            # [128, (dc,c2)] proj^T chunks
        ONES112 = SM[0:112, 40:41]
        EYE = SM[0:C, 41:61]
        LAB2 = SM[0:C, 61:63]
        BSEL = SM[0:P, 63:65]
        EYEBC = SM[0:P, 65:85]
        ONES20 = SM[0:C, 85:86]
        SH0 = SM[0:C, 86:126]
        SH1 = SM[0:C, 126:166]
        ONESM = SM[0:C, 166:186]
        ONES1R = SM[0:1, 186:226]    # [1, 40] ones

        # ---- early independent: n_b, 1/max(n_b,1), step-0 ccf constant ----
        NSps = psum.tile([1, 2], f32, name="nsps")
        nc.tensor.matmul(NSps[:], lhsT=ONES20, rhs=LAB2, start=True, stop=True)
        NS = pool.tile([1, 2], f32)
        nv.tensor_copy(NS[:], NSps[:])
        DN = pool.tile([1, 2], f32)
        nv.tensor_scalar(out=DN[:], in0=NS[:], scalar1=1.0, scalar2=None,
                         op0=ALU.max)
        RDN = pool.tile([1, 2], f32)
        nv.reciprocal(RDN[:], DN[:])
        CCF = pool.tile([1, 1], f32)
        nv.tensor_scalar(out=CCF[:], in0=NS[:, 0:1],
                         scalar1=-(L5 - L1M) / (C * C), scalar2=-L1M,
                         op0=ALU.mult, op1=ALU.add)

        # ---- raw coef^T = sum over 4 row-blocks (tree); scale LR rides later --
        H1 = pool.tile([112, 2 * 7 * P], f32)
        nv.tensor_tensor(out=H1[:], in0=AIN[:, :2 * 7 * P],
                         in1=AIN[:, 2 * 7 * P:], op=ALU.add)
        CTR = pool.tile([112, 7 * P], f32)
        nv.tensor_tensor(out=CTR[:], in0=H1[:, :7 * P], in1=H1[:, 7 * P:],
                         op=ALU.add)

        # ---- counts -> LR = label/max(cnt,1) as a [40,1] column via PE ----
        CNTps = psum.tile([1, 7 * P], f32, name="cntps")
        nc.tensor.matmul(CNTps[:], lhsT=ONES112, rhs=CTR[:], start=True,
                         stop=True)
        CNTR = pool.tile([1, 7 * P], f32)
        nv.tensor_copy(CNTR[:], CNTps[:])
        CNT = _emit_tree(nc, pool, CNTR, 7, P, ALU.add, "cnt", dt=f32, part=1)
        nc.sync.dma_start(o_cnt, CNT[:])
        DENR = pool.tile([1, P], f32)
        nv.tensor_scalar(out=DENR[:], in0=CNT[:], scalar1=1.0, scalar2=None,
                         op0=ALU.max)
        RDR = pool.tile([1, P], f32)
        nv.reciprocal(RDR[:], DENR[:])
        LRps = psum.tile([P, P], f32, name="lrps")
        nc.tensor.matmul(LRps[:], lhsT=RDR[:], rhs=ONES1R, start=True, stop=True)
        LR40 = pool.tile([P, 1], f32)
        nv.tensor_copy(LR40[:], LRps[:, 0:1])

        # ---- fsm^T (raw scale) = fmap_ds^T @ coef_raw^T ----
        FTps = [psum.tile([128, P], f32, name=f"ftps{dc}", tag="ftps", bufs=2)
                for dc in range(2)]
        for dc in range(2):
            for b2 in range(B):
                for u in range(7):
                    nc.tensor.matmul(
                        FTps[dc][:, b2 * C:(b2 + 1) * C],
                        lhsT=FM[:, u * (B * D) + b2 * D + dc * 128:
                                u * (B * D) + b2 * D + (dc + 1) * 128],
                        rhs=CTR[:, u * P + b2 * C:u * P + (b2 + 1) * C],
                        start=(u == 0), stop=(u == 6))
        FSMT = pool.tile([128, 2 * P], f32)   # [d, (dc, b, c)]
        nv.tensor_copy(FSMT[:, 0:P], FTps[0][:])
        nv.tensor_copy(FSMT[:, P:2 * P], FTps[1][:])

        # ---- Gram diagonals (raw norms) + raw cos dot + logits ----
        SMLps = psum.tile([C, 4 * C], f32, name="smlps")
        for b2 in range(B):
            for dc in range(2):
                nc.tensor.matmul(
                    SMLps[:, b2 * C:(b2 + 1) * C],
                    lhsT=FSMT[:, dc * P + b2 * C:dc * P + (b2 + 1) * C],
                    rhs=FSMT[:, dc * P + b2 * C:dc * P + (b2 + 1) * C],
                    start=(dc == 0), stop=(dc == 1))
        RAWps = SMLps[:, 2 * C:3 * C]
        for dc in range(2):
            nc.tensor.matmul(RAWps,
                             lhsT=FSMT[:, dc * P + C:(dc + 1) * P],
                             rhs=FSMT[:, dc * P:dc * P + C],
                             start=(dc == 0), stop=(dc == 1))
        MMps = psum.tile([P, C], f32, name="mmps")
        LOGps = MMps[:, 0:C]
        for dc in range(2):
            nc.tensor.matmul(LOGps, lhsT=FSMT[:, dc * P:(dc + 1) * P],
                             rhs=PJT[:, dc * C:(dc + 1) * C],
                             start=(dc == 0), stop=(dc == 1))

        SCRD = pool.tile([C, C], f32)
        NRM2B = pool.tile([C, 2], f32)
        for b2 in range(B):
            nv.tensor_tensor(out=SCRD[:], in0=SMLps[:, b2 * C:(b2 + 1) * C],
                             in1=EYE, op=ALU.mult)
            nv.tensor_reduce(out=NRM2B[:, b2:b2 + 1], in_=SCRD[:], axis=AX.X,
                             op=ALU.add)
        NRM2G = pool.tile([C, 2], f32)
        nv.tensor_scalar(out=NRM2G[:], in0=NRM2B[:], scalar1=1e-24, scalar2=None,
                         op0=ALU.max)
        LN2 = pool.tile([C, 2], f32)
        ns.activation(LN2[:], NRM2G[:], AFT.Ln)

        # ---- cos in log space: ln|dot| - ln||a|| - ln||b|| (+absent -> -50) --
        hp = ExitStack()
        hp.enter_context(tc.high_priority())
        ABSR = pool.tile([C, C], f32)
        nv.tensor_scalar(out=ABSR[:], in0=RAWps, scalar1=-1.0, scalar2=None,
                         op0=ALU.mult)
        nv.tensor_tensor(out=ABSR[:], in0=ABSR[:], in1=RAWps, op=ALU.max)
        nv.tensor_scalar(out=ABSR[:], in0=ABSR[:], scalar1=1e-30, scalar2=None,
                         op0=ALU.max)
        LNC = pool.tile([C, C], f32)
        ns.activation(LNC[:], ABSR[:], AFT.Ln)
        # row term: -0.5*ln n1_c ; column term via PE: -0.5*ln n0_j - 50*(1-p0_j)
        COLV = pool.tile([C, 1], f32)
        nv.tensor_scalar(out=COLV[:], in0=LAB2[:, 0:1], scalar1=50.0,
                         scalar2=-50.0, op0=ALU.mult, op1=ALU.add)
        nv.scalar_tensor_tensor(out=COLV[:], in0=LN2[:, 0:1], scalar=-0.5,
                                in1=COLV[:], op0=ALU.mult, op1=ALU.add)
        DIAGC = pool.tile([C, C], f32)
        nv.tensor_scalar(out=DIAGC[:], in0=EYE, scalar1=COLV[:], scalar2=None,
                         op0=ALU.mult)
        CSMps = SMLps[:, 3 * C:4 * C]
        nc.tensor.matmul(CSMps, lhsT=ONESM, rhs=DIAGC[:], start=True, stop=True)
        RV = pool.tile([C, 1], f32)
        nv.tensor_scalar(out=RV[:], in0=LN2[:, 1:2], scalar1=-0.5, scalar2=None,
                         op0=ALU.mult)
        nv.tensor_scalar(out=LNC[:], in0=LNC[:], scalar1=RV[:], scalar2=None,
                         op0=ALU.add)
        nv.tensor_tensor(out=LNC[:], in0=LNC[:], in1=CSMps, op=ALU.add)
        nv.tensor_scalar(out=LNC[:], in0=LNC[:], scalar1=LNLO, scalar2=LNHI,
                         op0=ALU.max, op1=ALU.min)
        COSC = pool.tile([C, C], f32)
        ns.activation(COSC[:], LNC[:], AFT.Exp)
        hp.close()

        # ---- softmax-BCE per (b,c) row (logits get the true LR scale) ----
        LOG = pool.tile([P, C], f32)
        nv.tensor_scalar(out=LOG[:], in0=LOGps, scalar1=LR40[:], scalar2=None,
                         op0=ALU.mult)
        MX = pool.tile([P, 1], f32)
        nv.tensor_reduce(out=MX[:], in_=LOG[:], axis=AX.X, op=ALU.max)
        XT = pool.tile([P, C], f32)
        nv.tensor_scalar(out=XT[:], in0=LOG[:], scalar1=MX[:], scalar2=None,
                         op0=ALU.subtract)
        ET = pool.tile([P, C], f32)
        SMR = pool.tile([P, 1], f32)
        ns.activation(ET[:], XT[:], AFT.Exp, accum_out=SMR[:])
        LGS = pool.tile([P, 1], f32)
        ns.activation(LGS[:], SMR[:], AFT.Ln)
        LGP = pool.tile([P, C], f32)
        nv.tensor_scalar(out=LGP[:], in0=XT[:], scalar1=LGS[:], scalar2=-100.0,
                         op0=ALU.subtract, op1=ALU.max)
        SME = pool.tile([P, C], f32)
        nv.scalar_tensor_tensor(out=SME[:], in0=ET[:], scalar=-1.0,
                                in1=SMR[:].broadcast_to([P, C]),
                                op0=ALU.mult, op1=ALU.add)
        LSME = pool.tile([P, C], f32)
        ns.activation(LSME[:], SME[:], AFT.Ln)
        L1P = pool.tile([P, C], f32)
        nv.tensor_scalar(out=L1P[:], in0=LSME[:], scalar1=LGS[:], scalar2=-100.0,
                         op0=ALU.subtract, op1=ALU.max)
        DD = pool.tile([P, C], f32)
        nv.tensor_tensor(out=DD[:], in0=LGP[:], in1=L1P[:], op=ALU.subtract)
        SCRP = pool.tile([P, C], f32)
        nv.tensor_tensor(out=SCRP[:], in0=DD[:], in1=EYEBC, op=ALU.mult)
        DDG = pool.tile([P, 1], f32)
        nv.tensor_reduce(out=DDG[:], in_=SCRP[:], axis=AX.X, op=ALU.add)
        RSM = pool.tile([P, 1], f32)
        nv.tensor_reduce(out=RSM[:], in_=L1P[:], axis=AX.X, op=ALU.add)
        TERM = pool.tile([P, 1], f32)
        nv.tensor_tensor(out=TERM[:], in0=DDG[:], in1=RSM[:], op=ALU.add)
        nv.tensor_scalar(out=TERM[:], in0=TERM[:], scalar1=-1.0 / C, scalar2=None,
                         op0=ALU.mult)

        # ---- qualify: b0 = present_0; b1 = present_1 & offdiag max < 0.6 ----
        COSM = pool.tile([C, C], f32)
        nv.scalar_tensor_tensor(out=COSM[:], in0=EYE, scalar=-1e9, in1=COSC[:],
                                op0=ALU.mult, op1=ALU.add)
        OFF = pool.tile([C, 1], f32)
        nv.tensor_reduce(out=OFF[:], in_=COSM[:], axis=AX.X, op=ALU.max)
        QB1 = pool.tile([C, 1], f32)
        nv.tensor_scalar(out=QB1[:], in0=OFF[:], scalar1=0.6, scalar2=None,
                         op0=ALU.is_lt)
        nv.tensor_tensor(out=QB1[:], in0=QB1[:], in1=LAB2[:, 1:2], op=ALU.mult)
        QRps = psum.tile([P, 6], f32, name="qrps")
        Q40ps = QRps[:, 0:1]
        nc.tensor.matmul(Q40ps, lhsT=SH0, rhs=LAB2[:, 0:1], start=True,
                         stop=False)
        nc.tensor.matmul(Q40ps, lhsT=SH1, rhs=QB1[:], start=False, stop=True)
        CONTR = pool.tile([P, 1], f32)
        nv.tensor_tensor(out=CONTR[:], in0=TERM[:], in1=Q40ps, op=ALU.mult)

        # ---- ccf step 1 (LGC == clipped LNC already) ----
        OM = pool.tile([C, C], f32)
        nv.tensor_scalar(out=OM[:], in0=COSC[:], scalar1=-1.0, scalar2=1.0,
                         op0=ALU.mult, op1=ALU.add)
        LOM = pool.tile([C, C], f32)
        R1 = pool.tile([C, 1], f32)
        ns.activation(LOM[:], OM[:], AFT.Ln, accum_out=R1[:])
        DIF = pool.tile([C, C], f32)
        nv.tensor_tensor(out=DIF[:], in0=LNC[:], in1=LOM[:], op=ALU.subtract)
        nv.tensor_scalar(out=DIF[:], in0=DIF[:], scalar1=LAB2[:, 1:2],
                         scalar2=None, op0=ALU.mult)
        CC1 = pool.tile([C, 1], f32)
        nv.tensor_tensor(out=SCRD[:], in0=DIF[:], in1=EYE, op=ALU.mult)
        nv.tensor_reduce(out=CC1[:], in_=SCRD[:], axis=AX.X, op=ALU.add)
        nv.tensor_tensor(out=CC1[:], in0=CC1[:], in1=R1[:], op=ALU.add)

        # ---- partition sums via PE, final scalar chain ----
        REDps = QRps[0:1, 1:6]
        nc.tensor.matmul(REDps[:, 2:4], lhsT=CONTR[:], rhs=BSEL, start=True,
                         stop=True)
        nc.tensor.matmul(REDps[:, 4:5], lhsT=ONES20, rhs=CC1[:], start=True,
                         stop=True)
        RED = pool.tile([1, 5], f32)
        nv.tensor_copy(RED[:, 2:5], REDps[:, 2:5])
        LC = pool.tile([1, 1], f32)
        nv.tensor_tensor(out=LC[:], in0=RED[:, 2:3], in1=RDN[:, 0:1], op=ALU.mult)
        nv.tensor_tensor(out=LC[:], in0=LC[:], in1=RED[:, 3:4], op=ALU.add)
        nv.tensor_tensor(out=LC[:], in0=LC[:], in1=RDN[:, 1:2], op=ALU.mult)
        CC1S = pool.tile([1, 1], f32)
        nv.tensor_scalar(out=CC1S[:], in0=RED[:, 4:5], scalar1=-1.0 / (C * C),
                         scalar2=None, op0=ALU.mult)
        OUT = pool.tile([1, 1], f32)
        nv.tensor_tensor(out=OUT[:], in0=LC[:], in1=CCF[:], op=ALU.add)
        nv.tensor_tensor(out=OUT[:], in0=OUT[:], in1=CC1S[:], op=ALU.add)
        nc.sync.dma_start(o_loss, OUT[:])

    nc.compile()
    return nc


def _marshal_a_fast(cam, CP, idxs):
    bf = mybir.dt.np(bf16)
    fh = mybir.dt.np(f16)
    in_maps = []
    for core in range(8):
        b, blk = core // NBLK, core % NBLK
        idx = idxs[b]
        camv = np.zeros((CP, NPIX), bf)
        if len(idx):
            camv[:len(idx)] = cam[b, idx, blk * RB:(blk + 1) * RB, :].reshape(
                len(idx), NPIX).astype(bf)
        wct = np.ascontiguousarray(
            W1D.reshape(4, RB, 28).transpose(1, 0, 2).reshape(RB, 4 * 28))
        in_maps.append({
            "camv": camv,
            "wrt": np.ascontiguousarray(W1D[blk * RB:(blk + 1) * RB, :]).astype(bf),
            "wct": wct.astype(fh),
        })
    return in_maps


def _marshal_b_fast(res_a, fmap, cls_label, proj_weight, CP, idxs):
    P = B * C
    a8 = np.stack([res_a[k]["o_a"] for k in range(8)])          # [8, 28, CP*28]
    a8 = a8.reshape(B, NBLK, 28, CP, 28)
    afull = np.zeros((P, NBLK, 784), np.float32)
    for b in range(B):
        idx = idxs[b]
        if len(idx):
            # [blk, 28i, slot, 28j] -> [slot, blk, (i,j)]
            afull[b * C + idx] = a8[b, :, :, :len(idx), :].transpose(
                2, 0, 1, 3).reshape(len(idx), NBLK, 784)
    # aint[p, k*280 + u*40 + pair] = afull[pair, k, u*112 + p]
    aint = np.ascontiguousarray(
        afull.reshape(P, NBLK, 7, 112).transpose(3, 1, 2, 0)
    ).reshape(112, NBLK * 7 * P)

    fm = np.asarray(fmap, np.float32).reshape(B, D, 784)
    # fmi[p, u*512 + b*256 + d] = fmap[b, d, u*112 + p]
    fmi = np.ascontiguousarray(
        fm.transpose(2, 0, 1).reshape(7, 112, B, D).transpose(1, 0, 2, 3)
    ).reshape(112, 7 * B * D)

    lab = np.asarray(cls_label, np.float32)
    smt = np.zeros((128, 226), np.float32)
    smt[:, 0:40] = np.ascontiguousarray(
        np.asarray(proj_weight, np.float32).T
    ).reshape(2, 128, C).transpose(1, 0, 2).reshape(128, 2 * C)
    smt[0:112, 40:41] = 1.0
    smt[0:C, 41:61] = np.eye(C, dtype=np.float32)
    smt[0:C, 61:63] = lab.T
    smt[0:P, 63:65] = (np.arange(P)[:, None] // C ==
                       np.arange(2)[None, :]).astype(np.float32)
    smt[0:P, 65:85] = np.tile(np.eye(C, dtype=np.float32), (B, 1))
    smt[0:C, 85:86] = 1.0
    smt[0:C, 86:126] = np.eye(C, P, dtype=np.float32)
    smt[0:C, 126:166] = np.eye(C, P, k=C, dtype=np.float32)
    smt[0:C, 166:186] = 1.0
    smt[0:1, 186:226] = 1.0
    return {"aint": aint, "fmi": fmi, "smt": smt}


# --------------------------------------------------------------------------
# Host marshaling + driver
# --------------------------------------------------------------------------

_CACHE = {}


def _get_programs(hig, low, bg, CP):
    stage = int(os.environ.get("BASSK_B_STAGE", "99"))
    key = ("slow", float(hig), float(low), float(bg), stage, CP)
    if key not in _CACHE:
        _CACHE[key] = (_build_a(hig, low, bg, CP), _build_b(stage))
    return _CACHE[key]


def _get_programs_fast(hig, low, bg, CP):
    key = (float(hig), float(low), float(bg), CP)
    if key not in _CACHE:
        _CACHE[key] = (_build_a_fast(hig, low, bg, CP), _build_b_fast())
    return _CACHE[key]


def _marshal_a(cam, cls_label, CP, idxs):
    eye128 = np.eye(128, dtype=np.float32)
    clst = np.tile((np.arange(CP, dtype=np.float32) + 1.0)[None, :], (RB, 1))
    iodt = np.tile((float(CP) - np.arange(CP, dtype=np.float32))[None, :], (RB, 1))
    wct = np.ascontiguousarray(
        W1D.reshape(4, RB, 28).transpose(1, 0, 2).reshape(RB, 4 * 28))
    in_maps = []
    for core in range(8):
        b, blk = core // NBLK, core % NBLK
        idx = idxs[b]
        camv = np.zeros((CP, NPIX), np.float32)
        if len(idx):
            camv[:len(idx)] = cam[b, idx, blk * RB:(blk + 1) * RB, :].reshape(
                len(idx), NPIX)
        labt = np.tile((np.arange(CP) < len(idx)).astype(np.float32)[None, :],
                       (RB, 1))
        in_maps.append({
            "camv": camv,
            "labt": labt,
            "clst": clst,
            "iodt": iodt,
            "wrt": np.ascontiguousarray(W1D[blk * RB:(blk + 1) * RB, :]),
            "wct": wct,
            "idn": eye128,
        })
    return in_maps


def _marshal_b(res_a, fmap, cls_label, proj_weight, feature_contrast, CP, idxs):
    P = B * C
    ntk = (CP + 7) // 8
    # scatter packed per-slot A partials back to global classes
    a8 = np.stack([res_a[k]["o_a"] for k in range(8)])          # [8, 28, CP*28]
    a8 = a8.reshape(B, NBLK, 28, CP, 28)
    afull = np.zeros((B, C, 28, 28, NBLK), np.float32)
    for b in range(B):
        idx = idxs[b]
        if len(idx):
            # [blk, 28, slot, 28] -> [slot, 28, 28, blk]
            afull[b, idx] = a8[b, :, :, :len(idx), :].transpose(2, 1, 3, 0)
    ain = np.ascontiguousarray(afull).reshape(P, 784 * NBLK)

    cand_v = np.zeros((P, NBLK * NCAND), np.float32)
    cand_i = np.zeros((P, NBLK * NCAND), np.uint32)
    for core in range(8):
        b, blk = core // NBLK, core % NBLK
        tks = [res_a[core][f"o_tk{t}"] for t in range(ntk)]
        for j, c in enumerate(idxs[b]):
            tk = tks[j // 8]
            rb = (j % 8) * 16
            vals = np.concatenate([tk[rb + 14, 0:16], tk[rb + 15, 0:16]])
            gidx = np.concatenate([tk[rb + 14, 16:32], tk[rb + 15, 16:32]])
            cand_v[b * C + c, blk * NCAND:(blk + 1) * NCAND] = vals.view(np.float32)
            cand_i[b * C + c, blk * NCAND:(blk + 1) * NCAND] = gidx

    bbs = np.zeros((P, NBLK * NCAND), np.float32)
    for blk in range(NBLK):
        bbs[:, blk * NCAND:(blk + 1) * NCAND] = blk * RB * W

    # pre-transposed fmap: fmt[sp, u*(B*D) + b*D + d] = fmap[b, d, u*112+sp]
    fm = np.asarray(fmap, np.float32).reshape(B, D, 7, 112)
    fmi = np.ascontiguousarray(fm.transpose(3, 2, 0, 1)).reshape(112, 7 * B * D)

    rnk = np.zeros((P, NCAND), np.float32)
    rnk[:, :K_TOP] = 1.0 / K_TOP

    return {
        "ain": ain,
        "cdv": cand_v,
        "cdi": cand_i,
        "bbs": bbs,
        "fmi": fmi,
        "prj": np.ascontiguousarray(
            np.asarray(proj_weight, np.float32).T.reshape(2, 128, C)
            .transpose(1, 0, 2)).reshape(128, 2 * C),
        "lab": np.asarray(cls_label, np.float32).reshape(P, 1),
        "lab2": np.ascontiguousarray(np.asarray(cls_label, np.float32).T),
        "fc0": np.asarray(feature_contrast, np.float32),
        "eye": np.eye(C, dtype=np.float32),
        "i28": np.tile(np.arange(28, dtype=np.float32)[None, :], (128, 1)),
        "i128": np.tile(np.arange(128, dtype=np.float32)[None, :], (P, 1)),
        "mmb": (np.arange(128)[:, None] // NCAND ==
                np.arange(76)[None, :] - 36).astype(np.float32),
        "rnk": rnk,
        "idn": np.eye(128, dtype=np.float32),
    }


LAST_EXEC_NS = {}


def _run(nc, in_maps, core_ids, tag="k"):
    if os.environ.get("BASSK_SIM") == "1":
        from concourse.bass_interp import CoreSim, MultiCoreSim
        if len(core_ids) == 1:
            sim = CoreSim(nc, trace=False, require_finite=False)
            sims = [sim]
        else:
            msim = MultiCoreSim(nc, num_cores=len(core_ids), trace=False,
                                require_finite=False)
            sims = [msim.cores[i] for i in core_ids]
            sim = msim
        for s, m in zip(sims, in_maps):
            for name, arr in m.items():
                s.tensor(name)[:] = arr
        sim.simulate(check_with_hw=False)
        outs = []
        for s in sims:
            d = {}
            for alloc in nc.m.functions[0].allocations:
                if getattr(alloc, "kind", None) == "ExternalOutput":
                    nm = alloc.memorylocations[0].name
                    d[nm] = np.array(s.tensor(nm))
            outs.append(d)
        return outs
    trace = os.environ.get("BASSK_TRACE") == "1"
    if trace:
        try:
            from antenv.axon_hooks import get_axon_ntff_profile_hook  # noqa: F401
        except Exception:
            trace = False
    res = run_bass_kernel_spmd(nc, in_maps, core_ids, trace=trace)
    if res.exec_time_ns is not None:
        LAST_EXEC_NS[tag] = res.exec_time_ns
    return res.results


def _kernel_slow(fmap, cam, cls_label, proj_weight, feature_contrast,
                 hig_thre, low_thre, bg_thre, idxs, CP):
    nca, ncb = _get_programs(float(hig_thre), float(low_thre), float(bg_thre), CP)
    res_a = _run(nca, _marshal_a(cam, cls_label, CP, idxs), list(range(8)), tag="A")
    in_b = _marshal_b(res_a, fmap, cls_label, proj_weight, feature_contrast, CP, idxs)
    res_b = _run(ncb, [in_b], [0], tag="B")
    return np.float32(res_b[0]["o_loss"].reshape(-1)[0])


def kernel(fmap, cam, cls_label, proj_weight, feature_contrast,
           hig_thre, low_thre, bg_thre):
    fmap = np.asarray(fmap, np.float32)
    cam = np.asarray(cam, np.float32)
    lab = np.asarray(cls_label, np.float32)
    idxs = [np.where(lab[b] > 0.5)[0] for b in range(B)]
    cp_act = max((len(i) for i in idxs), default=0)
    CP = min(C, max(4, ((cp_act + 3) // 4) * 4))

    fc_zero = not np.any(np.asarray(feature_contrast, np.float32))
    if fc_zero and os.environ.get("BASSK_FORCE_SLOW") != "1":
        CPF = max(1, cp_act)
        nca, ncb = _get_programs_fast(float(hig_thre), float(low_thre),
                                      float(bg_thre), CPF)
        res_a = _run(nca, _marshal_a_fast(cam, CPF, idxs), list(range(8)), tag="A")
        in_b = _marshal_b_fast(res_a, fmap, cls_label, proj_weight, CPF, idxs)
        res_b = _run(ncb, [in_b], [0], tag="B")
        cnt = res_b[0]["o_cnt"].reshape(B, C)
        # fast path assumed every present class has masked pixels; verify.
        if not np.any((lab > 0.5) & (cnt < 0.5)):
            loss = np.float32(res_b[0]["o_loss"].reshape(-1)[0])
            return np.asarray(loss, dtype=np.float32).reshape(())
    loss = _kernel_slow(fmap, cam, cls_label, proj_weight, feature_contrast,
                        hig_thre, low_thre, bg_thre, idxs, CP)
    return np.asarray(loss, dtype=np.float32).reshape(())



# revision 30
# speedup vs baseline: 1.0621x; 1.0375x over previous
"""Trainium2 Bass kernel for nn_CPCLoss (self-contained).

Strategy (8 NeuronCores, full inputs in / full output out):
  NEFF-A, SPMD on 8 cores — core k = (batch b=k//4, row-block blk=k%4 of 112
  dst rows). Each core reads its cam shard [20, 112, 448] and computes:
    * per-pixel top1/second/argmax over classes -> pseudo-label class map
    * A_partial[c] = Wr_blk^T @ onehot(q==c+1) @ Wc  (28x28 bilinear-downsample
      coefficient grid per class; Wr/Wc are the static jax.image.resize
      bilinear matrices) via PE matmuls
    * exact per-class top-256 (values+indices) over the 50176 shard pixels via
      the gpsimd topk instruction; top-32 shipped as merge candidates
  Host only reshapes/concats partials (no arithmetic).
  NEFF-B, 1 core — sums partials, merges exact top-25 per (b,c), builds the
  bilinear gather matrix G, selects coef = count==0 ? G/25 : A/max(count,1),
  fsm = coef @ fmap^T, then runs the 2-step EMA memory-bank scan and emits the
  scalar loss.
"""
import os
import sys

os.environ.setdefault("MYCRO_LOCAL_CACHE", "1")
if "/opt/trn_rl_repo" not in sys.path:
    sys.path.insert(0, "/opt/trn_rl_repo")

from contextlib import ExitStack

import numpy as np

from concourse import bacc, bass_isa, mybir, tile
from concourse.bass_utils import run_bass_kernel_spmd


class _StageDone(Exception):
    pass

f32 = mybir.dt.float32
u32 = mybir.dt.uint32
ALU = mybir.AluOpType
AFT = mybir.ActivationFunctionType
AX = mybir.AxisListType

B, C, D = 2, 20, 256
H = W = 448
FH = FW = 28
K_TOP = 25
NBLK = 4
RB = H // NBLK            # 112
NPIX = RB * W             # 50176
NCAND = 32                # candidates shipped per (core, class)
MARGIN = 0.3


def _make_w1d():
    scale = FH / H
    w = np.zeros((H, FH), dtype=np.float64)
    for x in range(H):
        s = (x + 0.5) * scale - 0.5
        i0 = int(np.floor(s))
        f = s - i0
        for i, wt in ((i0, 1.0 - f), (i0 + 1, f)):
            if 0 <= i < FH:
                w[x, i] += wt
        w[x] /= w[x].sum()
    return w.astype(np.float32)


W1D = _make_w1d()


def _emit_topk(nc, out_ap, in_ap, tokens):
    g = nc.gpsimd
    return g.add_instruction(bass_isa.InstTopk(
        name=f"I-{nc.next_id()}",
        ins=[g.lower_ap(in_ap, for_isa=True)],
        outs=[g.lower_ap(out_ap, for_isa=True)],
        _tokens=tokens, _n=NPIX, _k=256))


# --------------------------------------------------------------------------
# NEFF-A
# --------------------------------------------------------------------------

def _build_a(hig, low, bg, CP=C):
    nc = bacc.Bacc("TRN2", target_bir_lowering=False, debug=False, num_devices=8)

    camv = nc.dram_tensor("camv", [CP, NPIX], f32, kind="ExternalInput").ap()
    labt = nc.dram_tensor("labt", [RB, CP], f32, kind="ExternalInput").ap()
    clst = nc.dram_tensor("clst", [RB, CP], f32, kind="ExternalInput").ap()
    iodt = nc.dram_tensor("iodt", [RB, CP], f32, kind="ExternalInput").ap()
    wrt = nc.dram_tensor("wrt", [RB, 28], f32, kind="ExternalInput").ap()
    wct = nc.dram_tensor("wct", [RB, 4 * 28], f32, kind="ExternalInput").ap()
    idn = nc.dram_tensor("idn", [128, 128], f32, kind="ExternalInput").ap()

    o_a = nc.dram_tensor("o_a", [28, CP * 28], f32, kind="ExternalOutput").ap()
    ntk = (CP + 7) // 8
    tok = [min(8, CP - 8 * t) for t in range(ntk)]
    o_tk = [nc.dram_tensor(f"o_tk{t}", [16 * tok[t], 32], u32,
                           kind="ExternalOutput").ap() for t in range(ntk)]

    thmax = float(max(hig, low, bg))

    with tile.TileContext(nc) as tc, ExitStack() as ctx:
        pool = ctx.enter_context(tc.tile_pool(name="p", bufs=1))
        psum = ctx.enter_context(tc.tile_pool(name="ps", bufs=1, space="PSUM"))
        nv = nc.vector

        VP = pool.tile([RB, CP * W], f32)
        nc.sync.dma_start(VP[:], camv.rearrange("c (r w) -> r c w", w=W))
        VT = []
        for t in range(ntk):
            vt = pool.tile([16 * tok[t], NPIX // 16], f32, name=f"VT{t}")
            nc.sync.dma_start(vt[:], camv[8 * t:8 * t + tok[t]]
                              .rearrange("c (g f) -> (c g) f", f=NPIX // 16))
            VT.append(vt)

        LB = pool.tile([RB, CP], f32); nc.sync.dma_start(LB[:], labt)
        CL = pool.tile([RB, CP], f32); nc.sync.dma_start(CL[:], clst)
        IO = pool.tile([RB, CP], f32); nc.sync.dma_start(IO[:], iodt)
        WR = pool.tile([RB, 28], f32); nc.sync.dma_start(WR[:], wrt)
        WC = pool.tile([RB, 4 * 28], f32); nc.sync.dma_start(WC[:], wct)
        IDN = pool.tile([128, 128], f32); nc.sync.dma_start(IDN[:], idn)

        # ---- pseudo-label phase ----
        V_cw = VP[:].rearrange("p (c w) -> p c w", w=W)
        V_wc = VP[:].rearrange("p (c w) -> p w c", w=W)
        LB_b = LB[:].unsqueeze(2).broadcast_to([RB, CP, W])
        nv.tensor_tensor(out=V_cw, in0=V_cw, in1=LB_b, op=ALU.mult)  # valid in-place

        T1 = pool.tile([RB, W], f32)
        nv.tensor_reduce(out=T1[:], in_=V_wc, axis=AX.X, op=ALU.max)

        GE = pool.tile([RB, CP * W], f32)
        GE_cw = GE[:].rearrange("p (c w) -> p c w", w=W)
        T1_b = T1[:].unsqueeze(1).broadcast_to([RB, CP, W])
        nv.tensor_tensor(out=GE_cw, in0=V_cw, in1=T1_b, op=ALU.is_ge)

        EN = pool.tile([RB, CP * W], f32, tag="scr")
        EN_cw = EN[:].rearrange("p (c w) -> p c w", w=W)
        IO_b = IO[:].unsqueeze(2).broadcast_to([RB, CP, W])
        nv.tensor_tensor(out=EN_cw, in0=GE_cw, in1=IO_b, op=ALU.mult)
        AM = pool.tile([RB, W], f32)
        nv.tensor_reduce(out=AM[:], in_=EN[:].rearrange("p (c w) -> p w c", w=W),
                         axis=AX.X, op=ALU.max)

        MK = pool.tile([RB, CP * W], f32, tag="scr")
        MK_cw = MK[:].rearrange("p (c w) -> p c w", w=W)
        nv.scalar_tensor_tensor(out=MK_cw, in0=GE_cw, scalar=-1e9, in1=V_cw,
                                op0=ALU.mult, op1=ALU.add)
        SC = pool.tile([RB, W], f32)
        nv.tensor_reduce(out=SC[:], in_=MK[:].rearrange("p (c w) -> p w c", w=W),
                         axis=AX.X, op=ALU.max)

        # keep iff top1 >= max(hig,low,bg) and (margin >= 0.3 or top1 <= hig)
        KG = pool.tile([RB, W], f32)
        nv.tensor_scalar(out=KG[:], in0=T1[:], scalar1=thmax, scalar2=None, op0=ALU.is_ge)
        MGOK = pool.tile([RB, W], f32)
        nv.tensor_tensor(out=MGOK[:], in0=T1[:], in1=SC[:], op=ALU.subtract)
        nv.tensor_scalar(out=MGOK[:], in0=MGOK[:], scalar1=MARGIN, scalar2=None, op0=ALU.is_ge)
        LEH = pool.tile([RB, W], f32)
        nv.tensor_scalar(out=LEH[:], in0=T1[:], scalar1=float(hig), scalar2=None, op0=ALU.is_le)
        nv.tensor_tensor(out=MGOK[:], in0=MGOK[:], in1=LEH[:], op=ALU.max)
        nv.tensor_tensor(out=KG[:], in0=KG[:], in1=MGOK[:], op=ALU.mult)
        Q = pool.tile([RB, W], f32)
        nv.tensor_scalar(out=Q[:], in0=AM[:], scalar1=-1.0, scalar2=float(CP + 1),
                         op0=ALU.mult, op1=ALU.add)
        nv.tensor_tensor(out=Q[:], in0=Q[:], in1=KG[:], op=ALU.mult)

        # ---- q transpose + one-hot EQT + matmuls for A ----
        QT = pool.tile([RB, 4 * RB], f32)
        for u in range(4):
            QTP = psum.tile([RB, RB], f32, tag="qtp")
            nc.tensor.transpose(QTP[:], Q[:, u * RB:(u + 1) * RB], IDN[:RB, :RB])
            nc.scalar.copy(QT[:, u * RB:(u + 1) * RB], QTP[:])

        EQT = pool.tile([RB, 4 * CP * RB], f32)
        for u in range(4):
            sl = EQT[:, u * CP * RB:(u + 1) * CP * RB]
            sl_cw = sl.rearrange("p (c r) -> p c r", r=RB)
            QT_b = QT[:, u * RB:(u + 1) * RB].unsqueeze(1).broadcast_to([RB, CP, RB])
            CL_b = CL[:].unsqueeze(2).broadcast_to([RB, CP, RB])
            nv.tensor_tensor(out=sl_cw, in0=QT_b, in1=CL_b, op=ALU.is_equal)
        # PSUM bank = 512 f32: hold 5 classes (140 cols) per bank-tile
        ngrp = (CP + 4) // 5
        T0sb = pool.tile([RB, CP * 28], f32)
        Asb = pool.tile([28, CP * 28], f32)
        T0ps = [psum.tile([RB, 5 * 28], f32, name=f"t0ps{i}", tag="accps", bufs=4)
                for i in range(ngrp)]
        Aps = [psum.tile([28, 5 * 28], f32, name=f"aps{i}", tag="accps", bufs=4)
               for i in range(ngrp)]
        for c in range(CP):
            grp, off = c // 5, (c % 5) * 28
            for u in range(4):
                nc.tensor.matmul(
                    T0ps[grp][:, off:off + 28],
                    lhsT=EQT[:, u * CP * RB + c * RB:u * CP * RB + (c + 1) * RB],
                    rhs=WC[:, u * 28:(u + 1) * 28],
                    start=(u == 0), stop=(u == 3))
        for i in range(ngrp):
            w0 = i * 140
            w1 = min(w0 + 140, CP * 28)
            nc.scalar.copy(T0sb[:, w0:w1], T0ps[i][:, 0:w1 - w0])
        for c in range(CP):
            grp, off = c // 5, (c % 5) * 28
            nc.tensor.matmul(Aps[grp][:, off:off + 28], lhsT=WR[:],
                             rhs=T0sb[:, c * 28:(c + 1) * 28], start=True, stop=True)
        for i in range(ngrp):
            w0 = i * 140
            w1 = min(w0 + 140, CP * 28)
            nc.scalar.copy(Asb[:, w0:w1], Aps[i][:, 0:w1 - w0])
        nc.sync.dma_start(o_a, Asb[:])

        # ---- per-class topk ----
        for t in range(ntk):
            tkt = pool.tile([16 * tok[t], 32], u32, name=f"TK{t}")
            _emit_topk(nc, tkt[:], VT[t][:], tokens=tok[t])
            nc.sync.dma_start(o_tk[t], tkt[:])

    nc.compile()
    return nc


# --------------------------------------------------------------------------
# NEFF-B
# --------------------------------------------------------------------------

def _build_b(stage=99):
    nc = bacc.Bacc("TRN2", target_bir_lowering=False, debug=False, num_devices=1)
    P = B * C  # 40 (b,c) pairs

    ain = nc.dram_tensor("ain", [P, 784 * NBLK], f32, kind="ExternalInput").ap()
    cdv = nc.dram_tensor("cdv", [P, NBLK * NCAND], f32, kind="ExternalInput").ap()
    cdi = nc.dram_tensor("cdi", [P, NBLK * NCAND], u32, kind="ExternalInput").ap()
    bbs = nc.dram_tensor("bbs", [P, NBLK * NCAND], f32, kind="ExternalInput").ap()
    fmi = nc.dram_tensor("fmi", [112, 7 * B * D], f32, kind="ExternalInput").ap()
    prj = nc.dram_tensor("prj", [128, 2 * C], f32, kind="ExternalInput").ap()
    lab = nc.dram_tensor("lab", [P, 1], f32, kind="ExternalInput").ap()
    lab2 = nc.dram_tensor("lab2", [C, B], f32, kind="ExternalInput").ap()
    fc0 = nc.dram_tensor("fc0", [C, D], f32, kind="ExternalInput").ap()
    eye = nc.dram_tensor("eye", [C, C], f32, kind="ExternalInput").ap()
    i28 = nc.dram_tensor("i28", [128, 28], f32, kind="ExternalInput").ap()
    i128 = nc.dram_tensor("i128", [P, 128], f32, kind="ExternalInput").ap()
    mmb = nc.dram_tensor("mmb", [128, 76], f32, kind="ExternalInput").ap()
    rnk = nc.dram_tensor("rnk", [P, NCAND], f32, kind="ExternalInput").ap()
    idn = nc.dram_tensor("idn", [128, 128], f32, kind="ExternalInput").ap()

    o_loss = nc.dram_tensor("o_loss", [1, 1], f32, kind="ExternalOutput").ap()
    o_dbg = nc.dram_tensor("o_dbg", [128, 1024], f32, kind="ExternalOutput").ap()

    NC128 = NBLK * NCAND  # 128 candidates per pair

    try:
      with tile.TileContext(nc) as tc, ExitStack() as ctx:
        pool = ctx.enter_context(tc.tile_pool(name="p", bufs=1))
        psum = ctx.enter_context(tc.tile_pool(name="ps", bufs=1, space="PSUM"))
        nv = nc.vector
        ns = nc.scalar

        AIN = pool.tile([P, 784 * NBLK], f32); nc.sync.dma_start(AIN[:], ain)
        CV = pool.tile([P, NC128], f32); nc.sync.dma_start(CV[:], cdv)
        CI = pool.tile([P, NC128], u32); nc.sync.dma_start(CI[:], cdi)
        BBS = pool.tile([P, NC128], f32); nc.sync.dma_start(BBS[:], bbs)
        FM = pool.tile([112, 7 * B * D], f32); nc.sync.dma_start(FM[:], fmi)
        PJT = pool.tile([128, 2 * C], f32); nc.sync.dma_start(PJT[:], prj)
        LAB = pool.tile([P, 1], f32); nc.sync.dma_start(LAB[:], lab)
        LAB2 = pool.tile([C, B], f32); nc.sync.dma_start(LAB2[:], lab2)
        FC = pool.tile([C, D], f32); nc.sync.dma_start(FC[:], fc0)
        EYE = pool.tile([C, C], f32); nc.sync.dma_start(EYE[:], eye)
        I28 = pool.tile([128, 28], f32); nc.sync.dma_start(I28[:], i28)
        I128 = pool.tile([P, 128], f32); nc.sync.dma_start(I128[:], i128)
        MMB = pool.tile([128, 76], f32); nc.sync.dma_start(MMB[:], mmb)
        RNK = pool.tile([P, NCAND], f32); nc.sync.dma_start(RNK[:], rnk)
        IDN = pool.tile([128, 128], f32); nc.sync.dma_start(IDN[:], idn)

        # ---- A, counts ----
        A = pool.tile([P, 784], f32)
        nv.tensor_reduce(out=A[:], in_=AIN[:].rearrange("p (s k) -> p s k", k=NBLK),
                         axis=AX.X, op=ALU.add)
        CNT = pool.tile([P, 1], f32)
        nv.tensor_reduce(out=CNT[:], in_=A[:], axis=AX.X, op=ALU.add)
        ISZ = pool.tile([P, 1], u32)
        nv.tensor_scalar(out=ISZ[:], in0=CNT[:], scalar1=0.5, scalar2=None, op0=ALU.is_lt)
        DEN = pool.tile([P, 1], f32)
        nv.tensor_scalar(out=DEN[:], in0=CNT[:], scalar1=1.0, scalar2=None, op0=ALU.max)

        # ---- merge top-32 of 128 candidates ----
        CIF = pool.tile([P, NC128], f32)
        nv.tensor_copy(CIF[:], CI[:])
        nv.tensor_tensor(out=CIF[:], in0=CIF[:], in1=BBS[:], op=ALU.add)
        CVa = pool.tile([P, NC128], f32)
        nv.tensor_copy(CVa[:], CV[:])
        MV = pool.tile([P, NCAND], f32)
        MP = pool.tile([P, NCAND], u32)
        for r in range(4):
            nv.max(out=MV[:, r * 8:(r + 1) * 8], in_=CVa[:])
            nv.max_index(out=MP[:, r * 8:(r + 1) * 8],
                         in_max=MV[:, r * 8:(r + 1) * 8], in_values=CVa[:])
            nv.match_replace(out=CVa[:], in_to_replace=MV[:, r * 8:(r + 1) * 8],
                             in_values=CVa[:], imm_value=-1.0)
        MPF = pool.tile([P, NCAND], f32)
        nv.tensor_copy(MPF[:], MP[:])
        # gather global idx at positions
        EQP = pool.tile([P, NCAND * 128], f32)
        EQP_v = EQP[:].rearrange("p (k q) -> p k q", q=128)
        nv.tensor_tensor(out=EQP_v, in0=MPF[:].unsqueeze(2).broadcast_to([P, NCAND, 128]),
                         in1=I128[:].unsqueeze(1).broadcast_to([P, NCAND, 128]),
                         op=ALU.is_equal)
        nv.tensor_tensor(out=EQP_v, in0=EQP_v,
                         in1=CIF[:].unsqueeze(1).broadcast_to([P, NCAND, 128]), op=ALU.mult)
        GIX = pool.tile([P, NCAND], f32)
        nv.tensor_reduce(out=GIX[:], in_=EQP_v, axis=AX.X, op=ALU.max)

        if stage <= 1:
            DBG = pool.tile([P, 64], f32)
            nv.tensor_copy(DBG[:, 0:32], GIX[:])
            nv.tensor_copy(DBG[:, 32:64], MPF[:])
            nc.sync.dma_start(o_dbg[0:P, 0:64], DBG[:])
        # ---- interpolation coefficients ----
        def ts(dst, src, s1, s2, op0, op1=None):
            nv.tensor_scalar(out=dst, in0=src, scalar1=s1, scalar2=s2, op0=op0,
                             **({"op1": op1} if op1 is not None else {}))

        if stage <= 1:
            OUTZ = pool.tile([1, 1], f32)
            nv.memset(OUTZ[:], 0.0)
            nc.sync.dma_start(o_loss, OUTZ[:])
            raise _StageDone()

        i32 = mybir.dt.int32

        def floor_pos(XX, pfx):
            """floor(x) for x>=0: round-to-nearest (f32->i32->f32 copy) then
            subtract 1 where round went up."""
            RI = pool.tile([P, NCAND], i32, name=f"{pfx}_ri", tag=f"{pfx}_ri")
            nv.tensor_copy(RI[:], XX[:])
            RF = pool.tile([P, NCAND], f32, name=f"{pfx}_rf", tag=f"{pfx}_rf")
            nv.tensor_copy(RF[:], RI[:])
            GT = pool.tile([P, NCAND], f32, name=f"{pfx}_gt", tag=f"{pfx}_gt")
            nv.tensor_tensor(out=GT[:], in0=RF[:], in1=XX[:], op=ALU.is_gt)
            nv.tensor_tensor(out=RF[:], in0=RF[:], in1=GT[:], op=ALU.subtract)
            return RF

        TT = pool.tile([P, NCAND], f32)
        ts(TT[:], GIX[:], 1.0 / 448.0, None, ALU.mult)
        HH = floor_pos(TT, "fh")
        WW = pool.tile([P, NCAND], f32)
        nv.scalar_tensor_tensor(out=WW[:], in0=HH[:], scalar=-448.0, in1=GIX[:],
                                op0=ALU.mult, op1=ALU.add)

        def coeffs(XX, pfx):
            U = pool.tile([P, NCAND], f32, name=f"{pfx}_u", tag=f"{pfx}_u")
            ts(U[:], XX[:], 8.5, 1.0 / 16.0, ALU.add, ALU.mult)
            FL = floor_pos(U, f"{pfx}_flr")
            F = pool.tile([P, NCAND], f32, name=f"{pfx}_f", tag=f"{pfx}_f")
            nv.tensor_tensor(out=F[:], in0=U[:], in1=FL[:], op=ALU.subtract)
            X0 = pool.tile([P, NCAND], f32, name=f"{pfx}_x0", tag=f"{pfx}_x0")
            ts(X0[:], FL[:], 1.0, None, ALU.subtract)
            ts(X0[:], X0[:], 0.0, 27.0, ALU.max, ALU.min)
            X1 = pool.tile([P, NCAND], f32, name=f"{pfx}_x1", tag=f"{pfx}_x1")
            ts(X1[:], FL[:], 0.0, 27.0, ALU.max, ALU.min)
            W1 = F
            W0 = pool.tile([P, NCAND], f32, name=f"{pfx}_w0", tag=f"{pfx}_w0")
            ts(W0[:], F[:], -1.0, 1.0, ALU.mult, ALU.add)
            return X0, X1, W0, W1

        I0, I1, WH0, WH1 = coeffs(HH, "ch")
        J0, J1, WWA, WWB = coeffs(WW, "cw")
        WW0 = pool.tile([P, NCAND], f32)
        nv.tensor_tensor(out=WW0[:], in0=WWA[:], in1=RNK[:], op=ALU.mult)
        WW1 = pool.tile([P, NCAND], f32)
        nv.tensor_tensor(out=WW1[:], in0=WWB[:], in1=RNK[:], op=ALU.mult)

        if stage == 2:
            DBG2 = pool.tile([P, 128], f32)
            for i, t in enumerate([I0, I1, WH0, WH1]):
                nv.tensor_copy(DBG2[:, i * 32:(i + 1) * 32], t[:])
            nc.sync.dma_start(o_dbg[0:P, 0:128], DBG2[:])
        # ---- stage (pair,k)-flatten and G build ----
        STG = pool.tile([P, NCAND * 8], f32)
        STG_v = STG[:].rearrange("p (k a) -> p k a", a=8)
        for idx, arr in enumerate([I0, I1, WH0, WH1, J0, J1, WW0, WW1]):
            nv.tensor_copy(STG_v[:, :, idx:idx + 1], arr[:].unsqueeze(2))

        if stage == 2:
            OUTZ = pool.tile([1, 1], f32)
            nv.memset(OUTZ[:], 0.0)
            nc.sync.dma_start(o_loss, OUTZ[:])
            raise _StageDone()

        FLT = pool.tile([128, 80], f32)
        for g in range(10):
            nc.sync.dma_start(
                FLT[:, g * 8:(g + 1) * 8],
                STG[g * 4:(g + 1) * 4, :].rearrange("p (k a) -> p k a", a=8))

        G = pool.tile([P, 784], f32)
        GpsA = psum.tile([P, 392], f32)
        GpsB = psum.tile([P, 392], f32)
        for g in range(10):
            col = lambda i: FLT[:, g * 8 + i:g * 8 + i + 1]
            EQR0 = pool.tile([128, 28], f32, tag="eqr", bufs=2)
            nv.tensor_scalar(out=EQR0[:], in0=I28[:], scalar1=col(0), scalar2=None,
                             op0=ALU.is_equal)
            RQ = pool.tile([128, 28], f32, tag="rq", bufs=2)
            nv.tensor_scalar(out=RQ[:], in0=EQR0[:], scalar1=col(2), scalar2=None,
                             op0=ALU.mult)
            EQR1 = pool.tile([128, 28], f32, tag="eqr2", bufs=2)
            nv.tensor_scalar(out=EQR1[:], in0=I28[:], scalar1=col(1), scalar2=None,
                             op0=ALU.is_equal)
            nv.scalar_tensor_tensor(out=RQ[:], in0=EQR1[:], scalar=col(3), in1=RQ[:],
                                    op0=ALU.mult, op1=ALU.add)
            EQC0 = pool.tile([128, 28], f32, tag="eqr", bufs=2)
            nv.tensor_scalar(out=EQC0[:], in0=I28[:], scalar1=col(4), scalar2=None,
                             op0=ALU.is_equal)
            CQ = pool.tile([128, 28], f32, tag="cq", bufs=2)
            nv.tensor_scalar(out=CQ[:], in0=EQC0[:], scalar1=col(6), scalar2=None,
                             op0=ALU.mult)
            EQC1 = pool.tile([128, 28], f32, tag="eqr2", bufs=2)
            nv.tensor_scalar(out=EQC1[:], in0=I28[:], scalar1=col(5), scalar2=None,
                             op0=ALU.is_equal)
            nv.scalar_tensor_tensor(out=CQ[:], in0=EQC1[:], scalar=col(7), in1=CQ[:],
                                    op0=ALU.mult, op1=ALU.add)
            RHS = pool.tile([128, 784], f32, tag="rhs", bufs=2)
            nv.tensor_tensor(out=RHS[:].rearrange("p (a b) -> p a b", b=28),
                             in0=RQ[:].unsqueeze(2).broadcast_to([128, 28, 28]),
                             in1=CQ[:].unsqueeze(1).broadcast_to([128, 28, 28]),
                             op=ALU.mult)
            # band-membership lhsT: col j of MMB[:, 36-4g : 76-4g] is
            # one-hot(q//32 == j-4g) -> group g's 4 pairs land on rows 4g..4g+3
            lhsT_g = MMB[:, 36 - 4 * g:76 - 4 * g]
            nc.tensor.matmul(GpsA[:], lhsT=lhsT_g, rhs=RHS[:, 0:392],
                             start=(g == 0), stop=(g == 9))
            nc.tensor.matmul(GpsB[:], lhsT=lhsT_g, rhs=RHS[:, 392:784],
                             start=(g == 0), stop=(g == 9))
        ns.copy(G[:, 0:392], GpsA[:])
        ns.copy(G[:, 392:784], GpsB[:])

        if stage == 3:
            nc.sync.dma_start(o_dbg[0:P, 0:784], G[:])
        if stage == 35:
            nc.sync.dma_start(o_dbg[0:128, 0:80], FLT[:])
        # ---- coef + fsm ----
        if stage in (3, 35):
            OUTZ = pool.tile([1, 1], f32)
            nv.memset(OUTZ[:], 0.0)
            nc.sync.dma_start(o_loss, OUTZ[:])
            raise _StageDone()

        RDEN = pool.tile([P, 1], f32)
        nv.reciprocal(RDEN[:], DEN[:])
        AMN = pool.tile([P, 784], f32)
        nv.tensor_scalar(out=AMN[:], in0=A[:], scalar1=RDEN[:], scalar2=None, op0=ALU.mult)
        COEF = pool.tile([P, 784], f32)
        nv.select(COEF[:], ISZ[:].broadcast_to([P, 784]), G[:], AMN[:])
        nv.tensor_scalar(out=COEF[:], in0=COEF[:], scalar1=LAB[:], scalar2=None, op0=ALU.mult)

        CT = pool.tile([RB, 7 * P], f32)
        for u in range(7):
            TPS = psum.tile([RB, P], f32, tag="tps", bufs=2)
            nc.tensor.transpose(TPS[:], COEF[:, u * RB:(u + 1) * RB], IDN[:P, :P])
            ns.copy(CT[:, u * P:(u + 1) * P], TPS[:])

        FSM = pool.tile([C, B * D], f32)
        for b2 in range(B):
            FSps = psum.tile([C, D], f32, tag="fsps")
            for u in range(7):
                nc.tensor.matmul(FSps[:], lhsT=CT[:, u * P + b2 * C:u * P + (b2 + 1) * C],
                                 rhs=FM[:, u * (B * D) + b2 * D:u * (B * D) + (b2 + 1) * D],
                                 start=(u == 0), stop=(u == 6))
            ns.copy(FSM[:, b2 * D:(b2 + 1) * D], FSps[:])

        if stage == 4:
            nc.sync.dma_start(o_dbg[0:C, 0:B * D], FSM[:])
        # ---- scan ----
        if stage == 4:
            OUTZ = pool.tile([1, 1], f32)
            nv.memset(OUTZ[:], 0.0)
            nc.sync.dma_start(o_loss, OUTZ[:])
            raise _StageDone()

        ONES20 = pool.tile([C, 1], f32)
        nv.memset(ONES20[:], 1.0)
        LC = pool.tile([1, 1], f32); nv.memset(LC[:], 0.0)
        CCF = pool.tile([1, 1], f32); nv.memset(CCF[:], 0.0)
        SCR = pool.tile([C, D], f32, tag="scr")
        SCR2 = pool.tile([C, C], f32, tag="scr2")

        def l2norm_div(dst, src):
            nn2 = pool.tile([C, 1], f32, tag="nn2")
            nv.tensor_tensor(out=SCR[:], in0=src, in1=src, op=ALU.mult)
            nv.tensor_reduce(out=nn2[:], in_=SCR[:], axis=AX.X, op=ALU.add)
            nr = pool.tile([C, 1], f32, tag="nr")
            ns.activation(nr[:], nn2[:], AFT.Sqrt)
            nv.tensor_scalar(out=nr[:], in0=nr[:], scalar1=1e-12, scalar2=None, op0=ALU.max)
            rn = pool.tile([C, 1], f32, tag="rn")
            nv.reciprocal(rn[:], nr[:])
            nv.tensor_scalar(out=dst, in0=src, scalar1=rn[:], scalar2=None, op0=ALU.mult)

        for b2 in range(B):
            FSMb = FSM[:, b2 * D:(b2 + 1) * D]
            presb = LAB2[:, b2:b2 + 1]

            FSMN = pool.tile([C, D], f32, tag="fsmn")
            l2norm_div(FSMN[:], FSMb)
            FCN = pool.tile([C, D], f32, tag="fcn")
            l2norm_div(FCN[:], FC[:])

            # transposes of fsm (raw), fsm_n, fc_n -> [128, C] chunks
            TRS = {}
            for nm, srct in (("fsm", FSMb), ("fsmn", FSMN[:]), ("fcn", FCN[:])):
                dst = pool.tile([128, 2 * C], f32, tag=f"tr_{nm}", name=f"tr_{nm}_{b2}")
                for h2 in range(2):
                    TPS4 = psum.tile([128, C], f32, tag="tps", bufs=2)
                    nc.tensor.transpose(TPS4[:], srct[:, h2 * 128:(h2 + 1) * 128],
                                        IDN[:C, :C])
                    ns.copy(dst[:, h2 * C:(h2 + 1) * C], TPS4[:])
                TRS[nm] = dst

            COSps = psum.tile([C, C], f32, tag="cosps")
            for h2 in range(2):
                nc.tensor.matmul(COSps[:], lhsT=TRS["fsmn"][:, h2 * C:(h2 + 1) * C],
                                 rhs=TRS["fcn"][:, h2 * C:(h2 + 1) * C],
                                 start=(h2 == 0), stop=(h2 == 1))
            COSC = pool.tile([C, C], f32, tag="cosc")
            ns.activation(COSC[:], COSps[:], AFT.Abs)
            nv.tensor_scalar(out=COSC[:], in0=COSC[:], scalar1=1e-5, scalar2=1.0 - 1e-5,
                             op0=ALU.max, op1=ALU.min)
            LGC = pool.tile([C, C], f32, tag="lgc")
            ns.activation(LGC[:], COSC[:], AFT.Ln)
            OM = pool.tile([C, C], f32, tag="om")
            nv.tensor_scalar(out=OM[:], in0=COSC[:], scalar1=-1.0, scalar2=1.0,
                             op0=ALU.mult, op1=ALU.add)
            LOM = pool.tile([C, C], f32, tag="lom")
            ns.activation(LOM[:], OM[:], AFT.Ln)

            IDM = pool.tile([C, C], f32, tag="idm")
            nv.tensor_scalar(out=IDM[:], in0=EYE[:], scalar1=presb, scalar2=None, op0=ALU.mult)
            DIF = pool.tile([C, C], f32, tag="dif")
            nv.tensor_tensor(out=DIF[:], in0=LGC[:], in1=LOM[:], op=ALU.subtract)
            CCFD = pool.tile([C, 1], f32, tag="ccfd")
            nv.tensor_tensor(out=SCR2[:], in0=IDM[:], in1=DIF[:], op=ALU.mult)
            nv.tensor_reduce(out=CCFD[:], in_=SCR2[:], axis=AX.X, op=ALU.add)
            R1 = pool.tile([C, 1], f32, tag="r1")
            nv.tensor_reduce(out=R1[:], in_=LOM[:], axis=AX.X, op=ALU.add)
            nv.tensor_tensor(out=CCFD[:], in0=CCFD[:], in1=R1[:], op=ALU.add)

            COSM = pool.tile([C, C], f32, tag="cosm")
            nv.scalar_tensor_tensor(out=COSM[:], in0=EYE[:], scalar=-1e9, in1=COSC[:],
                                    op0=ALU.mult, op1=ALU.add)
            OFF = pool.tile([C, 1], f32, tag="off")
            nv.tensor_reduce(out=OFF[:], in_=COSM[:], axis=AX.X, op=ALU.max)
            QUAL = pool.tile([C, 1], f32, tag="qual")
            nv.tensor_scalar(out=QUAL[:], in0=OFF[:], scalar1=0.6, scalar2=None, op0=ALU.is_lt)
            nv.tensor_tensor(out=QUAL[:], in0=QUAL[:], in1=presb, op=ALU.mult)

            LOGps = psum.tile([C, C], f32, tag="cosps")
            for h2 in range(2):
                nc.tensor.matmul(LOGps[:], lhsT=TRS["fsm"][:, h2 * C:(h2 + 1) * C],
                                 rhs=PJT[:, h2 * C:(h2 + 1) * C],
                                 start=(h2 == 0), stop=(h2 == 1))
            MX = pool.tile([C, 1], f32, tag="mx")
            nv.tensor_reduce(out=MX[:], in_=LOGps, axis=AX.X, op=ALU.max)
            XT = pool.tile([C, C], f32, tag="xt")
            nv.tensor_scalar(out=XT[:], in0=LOGps, scalar1=MX[:], scalar2=None,
                             op0=ALU.subtract)
            ET = pool.tile([C, C], f32, tag="et")
            ns.activation(ET[:], XT[:], AFT.Exp)
            SM = pool.tile([C, 1], f32, tag="sm")
            nv.tensor_reduce(out=SM[:], in_=ET[:], axis=AX.X, op=ALU.add)
            LGS = pool.tile([C, 1], f32, tag="lgs")
            ns.activation(LGS[:], SM[:], AFT.Ln)
            LGP = pool.tile([C, C], f32, tag="lgp")
            nv.tensor_scalar(out=LGP[:], in0=XT[:], scalar1=LGS[:], scalar2=-100.0,
                             op0=ALU.subtract, op1=ALU.max)
            SME = pool.tile([C, C], f32, tag="sme")
            nv.tensor_tensor(out=SME[:], in0=SM[:].broadcast_to([C, C]), in1=ET[:],
                             op=ALU.subtract)
            LSME = pool.tile([C, C], f32, tag="lsme")
            ns.activation(LSME[:], SME[:], AFT.Ln)
            L1P = pool.tile([C, C], f32, tag="l1p")
            nv.tensor_scalar(out=L1P[:], in0=LSME[:], scalar1=LGS[:], scalar2=-100.0,
                             op0=ALU.subtract, op1=ALU.max)

            DD = pool.tile([C, C], f32, tag="dd")
            nv.tensor_tensor(out=DD[:], in0=LGP[:], in1=L1P[:], op=ALU.subtract)
            DDG = pool.tile([C, 1], f32, tag="ddg")
            nv.tensor_tensor(out=SCR2[:], in0=EYE[:], in1=DD[:], op=ALU.mult)
            nv.tensor_reduce(out=DDG[:], in_=SCR2[:], axis=AX.X, op=ALU.add)
            RSM = pool.tile([C, 1], f32, tag="rsm")
            nv.tensor_reduce(out=RSM[:], in_=L1P[:], axis=AX.X, op=ALU.add)
            TERM = pool.tile([C, 1], f32, tag="term")
            nv.tensor_tensor(out=TERM[:], in0=DDG[:], in1=RSM[:], op=ALU.add)
            nv.tensor_scalar(out=TERM[:], in0=TERM[:], scalar1=-1.0 / C, scalar2=None,
                             op0=ALU.mult)
            CONTR = pool.tile([C, 1], f32, tag="contr")
            nv.tensor_tensor(out=CONTR[:], in0=TERM[:], in1=QUAL[:], op=ALU.mult)

            PR = pool.tile([C, 3], f32, tag="pr")
            nv.tensor_copy(PR[:, 0:1], QUAL[:])
            nv.tensor_copy(PR[:, 1:2], CONTR[:])
            nv.tensor_copy(PR[:, 2:3], CCFD[:])
            REDps = psum.tile([1, 3], f32, tag="redps")
            nc.tensor.matmul(REDps[:], lhsT=ONES20[:], rhs=PR[:], start=True, stop=True)
            RED = pool.tile([1, 3], f32, tag="red")
            ns.copy(RED[:], REDps[:])

            # loss_cls = (loss_cls + S) / max(n, 1)   (divide-by-1 when n==0)
            nv.tensor_tensor(out=LC[:], in0=LC[:], in1=RED[:, 1:2], op=ALU.add)
            NB1 = pool.tile([1, 1], f32, tag="nb1")
            nv.tensor_scalar(out=NB1[:], in0=RED[:, 0:1], scalar1=1.0, scalar2=None,
                             op0=ALU.max)
            RNB = pool.tile([1, 1], f32, tag="rnb")
            nv.reciprocal(RNB[:], NB1[:])
            nv.tensor_scalar(out=LC[:], in0=LC[:], scalar1=RNB[:], scalar2=None,
                             op0=ALU.mult)
            # loss_ccf += -(1/400) * ccf_sum
            nv.scalar_tensor_tensor(out=CCF[:], in0=RED[:, 2:3], scalar=-1.0 / (C * C),
                                    in1=CCF[:], op0=ALU.mult, op1=ALU.add)

            # fc = fc + 0.05 * qual * (fsm - fc)
            DFC = pool.tile([C, D], f32, tag="dfc")
            nv.tensor_tensor(out=DFC[:], in0=FSMb, in1=FC[:], op=ALU.subtract)
            Q05 = pool.tile([C, 1], f32, tag="q05")
            nv.tensor_scalar(out=Q05[:], in0=QUAL[:], scalar1=0.05, scalar2=None,
                             op0=ALU.mult)
            nv.scalar_tensor_tensor(out=FC[:], in0=DFC[:], scalar=Q05[:], in1=FC[:],
                                    op0=ALU.mult, op1=ALU.add)

        OUT = pool.tile([1, 1], f32)
        nv.tensor_tensor(out=OUT[:], in0=LC[:], in1=CCF[:], op=ALU.add)
        nc.sync.dma_start(o_loss, OUT[:])
    except _StageDone:
        pass

    nc.compile()
    return nc


# --------------------------------------------------------------------------
# Fast path (no top-k: valid when every present class has count > 0).
# --------------------------------------------------------------------------

bf16 = mybir.dt.bfloat16
f16 = mybir.dt.float16


def _emit_tree(nc, pool, src, n, width, op, pfx, dt=None, part=None):
    """Binary-tree reduce over n leaves of `width` cols each -> [P, width]."""
    nv = nc.vector
    dt = bf16 if dt is None else dt
    part = RB if part is None else part
    cur = src
    lvl = 0
    while n > 1:
        h = n // 2
        odd = n - 2 * h
        dst = pool.tile([part, h * width], dt, name=f"{pfx}_l{lvl}")
        nv.tensor_tensor(out=dst[:], in0=cur[:, :h * width],
                         in1=cur[:, h * width:2 * h * width], op=op)
        if odd:
            nv.tensor_tensor(out=dst[:, :width], in0=dst[:, :width],
                             in1=cur[:, 2 * h * width:(2 * h + 1) * width], op=op)
        cur, n, lvl = dst, h, lvl + 1
    return cur


def _build_a_fast(hig, low, bg, CP):
    nc = bacc.Bacc("TRN2", target_bir_lowering=False, debug=False, num_devices=8)

    camv = nc.dram_tensor("camv", [CP, NPIX], bf16, kind="ExternalInput").ap()
    wrt = nc.dram_tensor("wrt", [RB, 28], bf16, kind="ExternalInput").ap()
    wct = nc.dram_tensor("wct", [RB, 4 * 28], f16, kind="ExternalInput").ap()
    o_a = nc.dram_tensor("o_a", [28, CP * 28], f32, kind="ExternalOutput").ap()

    thmax = float(max(hig, low, bg))
    # class groups of <=4 (PSUM bank = 512 f32 = 4 classes x 4 u x 28)
    grps = []
    c0 = 0
    while c0 < CP:
        n = min(4, CP - c0)
        grps.append((c0, n))
        c0 += n
    ch = (CP + 1) // 2  # class-split DMA halves

    with tile.TileContext(nc) as tc, ExitStack() as ctx:
        pool = ctx.enter_context(tc.tile_pool(name="p", bufs=1))
        psum = ctx.enter_context(tc.tile_pool(name="ps", bufs=1, space="PSUM"))
        nv = nc.vector
        ns = nc.scalar

        VP = pool.tile([RB, CP * W], bf16)
        # class-quarters so partial max trees overlap the later DMA chunks
        qs = []
        q0 = 0
        while q0 < CP:
            qn = min(max(1, (CP + 3) // 4), CP - q0)
            qs.append((q0, qn))
            q0 += qn
        for (q0_, qn_) in qs:
            nc.sync.dma_start(VP[:, q0_ * W:(q0_ + qn_) * W],
                              camv[q0_:q0_ + qn_].rearrange("c (r w) -> r c w",
                                                            w=W))
        WR = pool.tile([RB, 28], bf16)
        nc.sync.dma_start(WR[:], wrt)
        WC = pool.tile([RB, 4 * 28], f16)
        nc.sync.dma_start(WC[:], wct)

        # ---- per-pixel keep-gate (bf16, w innermost so TTs hit 2x mode) ----
        parts = [_emit_tree(nc, pool, VP[:, a * W:(a + n) * W], n, W, ALU.max,
                            f"t1q{i}") for i, (a, n) in enumerate(qs)]
        while len(parts) > 1:
            nxt = []
            for i in range(0, len(parts) - 1, 2):
                t = pool.tile([RB, W], bf16, name=f"t1m{len(parts)}_{i}")
                nv.tensor_tensor(out=t[:], in0=parts[i][:], in1=parts[i + 1][:],
                                 op=ALU.max)
                nxt.append(t)
            if len(parts) % 2:
                nxt.append(parts[-1])
            parts = nxt
        T1 = parts[0]
        T13 = pool.tile([RB, W], bf16)
        nv.tensor_scalar(out=T13[:], in0=T1[:], scalar1=-MARGIN, scalar2=None,
                         op0=ALU.add)
        NG = pool.tile([RB, CP * W], bf16)
        NG_cw = NG[:].rearrange("p (c w) -> p c w", w=W)
        V_cw = VP[:].rearrange("p (c w) -> p c w", w=W)
        nv.tensor_tensor(out=NG_cw, in0=V_cw,
                         in1=T13[:].unsqueeze(1).broadcast_to([RB, CP, W]),
                         op=ALU.is_gt)
        NGS = _emit_tree(nc, pool, NG, CP, W, ALU.add, "ngs")

        # keep iff t1 >= thmax and (exactly one class above t1-0.3 or t1 <= hig)
        LEH = pool.tile([RB, W], bf16)
        nv.tensor_scalar(out=LEH[:], in0=T1[:], scalar1=float(hig),
                         scalar2=None, op0=ALU.is_le)
        K1 = pool.tile([RB, W], bf16)
        nv.tensor_scalar(out=K1[:], in0=T1[:], scalar1=thmax,
                         scalar2=None, op0=ALU.is_ge)
        MOK = pool.tile([RB, W], bf16)
        nv.tensor_scalar(out=MOK[:], in0=NGS[:], scalar1=1.5, scalar2=None,
                         op0=ALU.is_lt)
        nv.tensor_tensor(out=MOK[:], in0=MOK[:], in1=LEH[:], op=ALU.max)
        KEEP = pool.tile([RB, W], bf16)
        nv.tensor_tensor(out=KEEP[:], in0=K1[:], in1=MOK[:], op=ALU.mult)
        # threshold map: t1 where kept else 2.0 (cam < 1, so M == 0 there).
        # Kept pixels have margin >= 0.3 -> no tie at the max -> M is one-hot.
        # KEEP is exactly 0/1 so this select-by-arithmetic is exact in bf16.
        T1K = pool.tile([RB, W], bf16)
        nv.tensor_tensor(out=T1K[:], in0=T1[:], in1=KEEP[:], op=ALU.mult)
        NK2 = pool.tile([RB, W], bf16)
        nv.tensor_scalar(out=NK2[:], in0=KEEP[:], scalar1=-2.0,
                         scalar2=2.0, op0=ALU.mult, op1=ALU.add)
        T1X = pool.tile([RB, W], bf16)
        nv.tensor_tensor(out=T1X[:], in0=T1K[:], in1=NK2[:], op=ALU.add)

        # ---- M chunks + PE bilinear downsample (exact: weights are k/32) ----
        M = pool.tile([RB, CP * W], bf16)
        M_cw = M[:].rearrange("p (c w) -> p c w", w=W)
        Yps = [psum.tile([RB, n * 4 * 28], f32, name=f"yps{g}")
               for g, (c0, n) in enumerate(grps)]
        Ysb = [pool.tile([RB, n * 4 * 28], f16, name=f"ysb{g}")
               for g, (c0, n) in enumerate(grps)]
        Aps = psum.tile([28, CP * 28], f32)
        # stage 1 groups back-to-back on PE; copies trail on Act/DVE; then
        # stage 2 groups (so PE never waits a copy mid-stream)
        for g, (c0, n) in enumerate(grps):
            T1X_b = T1X[:].unsqueeze(1).broadcast_to([RB, n, W])
            nv.tensor_tensor(out=M_cw[:, c0:c0 + n, :],
                             in0=V_cw[:, c0:c0 + n, :], in1=T1X_b,
                             op=ALU.is_ge)
            for cr in range(n):
                c = c0 + cr
                for u in range(4):
                    nc.tensor.matmul(
                        Yps[g][:, (cr * 4 + u) * 28:(cr * 4 + u + 1) * 28],
                        lhsT=M[:, c * W + u * RB:c * W + (u + 1) * RB],
                        rhs=WR[:], start=True, stop=True)
            if g % 2 == 0:
                ns.copy(Ysb[g][:], Yps[g][:])
            else:
                nv.tensor_copy(Ysb[g][:], Yps[g][:])
        for g, (c0, n) in enumerate(grps):
            for cr in range(n):
                c = c0 + cr
                for u in range(4):
                    nc.tensor.matmul(
                        Aps[:, c * 28:(c + 1) * 28],
                        lhsT=Ysb[g][:, (cr * 4 + u) * 28:(cr * 4 + u + 1) * 28],
                        rhs=WC[:, u * 28:(u + 1) * 28],
                        start=(u == 0), stop=(u == 3))
        Asb = pool.tile([28, CP * 28], f32)
        ns.copy(Asb[:], Aps[:])
        nc.sync.dma_start(o_a, Asb[:])


    nc.compile()
    return nc


def _build_b_fast():
    nc = bacc.Bacc("TRN2", target_bir_lowering=False, debug=False, num_devices=1)
    P = B * C  # 40

    # aint layout: [pix%112, k*280 + u*40 + pair]  (A^T partials, block-major)
    aint = nc.dram_tensor("aint", [112, NBLK * 7 * P], f32,
                          kind="ExternalInput").ap()
    fmi = nc.dram_tensor("fmi", [112, 7 * B * D], bf16, kind="ExternalInput").ap()
    smt = nc.dram_tensor("smt", [128, 226], f32, kind="ExternalInput").ap()

    o_loss = nc.dram_tensor("o_loss", [1, 1], f32, kind="ExternalOutput").ap()
    o_cnt = nc.dram_tensor("o_cnt", [1, P], f32, kind="ExternalOutput").ap()

    L5 = float(np.log(1e-5))
    L1M = float(np.log1p(-1e-5))
    LNLO = float(np.log(1e-5))
    LNHI = float(np.log1p(-1e-5))

    with tile.TileContext(nc) as tc, ExitStack() as ctx:
        pool = ctx.enter_context(tc.tile_pool(name="p", bufs=1))
        psum = ctx.enter_context(tc.tile_pool(name="ps", bufs=1, space="PSUM"))
        nv = nc.vector
        ns = nc.scalar

        AIN = pool.tile([112, NBLK * 7 * P], f32)
        nc.sync.dma_start(AIN[:], aint)
        SM = pool.tile([128, 226], f32)
        nc.sync.dma_start(SM[:], smt)
        FM = pool.tile([112, 7 * B * D], bf16)
        for fc in range(4):
            c0, c1 = fc * 1024, min((fc + 1) * 1024, 7 * B * D)
            nc.sync.dma_start(FM[:, c0:c1], fmi[:, c0:c1])
        PJT = SM[:, 0:40]            # [128, (dc,c2)] proj^T chunks
        ONES112 = SM[0:112, 40:41]
        EYE = SM[0:C, 41:61]
        LAB2 = SM[0:C, 61:63]
        BSEL = SM[0:P, 63:65]
        EYEBC = SM[0:P, 65:85]
        ONES20 = SM[0:C, 85:86]
        SH0 = SM[0:C, 86:126]
        SH1 = SM[0:C, 126:166]
        ONESM = SM[0:C, 166:186]
        ONES1R = SM[0:1, 186:226]    # [1, 40] ones

        # ---- early independent: n_b, 1/max(n_b,1), step-0 ccf constant ----
        NSps = psum.tile([1, 2], f32, name="nsps")
        nc.tensor.matmul(NSps[:], lhsT=ONES20, rhs=LAB2, start=True, stop=True)
        NS = pool.tile([1, 2], f32)
        nv.tensor_copy(NS[:], NSps[:])
        DN = pool.tile([1, 2], f32)
        nv.tensor_scalar(out=DN[:], in0=NS[:], scalar1=1.0, scalar2=None,
                         op0=ALU.max)
        RDN = pool.tile([1, 2], f32)
        nv.reciprocal(RDN[:], DN[:])
        CCF = pool.tile([1, 1], f32)
        nv.tensor_scalar(out=CCF[:], in0=NS[:, 0:1],
                         scalar1=-(L5 - L1M) / (C * C), scalar2=-L1M,
                         op0=ALU.mult, op1=ALU.add)

        # ---- raw coef^T = sum over 4 row-blocks (tree); scale LR rides later --
        H1 = pool.tile([112, 2 * 7 * P], f32)
        nv.tensor_tensor(out=H1[:], in0=AIN[:, :2 * 7 * P],
                         in1=AIN[:, 2 * 7 * P:], op=ALU.add)
        CTR = pool.tile([112, 7 * P], bf16)
        nv.tensor_tensor(out=CTR[:], in0=H1[:, :7 * P], in1=H1[:, 7 * P:],
                         op=ALU.add)
        ONE112B = pool.tile([112, 1], bf16)
        nv.memset(ONE112B[:], 1.0)

        # ---- counts -> LR = label/max(cnt,1) as a [40,1] column via PE ----
        CNTps = psum.tile([1, 7 * P], f32, name="cntps")
        nc.tensor.matmul(CNTps[:], lhsT=ONE112B[:], rhs=CTR[:], start=True,
                         stop=True)
        CNTR = pool.tile([1, 7 * P], f32)
        nv.tensor_copy(CNTR[:], CNTps[:])
        CNT = _emit_tree(nc, pool, CNTR, 7, P, ALU.add, "cnt", dt=f32, part=1)
        nc.sync.dma_start(o_cnt, CNT[:])
        DENR = pool.tile([1, P], f32)
        nv.tensor_scalar(out=DENR[:], in0=CNT[:], scalar1=1.0, scalar2=None,
                         op0=ALU.max)
        RDR = pool.tile([1, P], f32)
        nv.reciprocal(RDR[:], DENR[:])
        LRps = psum.tile([P, P], f32, name="lrps")
        nc.tensor.matmul(LRps[:], lhsT=RDR[:], rhs=ONES1R, start=True, stop=True)
        LR40 = pool.tile([P, 1], f32)
        nv.tensor_copy(LR40[:], LRps[:, 0:1])

        # ---- fsm^T (raw scale) = fmap_ds^T @ coef_raw^T ----
        FTps = [psum.tile([128, P], f32, name=f"ftps{dc}", tag="ftps", bufs=2)
                for dc in range(2)]
        for dc in range(2):
            for b2 in range(B):
                for u in range(7):
                    nc.tensor.matmul(
                        FTps[dc][:, b2 * C:(b2 + 1) * C],
                        lhsT=FM[:, u * (B * D) + b2 * D + dc * 128:
                                u * (B * D) + b2 * D + (dc + 1) * 128],
                        rhs=CTR[:, u * P + b2 * C:u * P + (b2 + 1) * C],
                        start=(u == 0), stop=(u == 6))
        FSMT = pool.tile([128, 2 * P], f32)   # [d, (dc, b, c)]
        nv.tensor_copy(FSMT[:, 0:P], FTps[0][:])
        nv.tensor_copy(FSMT[:, P:2 * P], FTps[1][:])

        # ---- Gram diagonals (raw norms) + raw cos dot + logits ----
        SMLps = psum.tile([C, 4 * C], f32, name="smlps")
        for b2 in range(B):
            for dc in range(2):
                nc.tensor.matmul(
                    SMLps[:, b2 * C:(b2 + 1) * C],
                    lhsT=FSMT[:, dc * P + b2 * C:dc * P + (b2 + 1) * C],
                    rhs=FSMT[:, dc * P + b2 * C:dc * P + (b2 + 1) * C],
                    start=(dc == 0), stop=(dc == 1))
        RAWps = SMLps[:, 2 * C:3 * C]
        for dc in range(2):
            nc.tensor.matmul(RAWps,
                             lhsT=FSMT[:, dc * P + C:(dc + 1) * P],
                             rhs=FSMT[:, dc * P:dc * P + C],
                             start=(dc == 0), stop=(dc == 1))
        MMps = psum.tile([P, C], f32, name="mmps")
        LOGps = MMps[:, 0:C]
        for dc in range(2):
            nc.tensor.matmul(LOGps, lhsT=FSMT[:, dc * P:(dc + 1) * P],
                             rhs=PJT[:, dc * C:(dc + 1) * C],
                             start=(dc == 0), stop=(dc == 1))

        SCRD = pool.tile([C, C], f32)
        NRM2B = pool.tile([C, 2], f32)
        for b2 in range(B):
            nv.tensor_tensor(out=SCRD[:], in0=SMLps[:, b2 * C:(b2 + 1) * C],
                             in1=EYE, op=ALU.mult)
            nv.tensor_reduce(out=NRM2B[:, b2:b2 + 1], in_=SCRD[:], axis=AX.X,
                             op=ALU.add)
        NRM2G = pool.tile([C, 2], f32)
        nv.tensor_scalar(out=NRM2G[:], in0=NRM2B[:], scalar1=1e-24, scalar2=None,
                         op0=ALU.max)
        LN2 = pool.tile([C, 2], f32)
        ns.activation(LN2[:], NRM2G[:], AFT.Ln)

        # ---- cos in log space: ln|dot| - ln||a|| - ln||b|| (+absent -> -50) --
        hp = ExitStack()
        hp.enter_context(tc.high_priority())
        ABSR = pool.tile([C, C], f32)
        nv.tensor_scalar(out=ABSR[:], in0=RAWps, scalar1=-1.0, scalar2=None,
                         op0=ALU.mult)
        nv.tensor_tensor(out=ABSR[:], in0=ABSR[:], in1=RAWps, op=ALU.max)
        nv.tensor_scalar(out=ABSR[:], in0=ABSR[:], scalar1=1e-30, scalar2=None,
                         op0=ALU.max)
        LNC = pool.tile([C, C], f32)
        ns.activation(LNC[:], ABSR[:], AFT.Ln)
        # row term: -0.5*ln n1_c ; column term via PE: -0.5*ln n0_j - 50*(1-p0_j)
        COLV = pool.tile([C, 1], f32)
        nv.tensor_scalar(out=COLV[:], in0=LAB2[:, 0:1], scalar1=50.0,
                         scalar2=-50.0, op0=ALU.mult, op1=ALU.add)
        nv.scalar_tensor_tensor(out=COLV[:], in0=LN2[:, 0:1], scalar=-0.5,
                                in1=COLV[:], op0=ALU.mult, op1=ALU.add)
        DIAGC = pool.tile([C, C], f32)
        nv.tensor_scalar(out=DIAGC[:], in0=EYE, scalar1=COLV[:], scalar2=None,
                         op0=ALU.mult)
        CSMps = SMLps[:, 3 * C:4 * C]
        nc.tensor.matmul(CSMps, lhsT=ONESM, rhs=DIAGC[:], start=True, stop=True)
        RV = pool.tile([C, 1], f32)
        nv.tensor_scalar(out=RV[:], in0=LN2[:, 1:2], scalar1=-0.5, scalar2=None,
                         op0=ALU.mult)
        nv.tensor_scalar(out=LNC[:], in0=LNC[:], scalar1=RV[:], scalar2=None,
                         op0=ALU.add)
        nv.tensor_tensor(out=LNC[:], in0=LNC[:], in1=CSMps, op=ALU.add)
        nv.tensor_scalar(out=LNC[:], in0=LNC[:], scalar1=LNLO, scalar2=LNHI,
                         op0=ALU.max, op1=ALU.min)
        COSC = pool.tile([C, C], f32)
        ns.activation(COSC[:], LNC[:], AFT.Exp)
        hp.close()

        # ---- softmax-BCE per (b,c) row (logits get the true LR scale) ----
        LOG = pool.tile([P, C], f32)
        nv.tensor_scalar(out=LOG[:], in0=LOGps, scalar1=LR40[:], scalar2=None,
                         op0=ALU.mult)
        MX = pool.tile([P, 1], f32)
        nv.tensor_reduce(out=MX[:], in_=LOG[:], axis=AX.X, op=ALU.max)
        XT = pool.tile([P, C], f32)
        nv.tensor_scalar(out=XT[:], in0=LOG[:], scalar1=MX[:], scalar2=None,
                         op0=ALU.subtract)
        ET = pool.tile([P, C], f32)
        SMR = pool.tile([P, 1], f32)
        ns.activation(ET[:], XT[:], AFT.Exp, accum_out=SMR[:])
        LGS = pool.tile([P, 1], f32)
        ns.activation(LGS[:], SMR[:], AFT.Ln)
        LGP = pool.tile([P, C], f32)
        nv.tensor_scalar(out=LGP[:], in0=XT[:], scalar1=LGS[:], scalar2=-100.0,
                         op0=ALU.subtract, op1=ALU.max)
        SME = pool.tile([P, C], f32)
        nv.scalar_tensor_tensor(out=SME[:], in0=ET[:], scalar=-1.0,
                                in1=SMR[:].broadcast_to([P, C]),
                                op0=ALU.mult, op1=ALU.add)
        LSME = pool.tile([P, C], f32)
        ns.activation(LSME[:], SME[:], AFT.Ln)
        L1P = pool.tile([P, C], f32)
        nv.tensor_scalar(out=L1P[:], in0=LSME[:], scalar1=LGS[:], scalar2=-100.0,
                         op0=ALU.subtract, op1=ALU.max)
        DD = pool.tile([P, C], f32)
        nv.tensor_tensor(out=DD[:], in0=LGP[:], in1=L1P[:], op=ALU.subtract)
        SCRP = pool.tile([P, C], f32)
        nv.tensor_tensor(out=SCRP[:], in0=DD[:], in1=EYEBC, op=ALU.mult)
        DDG = pool.tile([P, 1], f32)
        nv.tensor_reduce(out=DDG[:], in_=SCRP[:], axis=AX.X, op=ALU.add)
        RSM = pool.tile([P, 1], f32)
        nv.tensor_reduce(out=RSM[:], in_=L1P[:], axis=AX.X, op=ALU.add)
        TERM = pool.tile([P, 1], f32)
        nv.tensor_tensor(out=TERM[:], in0=DDG[:], in1=RSM[:], op=ALU.add)
        nv.tensor_scalar(out=TERM[:], in0=TERM[:], scalar1=-1.0 / C, scalar2=None,
                         op0=ALU.mult)

        # ---- qualify: b0 = present_0; b1 = present_1 & offdiag max < 0.6 ----
        COSM = pool.tile([C, C], f32)
        nv.scalar_tensor_tensor(out=COSM[:], in0=EYE, scalar=-1e9, in1=COSC[:],
                                op0=ALU.mult, op1=ALU.add)
        OFF = pool.tile([C, 1], f32)
        nv.tensor_reduce(out=OFF[:], in_=COSM[:], axis=AX.X, op=ALU.max)
        QB1 = pool.tile([C, 1], f32)
        nv.tensor_scalar(out=QB1[:], in0=OFF[:], scalar1=0.6, scalar2=None,
                         op0=ALU.is_lt)
        nv.tensor_tensor(out=QB1[:], in0=QB1[:], in1=LAB2[:, 1:2], op=ALU.mult)
        QRps = psum.tile([P, 6], f32, name="qrps")
        Q40ps = QRps[:, 0:1]
        nc.tensor.matmul(Q40ps, lhsT=SH0, rhs=LAB2[:, 0:1], start=True,
                         stop=False)
        nc.tensor.matmul(Q40ps, lhsT=SH1, rhs=QB1[:], start=False, stop=True)
        CONTR = pool.tile([P, 1], f32)
        nv.tensor_tensor(out=CONTR[:], in0=TERM[:], in1=Q40ps, op=ALU.mult)

        # ---- ccf step 1 (LGC == clipped LNC already) ----
        OM = pool.tile([C, C], f32)
        nv.tensor_scalar(out=OM[:], in0=COSC[:], scalar1=-1.0, scalar2=1.0,
                         op0=ALU.mult, op1=ALU.add)
        LOM = pool.tile([C, C], f32)
        R1 = pool.tile([C, 1], f32)
        ns.activation(LOM[:], OM[:], AFT.Ln, accum_out=R1[:])
        DIF = pool.tile([C, C], f32)
        nv.tensor_tensor(out=DIF[:], in0=LNC[:], in1=LOM[:], op=ALU.subtract)
        nv.tensor_scalar(out=DIF[:], in0=DIF[:], scalar1=LAB2[:, 1:2],
                         scalar2=None, op0=ALU.mult)
        CC1 = pool.tile([C, 1], f32)
        nv.tensor_tensor(out=SCRD[:], in0=DIF[:], in1=EYE, op=ALU.mult)
        nv.tensor_reduce(out=CC1[:], in_=SCRD[:], axis=AX.X, op=ALU.add)
        nv.tensor_tensor(out=CC1[:], in0=CC1[:], in1=R1[:], op=ALU.add)

        # ---- partition sums via PE, final scalar chain ----
        REDps = QRps[0:1, 1:6]
        nc.tensor.matmul(REDps[:, 2:4], lhsT=CONTR[:], rhs=BSEL, start=True,
                         stop=True)
        nc.tensor.matmul(REDps[:, 4:5], lhsT=ONES20, rhs=CC1[:], start=True,
                         stop=True)
        RED = pool.tile([1, 5], f32)
        nv.tensor_copy(RED[:, 2:5], REDps[:, 2:5])
        LC = pool.tile([1, 1], f32)
        nv.tensor_tensor(out=LC[:], in0=RED[:, 2:3], in1=RDN[:, 0:1], op=ALU.mult)
        nv.tensor_tensor(out=LC[:], in0=LC[:], in1=RED[:, 3:4], op=ALU.add)
        nv.tensor_tensor(out=LC[:], in0=LC[:], in1=RDN[:, 1:2], op=ALU.mult)
        CC1S = pool.tile([1, 1], f32)
        nv.tensor_scalar(out=CC1S[:], in0=RED[:, 4:5], scalar1=-1.0 / (C * C),
                         scalar2=None, op0=ALU.mult)
        OUT = pool.tile([1, 1], f32)
        nv.tensor_tensor(out=OUT[:], in0=LC[:], in1=CCF[:], op=ALU.add)
        nv.tensor_tensor(out=OUT[:], in0=OUT[:], in1=CC1S[:], op=ALU.add)
        nc.sync.dma_start(o_loss, OUT[:])

    nc.compile()
    return nc


def _marshal_a_fast(cam, CP, idxs):
    bf = mybir.dt.np(bf16)
    fh = mybir.dt.np(f16)
    in_maps = []
    for core in range(8):
        b, blk = core // NBLK, core % NBLK
        idx = idxs[b]
        camv = np.zeros((CP, NPIX), bf)
        if len(idx):
            camv[:len(idx)] = cam[b, idx, blk * RB:(blk + 1) * RB, :].reshape(
                len(idx), NPIX).astype(bf)
        wct = np.ascontiguousarray(
            W1D.reshape(4, RB, 28).transpose(1, 0, 2).reshape(RB, 4 * 28))
        in_maps.append({
            "camv": camv,
            "wrt": np.ascontiguousarray(W1D[blk * RB:(blk + 1) * RB, :]).astype(bf),
            "wct": wct.astype(fh),
        })
    return in_maps


def _marshal_b_fast(res_a, fmap, cls_label, proj_weight, CP, idxs):
    P = B * C
    a8 = np.stack([res_a[k]["o_a"] for k in range(8)])          # [8, 28, CP*28]
    a8 = a8.reshape(B, NBLK, 28, CP, 28)
    afull = np.zeros((P, NBLK, 784), np.float32)
    for b in range(B):
        idx = idxs[b]
        if len(idx):
            # [blk, 28i, slot, 28j] -> [slot, blk, (i,j)]
            afull[b * C + idx] = a8[b, :, :, :len(idx), :].transpose(
                2, 0, 1, 3).reshape(len(idx), NBLK, 784)
    # aint[p, k*280 + u*40 + pair] = afull[pair, k, u*112 + p]
    aint = np.ascontiguousarray(
        afull.reshape(P, NBLK, 7, 112).transpose(3, 1, 2, 0)
    ).reshape(112, NBLK * 7 * P)

    fm = np.asarray(fmap, np.float32).reshape(B, D, 784)
    # fmi[p, u*512 + b*256 + d] = fmap[b, d, u*112 + p]
    fmi = np.ascontiguousarray(
        fm.transpose(2, 0, 1).reshape(7, 112, B, D).transpose(1, 0, 2, 3)
    ).reshape(112, 7 * B * D).astype(mybir.dt.np(bf16))

    lab = np.asarray(cls_label, np.float32)
    smt = np.zeros((128, 226), np.float32)
    smt[:, 0:40] = np.ascontiguousarray(
        np.asarray(proj_weight, np.float32).T
    ).reshape(2, 128, C).transpose(1, 0, 2).reshape(128, 2 * C)
    smt[0:112, 40:41] = 1.0
    smt[0:C, 41:61] = np.eye(C, dtype=np.float32)
    smt[0:C, 61:63] = lab.T
    smt[0:P, 63:65] = (np.arange(P)[:, None] // C ==
                       np.arange(2)[None, :]).astype(np.float32)
    smt[0:P, 65:85] = np.tile(np.eye(C, dtype=np.float32), (B, 1))
    smt[0:C, 85:86] = 1.0
    smt[0:C, 86:126] = np.eye(C, P, dtype=np.float32)
    smt[0:C, 126:166] = np.eye(C, P, k=C, dtype=np.float32)
    smt[0:C, 166:186] = 1.0
    smt[0:1, 186:226] = 1.0
    return {"aint": aint, "fmi": fmi, "smt": smt}


# --------------------------------------------------------------------------
# Host marshaling + driver
# --------------------------------------------------------------------------

_CACHE = {}


def _get_programs(hig, low, bg, CP):
    stage = int(os.environ.get("BASSK_B_STAGE", "99"))
    key = ("slow", float(hig), float(low), float(bg), stage, CP)
    if key not in _CACHE:
        _CACHE[key] = (_build_a(hig, low, bg, CP), _build_b(stage))
    return _CACHE[key]


def _get_programs_fast(hig, low, bg, CP):
    key = (float(hig), float(low), float(bg), CP)
    if key not in _CACHE:
        _CACHE[key] = (_build_a_fast(hig, low, bg, CP), _build_b_fast())
    return _CACHE[key]


def _marshal_a(cam, cls_label, CP, idxs):
    eye128 = np.eye(128, dtype=np.float32)
    clst = np.tile((np.arange(CP, dtype=np.float32) + 1.0)[None, :], (RB, 1))
    iodt = np.tile((float(CP) - np.arange(CP, dtype=np.float32))[None, :], (RB, 1))
    wct = np.ascontiguousarray(
        W1D.reshape(4, RB, 28).transpose(1, 0, 2).reshape(RB, 4 * 28))
    in_maps = []
    for core in range(8):
        b, blk = core // NBLK, core % NBLK
        idx = idxs[b]
        camv = np.zeros((CP, NPIX), np.float32)
        if len(idx):
            camv[:len(idx)] = cam[b, idx, blk * RB:(blk + 1) * RB, :].reshape(
                len(idx), NPIX)
        labt = np.tile((np.arange(CP) < len(idx)).astype(np.float32)[None, :],
                       (RB, 1))
        in_maps.append({
            "camv": camv,
            "labt": labt,
            "clst": clst,
            "iodt": iodt,
            "wrt": np.ascontiguousarray(W1D[blk * RB:(blk + 1) * RB, :]),
            "wct": wct,
            "idn": eye128,
        })
    return in_maps


def _marshal_b(res_a, fmap, cls_label, proj_weight, feature_contrast, CP, idxs):
    P = B * C
    ntk = (CP + 7) // 8
    # scatter packed per-slot A partials back to global classes
    a8 = np.stack([res_a[k]["o_a"] for k in range(8)])          # [8, 28, CP*28]
    a8 = a8.reshape(B, NBLK, 28, CP, 28)
    afull = np.zeros((B, C, 28, 28, NBLK), np.float32)
    for b in range(B):
        idx = idxs[b]
        if len(idx):
            # [blk, 28, slot, 28] -> [slot, 28, 28, blk]
            afull[b, idx] = a8[b, :, :, :len(idx), :].transpose(2, 1, 3, 0)
    ain = np.ascontiguousarray(afull).reshape(P, 784 * NBLK)

    cand_v = np.zeros((P, NBLK * NCAND), np.float32)
    cand_i = np.zeros((P, NBLK * NCAND), np.uint32)
    for core in range(8):
        b, blk = core // NBLK, core % NBLK
        tks = [res_a[core][f"o_tk{t}"] for t in range(ntk)]
        for j, c in enumerate(idxs[b]):
            tk = tks[j // 8]
            rb = (j % 8) * 16
            vals = np.concatenate([tk[rb + 14, 0:16], tk[rb + 15, 0:16]])
            gidx = np.concatenate([tk[rb + 14, 16:32], tk[rb + 15, 16:32]])
            cand_v[b * C + c, blk * NCAND:(blk + 1) * NCAND] = vals.view(np.float32)
            cand_i[b * C + c, blk * NCAND:(blk + 1) * NCAND] = gidx

    bbs = np.zeros((P, NBLK * NCAND), np.float32)
    for blk in range(NBLK):
        bbs[:, blk * NCAND:(blk + 1) * NCAND] = blk * RB * W

    # pre-transposed fmap: fmt[sp, u*(B*D) + b*D + d] = fmap[b, d, u*112+sp]
    fm = np.asarray(fmap, np.float32).reshape(B, D, 7, 112)
    fmi = np.ascontiguousarray(fm.transpose(3, 2, 0, 1)).reshape(112, 7 * B * D)

    rnk = np.zeros((P, NCAND), np.float32)
    rnk[:, :K_TOP] = 1.0 / K_TOP

    return {
        "ain": ain,
        "cdv": cand_v,
        "cdi": cand_i,
        "bbs": bbs,
        "fmi": fmi,
        "prj": np.ascontiguousarray(
            np.asarray(proj_weight, np.float32).T.reshape(2, 128, C)
            .transpose(1, 0, 2)).reshape(128, 2 * C),
        "lab": np.asarray(cls_label, np.float32).reshape(P, 1),
        "lab2": np.ascontiguousarray(np.asarray(cls_label, np.float32).T),
        "fc0": np.asarray(feature_contrast, np.float32),
        "eye": np.eye(C, dtype=np.float32),
        "i28": np.tile(np.arange(28, dtype=np.float32)[None, :], (128, 1)),
        "i128": np.tile(np.arange(128, dtype=np.float32)[None, :], (P, 1)),
        "mmb": (np.arange(128)[:, None] // NCAND ==
                np.arange(76)[None, :] - 36).astype(np.float32),
        "rnk": rnk,
        "idn": np.eye(128, dtype=np.float32),
    }


LAST_EXEC_NS = {}


def _run(nc, in_maps, core_ids, tag="k"):
    if os.environ.get("BASSK_SIM") == "1":
        from concourse.bass_interp import CoreSim, MultiCoreSim
        if len(core_ids) == 1:
            sim = CoreSim(nc, trace=False, require_finite=False)
            sims = [sim]
        else:
            msim = MultiCoreSim(nc, num_cores=len(core_ids), trace=False,
                                require_finite=False)
            sims = [msim.cores[i] for i in core_ids]
            sim = msim
        for s, m in zip(sims, in_maps):
            for name, arr in m.items():
                s.tensor(name)[:] = arr
        sim.simulate(check_with_hw=False)
        outs = []
        for s in sims:
            d = {}
            for alloc in nc.m.functions[0].allocations:
                if getattr(alloc, "kind", None) == "ExternalOutput":
                    nm = alloc.memorylocations[0].name
                    d[nm] = np.array(s.tensor(nm))
            outs.append(d)
        return outs
    trace = os.environ.get("BASSK_TRACE") == "1"
    if trace:
        try:
            from antenv.axon_hooks import get_axon_ntff_profile_hook  # noqa: F401
        except Exception:
            trace = False
    res = run_bass_kernel_spmd(nc, in_maps, core_ids, trace=trace)
    if res.exec_time_ns is not None:
        LAST_EXEC_NS[tag] = res.exec_time_ns
    return res.results


def _kernel_slow(fmap, cam, cls_label, proj_weight, feature_contrast,
                 hig_thre, low_thre, bg_thre, idxs, CP):
    nca, ncb = _get_programs(float(hig_thre), float(low_thre), float(bg_thre), CP)
    res_a = _run(nca, _marshal_a(cam, cls_label, CP, idxs), list(range(8)), tag="A")
    in_b = _marshal_b(res_a, fmap, cls_label, proj_weight, feature_contrast, CP, idxs)
    res_b = _run(ncb, [in_b], [0], tag="B")
    return np.float32(res_b[0]["o_loss"].reshape(-1)[0])


def kernel(fmap, cam, cls_label, proj_weight, feature_contrast,
           hig_thre, low_thre, bg_thre):
    fmap = np.asarray(fmap, np.float32)
    cam = np.asarray(cam, np.float32)
    lab = np.asarray(cls_label, np.float32)
    idxs = [np.where(lab[b] > 0.5)[0] for b in range(B)]
    cp_act = max((len(i) for i in idxs), default=0)
    CP = min(C, max(4, ((cp_act + 3) // 4) * 4))

    fc_zero = not np.any(np.asarray(feature_contrast, np.float32))
    if fc_zero and os.environ.get("BASSK_FORCE_SLOW") != "1":
        CPF = max(1, cp_act)
        nca, ncb = _get_programs_fast(float(hig_thre), float(low_thre),
                                      float(bg_thre), CPF)
        res_a = _run(nca, _marshal_a_fast(cam, CPF, idxs), list(range(8)), tag="A")
        in_b = _marshal_b_fast(res_a, fmap, cls_label, proj_weight, CPF, idxs)
        res_b = _run(ncb, [in_b], [0], tag="B")
        cnt = res_b[0]["o_cnt"].reshape(B, C)
        # fast path assumed every present class has masked pixels; verify.
        if not np.any((lab > 0.5) & (cnt < 0.5)):
            loss = np.float32(res_b[0]["o_loss"].reshape(-1)[0])
            return np.asarray(loss, dtype=np.float32).reshape(())
    loss = _kernel_slow(fmap, cam, cls_label, proj_weight, feature_contrast,
                        hig_thre, low_thre, bg_thre, idxs, CP)
    return np.asarray(loss, dtype=np.float32).reshape(())



# revision 34
# speedup vs baseline: 1.0871x; 1.0235x over previous
"""Trainium2 Bass kernel for nn_CPCLoss (self-contained).

Strategy (8 NeuronCores, full inputs in / full output out):
  NEFF-A, SPMD on 8 cores — core k = (batch b=k//4, row-block blk=k%4 of 112
  dst rows). Each core reads its cam shard [20, 112, 448] and computes:
    * per-pixel top1/second/argmax over classes -> pseudo-label class map
    * A_partial[c] = Wr_blk^T @ onehot(q==c+1) @ Wc  (28x28 bilinear-downsample
      coefficient grid per class; Wr/Wc are the static jax.image.resize
      bilinear matrices) via PE matmuls
    * exact per-class top-256 (values+indices) over the 50176 shard pixels via
      the gpsimd topk instruction; top-32 shipped as merge candidates
  Host only reshapes/concats partials (no arithmetic).
  NEFF-B, 1 core — sums partials, merges exact top-25 per (b,c), builds the
  bilinear gather matrix G, selects coef = count==0 ? G/25 : A/max(count,1),
  fsm = coef @ fmap^T, then runs the 2-step EMA memory-bank scan and emits the
  scalar loss.
"""
import os
import sys

os.environ.setdefault("MYCRO_LOCAL_CACHE", "1")
if "/opt/trn_rl_repo" not in sys.path:
    sys.path.insert(0, "/opt/trn_rl_repo")

from contextlib import ExitStack

import numpy as np

from concourse import bacc, bass_isa, mybir, tile
from concourse.bass_utils import run_bass_kernel_spmd


class _StageDone(Exception):
    pass

f32 = mybir.dt.float32
u32 = mybir.dt.uint32
ALU = mybir.AluOpType
AFT = mybir.ActivationFunctionType
AX = mybir.AxisListType

B, C, D = 2, 20, 256
H = W = 448
FH = FW = 28
K_TOP = 25
NBLK = 4
RB = H // NBLK            # 112
NPIX = RB * W             # 50176
NCAND = 32                # candidates shipped per (core, class)
MARGIN = 0.3


def _make_w1d():
    scale = FH / H
    w = np.zeros((H, FH), dtype=np.float64)
    for x in range(H):
        s = (x + 0.5) * scale - 0.5
        i0 = int(np.floor(s))
        f = s - i0
        for i, wt in ((i0, 1.0 - f), (i0 + 1, f)):
            if 0 <= i < FH:
                w[x, i] += wt
        w[x] /= w[x].sum()
    return w.astype(np.float32)


W1D = _make_w1d()


def _emit_topk(nc, out_ap, in_ap, tokens):
    g = nc.gpsimd
    return g.add_instruction(bass_isa.InstTopk(
        name=f"I-{nc.next_id()}",
        ins=[g.lower_ap(in_ap, for_isa=True)],
        outs=[g.lower_ap(out_ap, for_isa=True)],
        _tokens=tokens, _n=NPIX, _k=256))


# --------------------------------------------------------------------------
# NEFF-A
# --------------------------------------------------------------------------

def _build_a(hig, low, bg, CP=C):
    nc = bacc.Bacc("TRN2", target_bir_lowering=False, debug=False, num_devices=8)

    camv = nc.dram_tensor("camv", [CP, NPIX], f32, kind="ExternalInput").ap()
    labt = nc.dram_tensor("labt", [RB, CP], f32, kind="ExternalInput").ap()
    clst = nc.dram_tensor("clst", [RB, CP], f32, kind="ExternalInput").ap()
    iodt = nc.dram_tensor("iodt", [RB, CP], f32, kind="ExternalInput").ap()
    wrt = nc.dram_tensor("wrt", [RB, 28], f32, kind="ExternalInput").ap()
    wct = nc.dram_tensor("wct", [RB, 4 * 28], f32, kind="ExternalInput").ap()
    idn = nc.dram_tensor("idn", [128, 128], f32, kind="ExternalInput").ap()

    o_a = nc.dram_tensor("o_a", [28, CP * 28], f32, kind="ExternalOutput").ap()
    ntk = (CP + 7) // 8
    tok = [min(8, CP - 8 * t) for t in range(ntk)]
    o_tk = [nc.dram_tensor(f"o_tk{t}", [16 * tok[t], 32], u32,
                           kind="ExternalOutput").ap() for t in range(ntk)]

    thmax = float(max(hig, low, bg))

    with tile.TileContext(nc) as tc, ExitStack() as ctx:
        pool = ctx.enter_context(tc.tile_pool(name="p", bufs=1))
        psum = ctx.enter_context(tc.tile_pool(name="ps", bufs=1, space="PSUM"))
        nv = nc.vector

        VP = pool.tile([RB, CP * W], f32)
        nc.sync.dma_start(VP[:], camv.rearrange("c (r w) -> r c w", w=W))
        VT = []
        for t in range(ntk):
            vt = pool.tile([16 * tok[t], NPIX // 16], f32, name=f"VT{t}")
            nc.sync.dma_start(vt[:], camv[8 * t:8 * t + tok[t]]
                              .rearrange("c (g f) -> (c g) f", f=NPIX // 16))
            VT.append(vt)

        LB = pool.tile([RB, CP], f32); nc.sync.dma_start(LB[:], labt)
        CL = pool.tile([RB, CP], f32); nc.sync.dma_start(CL[:], clst)
        IO = pool.tile([RB, CP], f32); nc.sync.dma_start(IO[:], iodt)
        WR = pool.tile([RB, 28], f32); nc.sync.dma_start(WR[:], wrt)
        WC = pool.tile([RB, 4 * 28], f32); nc.sync.dma_start(WC[:], wct)
        IDN = pool.tile([128, 128], f32); nc.sync.dma_start(IDN[:], idn)

        # ---- pseudo-label phase ----
        V_cw = VP[:].rearrange("p (c w) -> p c w", w=W)
        V_wc = VP[:].rearrange("p (c w) -> p w c", w=W)
        LB_b = LB[:].unsqueeze(2).broadcast_to([RB, CP, W])
        nv.tensor_tensor(out=V_cw, in0=V_cw, in1=LB_b, op=ALU.mult)  # valid in-place

        T1 = pool.tile([RB, W], f32)
        nv.tensor_reduce(out=T1[:], in_=V_wc, axis=AX.X, op=ALU.max)

        GE = pool.tile([RB, CP * W], f32)
        GE_cw = GE[:].rearrange("p (c w) -> p c w", w=W)
        T1_b = T1[:].unsqueeze(1).broadcast_to([RB, CP, W])
        nv.tensor_tensor(out=GE_cw, in0=V_cw, in1=T1_b, op=ALU.is_ge)

        EN = pool.tile([RB, CP * W], f32, tag="scr")
        EN_cw = EN[:].rearrange("p (c w) -> p c w", w=W)
        IO_b = IO[:].unsqueeze(2).broadcast_to([RB, CP, W])
        nv.tensor_tensor(out=EN_cw, in0=GE_cw, in1=IO_b, op=ALU.mult)
        AM = pool.tile([RB, W], f32)
        nv.tensor_reduce(out=AM[:], in_=EN[:].rearrange("p (c w) -> p w c", w=W),
                         axis=AX.X, op=ALU.max)

        MK = pool.tile([RB, CP * W], f32, tag="scr")
        MK_cw = MK[:].rearrange("p (c w) -> p c w", w=W)
        nv.scalar_tensor_tensor(out=MK_cw, in0=GE_cw, scalar=-1e9, in1=V_cw,
                                op0=ALU.mult, op1=ALU.add)
        SC = pool.tile([RB, W], f32)
        nv.tensor_reduce(out=SC[:], in_=MK[:].rearrange("p (c w) -> p w c", w=W),
                         axis=AX.X, op=ALU.max)

        # keep iff top1 >= max(hig,low,bg) and (margin >= 0.3 or top1 <= hig)
        KG = pool.tile([RB, W], f32)
        nv.tensor_scalar(out=KG[:], in0=T1[:], scalar1=thmax, scalar2=None, op0=ALU.is_ge)
        MGOK = pool.tile([RB, W], f32)
        nv.tensor_tensor(out=MGOK[:], in0=T1[:], in1=SC[:], op=ALU.subtract)
        nv.tensor_scalar(out=MGOK[:], in0=MGOK[:], scalar1=MARGIN, scalar2=None, op0=ALU.is_ge)
        LEH = pool.tile([RB, W], f32)
        nv.tensor_scalar(out=LEH[:], in0=T1[:], scalar1=float(hig), scalar2=None, op0=ALU.is_le)
        nv.tensor_tensor(out=MGOK[:], in0=MGOK[:], in1=LEH[:], op=ALU.max)
        nv.tensor_tensor(out=KG[:], in0=KG[:], in1=MGOK[:], op=ALU.mult)
        Q = pool.tile([RB, W], f32)
        nv.tensor_scalar(out=Q[:], in0=AM[:], scalar1=-1.0, scalar2=float(CP + 1),
                         op0=ALU.mult, op1=ALU.add)
        nv.tensor_tensor(out=Q[:], in0=Q[:], in1=KG[:], op=ALU.mult)

        # ---- q transpose + one-hot EQT + matmuls for A ----
        QT = pool.tile([RB, 4 * RB], f32)
        for u in range(4):
            QTP = psum.tile([RB, RB], f32, tag="qtp")
            nc.tensor.transpose(QTP[:], Q[:, u * RB:(u + 1) * RB], IDN[:RB, :RB])
            nc.scalar.copy(QT[:, u * RB:(u + 1) * RB], QTP[:])

        EQT = pool.tile([RB, 4 * CP * RB], f32)
        for u in range(4):
            sl = EQT[:, u * CP * RB:(u + 1) * CP * RB]
            sl_cw = sl.rearrange("p (c r) -> p c r", r=RB)
            QT_b = QT[:, u * RB:(u + 1) * RB].unsqueeze(1).broadcast_to([RB, CP, RB])
            CL_b = CL[:].unsqueeze(2).broadcast_to([RB, CP, RB])
            nv.tensor_tensor(out=sl_cw, in0=QT_b, in1=CL_b, op=ALU.is_equal)
        # PSUM bank = 512 f32: hold 5 classes (140 cols) per bank-tile
        ngrp = (CP + 4) // 5
        T0sb = pool.tile([RB, CP * 28], f32)
        Asb = pool.tile([28, CP * 28], f32)
        T0ps = [psum.tile([RB, 5 * 28], f32, name=f"t0ps{i}", tag="accps", bufs=4)
                for i in range(ngrp)]
        Aps = [psum.tile([28, 5 * 28], f32, name=f"aps{i}", tag="accps", bufs=4)
               for i in range(ngrp)]
        for c in range(CP):
            grp, off = c // 5, (c % 5) * 28
            for u in range(4):
                nc.tensor.matmul(
                    T0ps[grp][:, off:off + 28],
                    lhsT=EQT[:, u * CP * RB + c * RB:u * CP * RB + (c + 1) * RB],
                    rhs=WC[:, u * 28:(u + 1) * 28],
                    start=(u == 0), stop=(u == 3))
        for i in range(ngrp):
            w0 = i * 140
            w1 = min(w0 + 140, CP * 28)
            nc.scalar.copy(T0sb[:, w0:w1], T0ps[i][:, 0:w1 - w0])
        for c in range(CP):
            grp, off = c // 5, (c % 5) * 28
            nc.tensor.matmul(Aps[grp][:, off:off + 28], lhsT=WR[:],
                             rhs=T0sb[:, c * 28:(c + 1) * 28], start=True, stop=True)
        for i in range(ngrp):
            w0 = i * 140
            w1 = min(w0 + 140, CP * 28)
            nc.scalar.copy(Asb[:, w0:w1], Aps[i][:, 0:w1 - w0])
        nc.sync.dma_start(o_a, Asb[:])

        # ---- per-class topk ----
        for t in range(ntk):
            tkt = pool.tile([16 * tok[t], 32], u32, name=f"TK{t}")
            _emit_topk(nc, tkt[:], VT[t][:], tokens=tok[t])
            nc.sync.dma_start(o_tk[t], tkt[:])

    nc.compile()
    return nc


# --------------------------------------------------------------------------
# NEFF-B
# --------------------------------------------------------------------------

def _build_b(stage=99):
    nc = bacc.Bacc("TRN2", target_bir_lowering=False, debug=False, num_devices=1)
    P = B * C  # 40 (b,c) pairs

    ain = nc.dram_tensor("ain", [P, 784 * NBLK], f32, kind="ExternalInput").ap()
    cdv = nc.dram_tensor("cdv", [P, NBLK * NCAND], f32, kind="ExternalInput").ap()
    cdi = nc.dram_tensor("cdi", [P, NBLK * NCAND], u32, kind="ExternalInput").ap()
    bbs = nc.dram_tensor("bbs", [P, NBLK * NCAND], f32, kind="ExternalInput").ap()
    fmi = nc.dram_tensor("fmi", [112, 7 * B * D], f32, kind="ExternalInput").ap()
    prj = nc.dram_tensor("prj", [128, 2 * C], f32, kind="ExternalInput").ap()
    lab = nc.dram_tensor("lab", [P, 1], f32, kind="ExternalInput").ap()
    lab2 = nc.dram_tensor("lab2", [C, B], f32, kind="ExternalInput").ap()
    fc0 = nc.dram_tensor("fc0", [C, D], f32, kind="ExternalInput").ap()
    eye = nc.dram_tensor("eye", [C, C], f32, kind="ExternalInput").ap()
    i28 = nc.dram_tensor("i28", [128, 28], f32, kind="ExternalInput").ap()
    i128 = nc.dram_tensor("i128", [P, 128], f32, kind="ExternalInput").ap()
    mmb = nc.dram_tensor("mmb", [128, 76], f32, kind="ExternalInput").ap()
    rnk = nc.dram_tensor("rnk", [P, NCAND], f32, kind="ExternalInput").ap()
    idn = nc.dram_tensor("idn", [128, 128], f32, kind="ExternalInput").ap()

    o_loss = nc.dram_tensor("o_loss", [1, 1], f32, kind="ExternalOutput").ap()
    o_dbg = nc.dram_tensor("o_dbg", [128, 1024], f32, kind="ExternalOutput").ap()

    NC128 = NBLK * NCAND  # 128 candidates per pair

    try:
      with tile.TileContext(nc) as tc, ExitStack() as ctx:
        pool = ctx.enter_context(tc.tile_pool(name="p", bufs=1))
        psum = ctx.enter_context(tc.tile_pool(name="ps", bufs=1, space="PSUM"))
        nv = nc.vector
        ns = nc.scalar

        AIN = pool.tile([P, 784 * NBLK], f32); nc.sync.dma_start(AIN[:], ain)
        CV = pool.tile([P, NC128], f32); nc.sync.dma_start(CV[:], cdv)
        CI = pool.tile([P, NC128], u32); nc.sync.dma_start(CI[:], cdi)
        BBS = pool.tile([P, NC128], f32); nc.sync.dma_start(BBS[:], bbs)
        FM = pool.tile([112, 7 * B * D], f32); nc.sync.dma_start(FM[:], fmi)
        PJT = pool.tile([128, 2 * C], f32); nc.sync.dma_start(PJT[:], prj)
        LAB = pool.tile([P, 1], f32); nc.sync.dma_start(LAB[:], lab)
        LAB2 = pool.tile([C, B], f32); nc.sync.dma_start(LAB2[:], lab2)
        FC = pool.tile([C, D], f32); nc.sync.dma_start(FC[:], fc0)
        EYE = pool.tile([C, C], f32); nc.sync.dma_start(EYE[:], eye)
        I28 = pool.tile([128, 28], f32); nc.sync.dma_start(I28[:], i28)
        I128 = pool.tile([P, 128], f32); nc.sync.dma_start(I128[:], i128)
        MMB = pool.tile([128, 76], f32); nc.sync.dma_start(MMB[:], mmb)
        RNK = pool.tile([P, NCAND], f32); nc.sync.dma_start(RNK[:], rnk)
        IDN = pool.tile([128, 128], f32); nc.sync.dma_start(IDN[:], idn)

        # ---- A, counts ----
        A = pool.tile([P, 784], f32)
        nv.tensor_reduce(out=A[:], in_=AIN[:].rearrange("p (s k) -> p s k", k=NBLK),
                         axis=AX.X, op=ALU.add)
        CNT = pool.tile([P, 1], f32)
        nv.tensor_reduce(out=CNT[:], in_=A[:], axis=AX.X, op=ALU.add)
        ISZ = pool.tile([P, 1], u32)
        nv.tensor_scalar(out=ISZ[:], in0=CNT[:], scalar1=0.5, scalar2=None, op0=ALU.is_lt)
        DEN = pool.tile([P, 1], f32)
        nv.tensor_scalar(out=DEN[:], in0=CNT[:], scalar1=1.0, scalar2=None, op0=ALU.max)

        # ---- merge top-32 of 128 candidates ----
        CIF = pool.tile([P, NC128], f32)
        nv.tensor_copy(CIF[:], CI[:])
        nv.tensor_tensor(out=CIF[:], in0=CIF[:], in1=BBS[:], op=ALU.add)
        CVa = pool.tile([P, NC128], f32)
        nv.tensor_copy(CVa[:], CV[:])
        MV = pool.tile([P, NCAND], f32)
        MP = pool.tile([P, NCAND], u32)
        for r in range(4):
            nv.max(out=MV[:, r * 8:(r + 1) * 8], in_=CVa[:])
            nv.max_index(out=MP[:, r * 8:(r + 1) * 8],
                         in_max=MV[:, r * 8:(r + 1) * 8], in_values=CVa[:])
            nv.match_replace(out=CVa[:], in_to_replace=MV[:, r * 8:(r + 1) * 8],
                             in_values=CVa[:], imm_value=-1.0)
        MPF = pool.tile([P, NCAND], f32)
        nv.tensor_copy(MPF[:], MP[:])
        # gather global idx at positions
        EQP = pool.tile([P, NCAND * 128], f32)
        EQP_v = EQP[:].rearrange("p (k q) -> p k q", q=128)
        nv.tensor_tensor(out=EQP_v, in0=MPF[:].unsqueeze(2).broadcast_to([P, NCAND, 128]),
                         in1=I128[:].unsqueeze(1).broadcast_to([P, NCAND, 128]),
                         op=ALU.is_equal)
        nv.tensor_tensor(out=EQP_v, in0=EQP_v,
                         in1=CIF[:].unsqueeze(1).broadcast_to([P, NCAND, 128]), op=ALU.mult)
        GIX = pool.tile([P, NCAND], f32)
        nv.tensor_reduce(out=GIX[:], in_=EQP_v, axis=AX.X, op=ALU.max)

        if stage <= 1:
            DBG = pool.tile([P, 64], f32)
            nv.tensor_copy(DBG[:, 0:32], GIX[:])
            nv.tensor_copy(DBG[:, 32:64], MPF[:])
            nc.sync.dma_start(o_dbg[0:P, 0:64], DBG[:])
        # ---- interpolation coefficients ----
        def ts(dst, src, s1, s2, op0, op1=None):
            nv.tensor_scalar(out=dst, in0=src, scalar1=s1, scalar2=s2, op0=op0,
                             **({"op1": op1} if op1 is not None else {}))

        if stage <= 1:
            OUTZ = pool.tile([1, 1], f32)
            nv.memset(OUTZ[:], 0.0)
            nc.sync.dma_start(o_loss, OUTZ[:])
            raise _StageDone()

        i32 = mybir.dt.int32

        def floor_pos(XX, pfx):
            """floor(x) for x>=0: round-to-nearest (f32->i32->f32 copy) then
            subtract 1 where round went up."""
            RI = pool.tile([P, NCAND], i32, name=f"{pfx}_ri", tag=f"{pfx}_ri")
            nv.tensor_copy(RI[:], XX[:])
            RF = pool.tile([P, NCAND], f32, name=f"{pfx}_rf", tag=f"{pfx}_rf")
            nv.tensor_copy(RF[:], RI[:])
            GT = pool.tile([P, NCAND], f32, name=f"{pfx}_gt", tag=f"{pfx}_gt")
            nv.tensor_tensor(out=GT[:], in0=RF[:], in1=XX[:], op=ALU.is_gt)
            nv.tensor_tensor(out=RF[:], in0=RF[:], in1=GT[:], op=ALU.subtract)
            return RF

        TT = pool.tile([P, NCAND], f32)
        ts(TT[:], GIX[:], 1.0 / 448.0, None, ALU.mult)
        HH = floor_pos(TT, "fh")
        WW = pool.tile([P, NCAND], f32)
        nv.scalar_tensor_tensor(out=WW[:], in0=HH[:], scalar=-448.0, in1=GIX[:],
                                op0=ALU.mult, op1=ALU.add)

        def coeffs(XX, pfx):
            U = pool.tile([P, NCAND], f32, name=f"{pfx}_u", tag=f"{pfx}_u")
            ts(U[:], XX[:], 8.5, 1.0 / 16.0, ALU.add, ALU.mult)
            FL = floor_pos(U, f"{pfx}_flr")
            F = pool.tile([P, NCAND], f32, name=f"{pfx}_f", tag=f"{pfx}_f")
            nv.tensor_tensor(out=F[:], in0=U[:], in1=FL[:], op=ALU.subtract)
            X0 = pool.tile([P, NCAND], f32, name=f"{pfx}_x0", tag=f"{pfx}_x0")
            ts(X0[:], FL[:], 1.0, None, ALU.subtract)
            ts(X0[:], X0[:], 0.0, 27.0, ALU.max, ALU.min)
            X1 = pool.tile([P, NCAND], f32, name=f"{pfx}_x1", tag=f"{pfx}_x1")
            ts(X1[:], FL[:], 0.0, 27.0, ALU.max, ALU.min)
            W1 = F
            W0 = pool.tile([P, NCAND], f32, name=f"{pfx}_w0", tag=f"{pfx}_w0")
            ts(W0[:], F[:], -1.0, 1.0, ALU.mult, ALU.add)
            return X0, X1, W0, W1

        I0, I1, WH0, WH1 = coeffs(HH, "ch")
        J0, J1, WWA, WWB = coeffs(WW, "cw")
        WW0 = pool.tile([P, NCAND], f32)
        nv.tensor_tensor(out=WW0[:], in0=WWA[:], in1=RNK[:], op=ALU.mult)
        WW1 = pool.tile([P, NCAND], f32)
        nv.tensor_tensor(out=WW1[:], in0=WWB[:], in1=RNK[:], op=ALU.mult)

        if stage == 2:
            DBG2 = pool.tile([P, 128], f32)
            for i, t in enumerate([I0, I1, WH0, WH1]):
                nv.tensor_copy(DBG2[:, i * 32:(i + 1) * 32], t[:])
            nc.sync.dma_start(o_dbg[0:P, 0:128], DBG2[:])
        # ---- stage (pair,k)-flatten and G build ----
        STG = pool.tile([P, NCAND * 8], f32)
        STG_v = STG[:].rearrange("p (k a) -> p k a", a=8)
        for idx, arr in enumerate([I0, I1, WH0, WH1, J0, J1, WW0, WW1]):
            nv.tensor_copy(STG_v[:, :, idx:idx + 1], arr[:].unsqueeze(2))

        if stage == 2:
            OUTZ = pool.tile([1, 1], f32)
            nv.memset(OUTZ[:], 0.0)
            nc.sync.dma_start(o_loss, OUTZ[:])
            raise _StageDone()

        FLT = pool.tile([128, 80], f32)
        for g in range(10):
            nc.sync.dma_start(
                FLT[:, g * 8:(g + 1) * 8],
                STG[g * 4:(g + 1) * 4, :].rearrange("p (k a) -> p k a", a=8))

        G = pool.tile([P, 784], f32)
        GpsA = psum.tile([P, 392], f32)
        GpsB = psum.tile([P, 392], f32)
        for g in range(10):
            col = lambda i: FLT[:, g * 8 + i:g * 8 + i + 1]
            EQR0 = pool.tile([128, 28], f32, tag="eqr", bufs=2)
            nv.tensor_scalar(out=EQR0[:], in0=I28[:], scalar1=col(0), scalar2=None,
                             op0=ALU.is_equal)
            RQ = pool.tile([128, 28], f32, tag="rq", bufs=2)
            nv.tensor_scalar(out=RQ[:], in0=EQR0[:], scalar1=col(2), scalar2=None,
                             op0=ALU.mult)
            EQR1 = pool.tile([128, 28], f32, tag="eqr2", bufs=2)
            nv.tensor_scalar(out=EQR1[:], in0=I28[:], scalar1=col(1), scalar2=None,
                             op0=ALU.is_equal)
            nv.scalar_tensor_tensor(out=RQ[:], in0=EQR1[:], scalar=col(3), in1=RQ[:],
                                    op0=ALU.mult, op1=ALU.add)
            EQC0 = pool.tile([128, 28], f32, tag="eqr", bufs=2)
            nv.tensor_scalar(out=EQC0[:], in0=I28[:], scalar1=col(4), scalar2=None,
                             op0=ALU.is_equal)
            CQ = pool.tile([128, 28], f32, tag="cq", bufs=2)
            nv.tensor_scalar(out=CQ[:], in0=EQC0[:], scalar1=col(6), scalar2=None,
                             op0=ALU.mult)
            EQC1 = pool.tile([128, 28], f32, tag="eqr2", bufs=2)
            nv.tensor_scalar(out=EQC1[:], in0=I28[:], scalar1=col(5), scalar2=None,
                             op0=ALU.is_equal)
            nv.scalar_tensor_tensor(out=CQ[:], in0=EQC1[:], scalar=col(7), in1=CQ[:],
                                    op0=ALU.mult, op1=ALU.add)
            RHS = pool.tile([128, 784], f32, tag="rhs", bufs=2)
            nv.tensor_tensor(out=RHS[:].rearrange("p (a b) -> p a b", b=28),
                             in0=RQ[:].unsqueeze(2).broadcast_to([128, 28, 28]),
                             in1=CQ[:].unsqueeze(1).broadcast_to([128, 28, 28]),
                             op=ALU.mult)
            # band-membership lhsT: col j of MMB[:, 36-4g : 76-4g] is
            # one-hot(q//32 == j-4g) -> group g's 4 pairs land on rows 4g..4g+3
            lhsT_g = MMB[:, 36 - 4 * g:76 - 4 * g]
            nc.tensor.matmul(GpsA[:], lhsT=lhsT_g, rhs=RHS[:, 0:392],
                             start=(g == 0), stop=(g == 9))
            nc.tensor.matmul(GpsB[:], lhsT=lhsT_g, rhs=RHS[:, 392:784],
                             start=(g == 0), stop=(g == 9))
        ns.copy(G[:, 0:392], GpsA[:])
        ns.copy(G[:, 392:784], GpsB[:])

        if stage == 3:
            nc.sync.dma_start(o_dbg[0:P, 0:784], G[:])
        if stage == 35:
            nc.sync.dma_start(o_dbg[0:128, 0:80], FLT[:])
        # ---- coef + fsm ----
        if stage in (3, 35):
            OUTZ = pool.tile([1, 1], f32)
            nv.memset(OUTZ[:], 0.0)
            nc.sync.dma_start(o_loss, OUTZ[:])
            raise _StageDone()

        RDEN = pool.tile([P, 1], f32)
        nv.reciprocal(RDEN[:], DEN[:])
        AMN = pool.tile([P, 784], f32)
        nv.tensor_scalar(out=AMN[:], in0=A[:], scalar1=RDEN[:], scalar2=None, op0=ALU.mult)
        COEF = pool.tile([P, 784], f32)
        nv.select(COEF[:], ISZ[:].broadcast_to([P, 784]), G[:], AMN[:])
        nv.tensor_scalar(out=COEF[:], in0=COEF[:], scalar1=LAB[:], scalar2=None, op0=ALU.mult)

        CT = pool.tile([RB, 7 * P], f32)
        for u in range(7):
            TPS = psum.tile([RB, P], f32, tag="tps", bufs=2)
            nc.tensor.transpose(TPS[:], COEF[:, u * RB:(u + 1) * RB], IDN[:P, :P])
            ns.copy(CT[:, u * P:(u + 1) * P], TPS[:])

        FSM = pool.tile([C, B * D], f32)
        for b2 in range(B):
            FSps = psum.tile([C, D], f32, tag="fsps")
            for u in range(7):
                nc.tensor.matmul(FSps[:], lhsT=CT[:, u * P + b2 * C:u * P + (b2 + 1) * C],
                                 rhs=FM[:, u * (B * D) + b2 * D:u * (B * D) + (b2 + 1) * D],
                                 start=(u == 0), stop=(u == 6))
            ns.copy(FSM[:, b2 * D:(b2 + 1) * D], FSps[:])

        if stage == 4:
            nc.sync.dma_start(o_dbg[0:C, 0:B * D], FSM[:])
        # ---- scan ----
        if stage == 4:
            OUTZ = pool.tile([1, 1], f32)
            nv.memset(OUTZ[:], 0.0)
            nc.sync.dma_start(o_loss, OUTZ[:])
            raise _StageDone()

        ONES20 = pool.tile([C, 1], f32)
        nv.memset(ONES20[:], 1.0)
        LC = pool.tile([1, 1], f32); nv.memset(LC[:], 0.0)
        CCF = pool.tile([1, 1], f32); nv.memset(CCF[:], 0.0)
        SCR = pool.tile([C, D], f32, tag="scr")
        SCR2 = pool.tile([C, C], f32, tag="scr2")

        def l2norm_div(dst, src):
            nn2 = pool.tile([C, 1], f32, tag="nn2")
            nv.tensor_tensor(out=SCR[:], in0=src, in1=src, op=ALU.mult)
            nv.tensor_reduce(out=nn2[:], in_=SCR[:], axis=AX.X, op=ALU.add)
            nr = pool.tile([C, 1], f32, tag="nr")
            ns.activation(nr[:], nn2[:], AFT.Sqrt)
            nv.tensor_scalar(out=nr[:], in0=nr[:], scalar1=1e-12, scalar2=None, op0=ALU.max)
            rn = pool.tile([C, 1], f32, tag="rn")
            nv.reciprocal(rn[:], nr[:])
            nv.tensor_scalar(out=dst, in0=src, scalar1=rn[:], scalar2=None, op0=ALU.mult)

        for b2 in range(B):
            FSMb = FSM[:, b2 * D:(b2 + 1) * D]
            presb = LAB2[:, b2:b2 + 1]

            FSMN = pool.tile([C, D], f32, tag="fsmn")
            l2norm_div(FSMN[:], FSMb)
            FCN = pool.tile([C, D], f32, tag="fcn")
            l2norm_div(FCN[:], FC[:])

            # transposes of fsm (raw), fsm_n, fc_n -> [128, C] chunks
            TRS = {}
            for nm, srct in (("fsm", FSMb), ("fsmn", FSMN[:]), ("fcn", FCN[:])):
                dst = pool.tile([128, 2 * C], f32, tag=f"tr_{nm}", name=f"tr_{nm}_{b2}")
                for h2 in range(2):
                    TPS4 = psum.tile([128, C], f32, tag="tps", bufs=2)
                    nc.tensor.transpose(TPS4[:], srct[:, h2 * 128:(h2 + 1) * 128],
                                        IDN[:C, :C])
                    ns.copy(dst[:, h2 * C:(h2 + 1) * C], TPS4[:])
                TRS[nm] = dst

            COSps = psum.tile([C, C], f32, tag="cosps")
            for h2 in range(2):
                nc.tensor.matmul(COSps[:], lhsT=TRS["fsmn"][:, h2 * C:(h2 + 1) * C],
                                 rhs=TRS["fcn"][:, h2 * C:(h2 + 1) * C],
                                 start=(h2 == 0), stop=(h2 == 1))
            COSC = pool.tile([C, C], f32, tag="cosc")
            ns.activation(COSC[:], COSps[:], AFT.Abs)
            nv.tensor_scalar(out=COSC[:], in0=COSC[:], scalar1=1e-5, scalar2=1.0 - 1e-5,
                             op0=ALU.max, op1=ALU.min)
            LGC = pool.tile([C, C], f32, tag="lgc")
            ns.activation(LGC[:], COSC[:], AFT.Ln)
            OM = pool.tile([C, C], f32, tag="om")
            nv.tensor_scalar(out=OM[:], in0=COSC[:], scalar1=-1.0, scalar2=1.0,
                             op0=ALU.mult, op1=ALU.add)
            LOM = pool.tile([C, C], f32, tag="lom")
            ns.activation(LOM[:], OM[:], AFT.Ln)

            IDM = pool.tile([C, C], f32, tag="idm")
            nv.tensor_scalar(out=IDM[:], in0=EYE[:], scalar1=presb, scalar2=None, op0=ALU.mult)
            DIF = pool.tile([C, C], f32, tag="dif")
            nv.tensor_tensor(out=DIF[:], in0=LGC[:], in1=LOM[:], op=ALU.subtract)
            CCFD = pool.tile([C, 1], f32, tag="ccfd")
            nv.tensor_tensor(out=SCR2[:], in0=IDM[:], in1=DIF[:], op=ALU.mult)
            nv.tensor_reduce(out=CCFD[:], in_=SCR2[:], axis=AX.X, op=ALU.add)
            R1 = pool.tile([C, 1], f32, tag="r1")
            nv.tensor_reduce(out=R1[:], in_=LOM[:], axis=AX.X, op=ALU.add)
            nv.tensor_tensor(out=CCFD[:], in0=CCFD[:], in1=R1[:], op=ALU.add)

            COSM = pool.tile([C, C], f32, tag="cosm")
            nv.scalar_tensor_tensor(out=COSM[:], in0=EYE[:], scalar=-1e9, in1=COSC[:],
                                    op0=ALU.mult, op1=ALU.add)
            OFF = pool.tile([C, 1], f32, tag="off")
            nv.tensor_reduce(out=OFF[:], in_=COSM[:], axis=AX.X, op=ALU.max)
            QUAL = pool.tile([C, 1], f32, tag="qual")
            nv.tensor_scalar(out=QUAL[:], in0=OFF[:], scalar1=0.6, scalar2=None, op0=ALU.is_lt)
            nv.tensor_tensor(out=QUAL[:], in0=QUAL[:], in1=presb, op=ALU.mult)

            LOGps = psum.tile([C, C], f32, tag="cosps")
            for h2 in range(2):
                nc.tensor.matmul(LOGps[:], lhsT=TRS["fsm"][:, h2 * C:(h2 + 1) * C],
                                 rhs=PJT[:, h2 * C:(h2 + 1) * C],
                                 start=(h2 == 0), stop=(h2 == 1))
            MX = pool.tile([C, 1], f32, tag="mx")
            nv.tensor_reduce(out=MX[:], in_=LOGps, axis=AX.X, op=ALU.max)
            XT = pool.tile([C, C], f32, tag="xt")
            nv.tensor_scalar(out=XT[:], in0=LOGps, scalar1=MX[:], scalar2=None,
                             op0=ALU.subtract)
            ET = pool.tile([C, C], f32, tag="et")
            ns.activation(ET[:], XT[:], AFT.Exp)
            SM = pool.tile([C, 1], f32, tag="sm")
            nv.tensor_reduce(out=SM[:], in_=ET[:], axis=AX.X, op=ALU.add)
            LGS = pool.tile([C, 1], f32, tag="lgs")
            ns.activation(LGS[:], SM[:], AFT.Ln)
            LGP = pool.tile([C, C], f32, tag="lgp")
            nv.tensor_scalar(out=LGP[:], in0=XT[:], scalar1=LGS[:], scalar2=-100.0,
                             op0=ALU.subtract, op1=ALU.max)
            SME = pool.tile([C, C], f32, tag="sme")
            nv.tensor_tensor(out=SME[:], in0=SM[:].broadcast_to([C, C]), in1=ET[:],
                             op=ALU.subtract)
            LSME = pool.tile([C, C], f32, tag="lsme")
            ns.activation(LSME[:], SME[:], AFT.Ln)
            L1P = pool.tile([C, C], f32, tag="l1p")
            nv.tensor_scalar(out=L1P[:], in0=LSME[:], scalar1=LGS[:], scalar2=-100.0,
                             op0=ALU.subtract, op1=ALU.max)

            DD = pool.tile([C, C], f32, tag="dd")
            nv.tensor_tensor(out=DD[:], in0=LGP[:], in1=L1P[:], op=ALU.subtract)
            DDG = pool.tile([C, 1], f32, tag="ddg")
            nv.tensor_tensor(out=SCR2[:], in0=EYE[:], in1=DD[:], op=ALU.mult)
            nv.tensor_reduce(out=DDG[:], in_=SCR2[:], axis=AX.X, op=ALU.add)
            RSM = pool.tile([C, 1], f32, tag="rsm")
            nv.tensor_reduce(out=RSM[:], in_=L1P[:], axis=AX.X, op=ALU.add)
            TERM = pool.tile([C, 1], f32, tag="term")
            nv.tensor_tensor(out=TERM[:], in0=DDG[:], in1=RSM[:], op=ALU.add)
            nv.tensor_scalar(out=TERM[:], in0=TERM[:], scalar1=-1.0 / C, scalar2=None,
                             op0=ALU.mult)
            CONTR = pool.tile([C, 1], f32, tag="contr")
            nv.tensor_tensor(out=CONTR[:], in0=TERM[:], in1=QUAL[:], op=ALU.mult)

            PR = pool.tile([C, 3], f32, tag="pr")
            nv.tensor_copy(PR[:, 0:1], QUAL[:])
            nv.tensor_copy(PR[:, 1:2], CONTR[:])
            nv.tensor_copy(PR[:, 2:3], CCFD[:])
            REDps = psum.tile([1, 3], f32, tag="redps")
            nc.tensor.matmul(REDps[:], lhsT=ONES20[:], rhs=PR[:], start=True, stop=True)
            RED = pool.tile([1, 3], f32, tag="red")
            ns.copy(RED[:], REDps[:])

            # loss_cls = (loss_cls + S) / max(n, 1)   (divide-by-1 when n==0)
            nv.tensor_tensor(out=LC[:], in0=LC[:], in1=RED[:, 1:2], op=ALU.add)
            NB1 = pool.tile([1, 1], f32, tag="nb1")
            nv.tensor_scalar(out=NB1[:], in0=RED[:, 0:1], scalar1=1.0, scalar2=None,
                             op0=ALU.max)
            RNB = pool.tile([1, 1], f32, tag="rnb")
            nv.reciprocal(RNB[:], NB1[:])
            nv.tensor_scalar(out=LC[:], in0=LC[:], scalar1=RNB[:], scalar2=None,
                             op0=ALU.mult)
            # loss_ccf += -(1/400) * ccf_sum
            nv.scalar_tensor_tensor(out=CCF[:], in0=RED[:, 2:3], scalar=-1.0 / (C * C),
                                    in1=CCF[:], op0=ALU.mult, op1=ALU.add)

            # fc = fc + 0.05 * qual * (fsm - fc)
            DFC = pool.tile([C, D], f32, tag="dfc")
            nv.tensor_tensor(out=DFC[:], in0=FSMb, in1=FC[:], op=ALU.subtract)
            Q05 = pool.tile([C, 1], f32, tag="q05")
            nv.tensor_scalar(out=Q05[:], in0=QUAL[:], scalar1=0.05, scalar2=None,
                             op0=ALU.mult)
            nv.scalar_tensor_tensor(out=FC[:], in0=DFC[:], scalar=Q05[:], in1=FC[:],
                                    op0=ALU.mult, op1=ALU.add)

        OUT = pool.tile([1, 1], f32)
        nv.tensor_tensor(out=OUT[:], in0=LC[:], in1=CCF[:], op=ALU.add)
        nc.sync.dma_start(o_loss, OUT[:])
    except _StageDone:
        pass

    nc.compile()
    return nc


# --------------------------------------------------------------------------
# Fast path (no top-k: valid when every present class has count > 0).
# --------------------------------------------------------------------------

bf16 = mybir.dt.bfloat16
f16 = mybir.dt.float16


def _emit_tree(nc, pool, src, n, width, op, pfx, dt=None, part=None):
    """Binary-tree reduce over n leaves of `width` cols each -> [P, width]."""
    nv = nc.vector
    dt = bf16 if dt is None else dt
    part = RB if part is None else part
    cur = src
    lvl = 0
    while n > 1:
        h = n // 2
        odd = n - 2 * h
        dst = pool.tile([part, h * width], dt, name=f"{pfx}_l{lvl}")
        nv.tensor_tensor(out=dst[:], in0=cur[:, :h * width],
                         in1=cur[:, h * width:2 * h * width], op=op)
        if odd:
            nv.tensor_tensor(out=dst[:, :width], in0=dst[:, :width],
                             in1=cur[:, 2 * h * width:(2 * h + 1) * width], op=op)
        cur, n, lvl = dst, h, lvl + 1
    return cur


def _build_a_fast(hig, low, bg, CP):
    nc = bacc.Bacc("TRN2", target_bir_lowering=False, debug=False, num_devices=8)

    camv = nc.dram_tensor("camv", [CP, NPIX], bf16, kind="ExternalInput").ap()
    wrt = nc.dram_tensor("wrt", [RB, 28], bf16, kind="ExternalInput").ap()
    wct = nc.dram_tensor("wct", [RB, 4 * 28], f16, kind="ExternalInput").ap()
    o_a = nc.dram_tensor("o_a", [28, CP * 28], f32, kind="ExternalOutput").ap()

    thmax = float(max(hig, low, bg))
    # class groups of <=4 (PSUM bank = 512 f32 = 4 classes x 4 u x 28)
    grps = []
    c0 = 0
    while c0 < CP:
        n = min(4, CP - c0)
        grps.append((c0, n))
        c0 += n
    ch = (CP + 1) // 2  # class-split DMA halves

    with tile.TileContext(nc) as tc, ExitStack() as ctx:
        pool = ctx.enter_context(tc.tile_pool(name="p", bufs=1))
        psum = ctx.enter_context(tc.tile_pool(name="ps", bufs=1, space="PSUM"))
        nv = nc.vector
        ns = nc.scalar

        VP = pool.tile([RB, CP * W], bf16)
        # class-quarters so partial max trees overlap the later DMA chunks
        qs = []
        q0 = 0
        while q0 < CP:
            qn = min(max(1, (CP + 3) // 4), CP - q0)
            qs.append((q0, qn))
            q0 += qn
        for (q0_, qn_) in qs:
            nc.sync.dma_start(VP[:, q0_ * W:(q0_ + qn_) * W],
                              camv[q0_:q0_ + qn_].rearrange("c (r w) -> r c w",
                                                            w=W))
        WR = pool.tile([RB, 28], bf16)
        nc.sync.dma_start(WR[:], wrt)
        WC = pool.tile([RB, 4 * 28], f16)
        nc.sync.dma_start(WC[:], wct)

        # ---- per-pixel keep-gate (bf16, w innermost so TTs hit 2x mode) ----
        parts = [_emit_tree(nc, pool, VP[:, a * W:(a + n) * W], n, W, ALU.max,
                            f"t1q{i}") for i, (a, n) in enumerate(qs)]
        while len(parts) > 1:
            nxt = []
            for i in range(0, len(parts) - 1, 2):
                t = pool.tile([RB, W], bf16, name=f"t1m{len(parts)}_{i}")
                nv.tensor_tensor(out=t[:], in0=parts[i][:], in1=parts[i + 1][:],
                                 op=ALU.max)
                nxt.append(t)
            if len(parts) % 2:
                nxt.append(parts[-1])
            parts = nxt
        T1 = parts[0]
        T13 = pool.tile([RB, W], bf16)
        nv.tensor_scalar(out=T13[:], in0=T1[:], scalar1=-MARGIN, scalar2=None,
                         op0=ALU.add)
        NG = pool.tile([RB, CP * W], bf16)
        NG_cw = NG[:].rearrange("p (c w) -> p c w", w=W)
        V_cw = VP[:].rearrange("p (c w) -> p c w", w=W)
        nv.tensor_tensor(out=NG_cw, in0=V_cw,
                         in1=T13[:].unsqueeze(1).broadcast_to([RB, CP, W]),
                         op=ALU.is_gt)
        # first tree level split likewise
        h1 = CP // 2
        odd1 = CP - 2 * h1
        NGH = pool.tile([RB, h1 * W], bf16)
        dvp = max(1, h1 - 1)
        nv.tensor_tensor(out=NGH[:, :dvp * W], in0=NG[:, :dvp * W],
                         in1=NG[:, h1 * W:(h1 + dvp) * W], op=ALU.add)
        if h1 > dvp:
            nc.gpsimd.tensor_tensor(out=NGH[:, dvp * W:h1 * W],
                                    in0=NG[:, dvp * W:h1 * W],
                                    in1=NG[:, (h1 + dvp) * W:2 * h1 * W],
                                    op=ALU.add)
        if odd1:
            nv.tensor_tensor(out=NGH[:, :W], in0=NGH[:, :W],
                             in1=NG[:, 2 * h1 * W:(2 * h1 + 1) * W], op=ALU.add)
        NGS = _emit_tree(nc, pool, NGH, h1, W, ALU.add, "ngs")

        # keep iff t1 >= thmax and (exactly one class above t1-0.3 or t1 <= hig)
        LEH = pool.tile([RB, W], bf16)
        nv.tensor_scalar(out=LEH[:], in0=T1[:], scalar1=float(hig),
                         scalar2=None, op0=ALU.is_le)
        K1 = pool.tile([RB, W], bf16)
        nv.tensor_scalar(out=K1[:], in0=T1[:], scalar1=thmax,
                         scalar2=None, op0=ALU.is_ge)
        MOK = pool.tile([RB, W], bf16)
        nv.tensor_scalar(out=MOK[:], in0=NGS[:], scalar1=1.5, scalar2=None,
                         op0=ALU.is_lt)
        nv.tensor_tensor(out=MOK[:], in0=MOK[:], in1=LEH[:], op=ALU.max)
        KEEP = pool.tile([RB, W], bf16)
        nv.tensor_tensor(out=KEEP[:], in0=K1[:], in1=MOK[:], op=ALU.mult)
        # threshold map: t1 where kept else 2.0 (cam < 1, so M == 0 there).
        # Kept pixels have margin >= 0.3 -> no tie at the max -> M is one-hot.
        # KEEP is exactly 0/1 so this select-by-arithmetic is exact in bf16.
        T1K = pool.tile([RB, W], bf16)
        nv.tensor_tensor(out=T1K[:], in0=T1[:], in1=KEEP[:], op=ALU.mult)
        NK2 = pool.tile([RB, W], bf16)
        nv.tensor_scalar(out=NK2[:], in0=KEEP[:], scalar1=-2.0,
                         scalar2=2.0, op0=ALU.mult, op1=ALU.add)
        T1X = pool.tile([RB, W], bf16)
        nv.tensor_tensor(out=T1X[:], in0=T1K[:], in1=NK2[:], op=ALU.add)

        # ---- M chunks + PE bilinear downsample (exact: weights are k/32) ----
        M = pool.tile([RB, CP * W], bf16)
        M_cw = M[:].rearrange("p (c w) -> p c w", w=W)
        Yps = [psum.tile([RB, n * 4 * 28], f32, name=f"yps{g}")
               for g, (c0, n) in enumerate(grps)]
        Ysb = [pool.tile([RB, n * 4 * 28], f16, name=f"ysb{g}")
               for g, (c0, n) in enumerate(grps)]
        Aps = psum.tile([28, CP * 28], f32)
        # stage 1 groups back-to-back on PE; copies trail on Act/DVE; then
        # stage 2 groups (so PE never waits a copy mid-stream)
        for g, (c0, n) in enumerate(grps):
            T1X_b = T1X[:].unsqueeze(1).broadcast_to([RB, n, W])
            nv.tensor_tensor(out=M_cw[:, c0:c0 + n, :],
                             in0=V_cw[:, c0:c0 + n, :], in1=T1X_b,
                             op=ALU.is_ge)
            for cr in range(n):
                c = c0 + cr
                for u in range(4):
                    nc.tensor.matmul(
                        Yps[g][:, (cr * 4 + u) * 28:(cr * 4 + u + 1) * 28],
                        lhsT=M[:, c * W + u * RB:c * W + (u + 1) * RB],
                        rhs=WR[:], start=True, stop=True)
            if g % 2 == 0:
                ns.copy(Ysb[g][:], Yps[g][:])
            else:
                nv.tensor_copy(Ysb[g][:], Yps[g][:])
        for g, (c0, n) in enumerate(grps):
            for cr in range(n):
                c = c0 + cr
                for u in range(4):
                    nc.tensor.matmul(
                        Aps[:, c * 28:(c + 1) * 28],
                        lhsT=Ysb[g][:, (cr * 4 + u) * 28:(cr * 4 + u + 1) * 28],
                        rhs=WC[:, u * 28:(u + 1) * 28],
                        start=(u == 0), stop=(u == 3))
        Asb = pool.tile([28, CP * 28], f32)
        ns.copy(Asb[:], Aps[:])
        nc.sync.dma_start(o_a, Asb[:])


    nc.compile()
    return nc


def _build_b_fast():
    nc = bacc.Bacc("TRN2", target_bir_lowering=False, debug=False, num_devices=1)
    P = B * C  # 40

    # aint layout: [pix%112, k*280 + u*40 + pair]  (A^T partials, block-major)
    aint = nc.dram_tensor("aint", [112, NBLK * 7 * P], f32,
                          kind="ExternalInput").ap()
    fmi = nc.dram_tensor("fmi", [112, 7 * B * D], bf16, kind="ExternalInput").ap()
    smt = nc.dram_tensor("smt", [128, 226], f32, kind="ExternalInput").ap()

    o_loss = nc.dram_tensor("o_loss", [1, 1], f32, kind="ExternalOutput").ap()
    o_cnt = nc.dram_tensor("o_cnt", [1, P], f32, kind="ExternalOutput").ap()

    L5 = float(np.log(1e-5))
    L1M = float(np.log1p(-1e-5))
    LNLO = float(np.log(1e-5))
    LNHI = float(np.log1p(-1e-5))

    with tile.TileContext(nc) as tc, ExitStack() as ctx:
        pool = ctx.enter_context(tc.tile_pool(name="p", bufs=1))
        psum = ctx.enter_context(tc.tile_pool(name="ps", bufs=1, space="PSUM"))
        nv = nc.vector
        ns = nc.scalar

        AIN = pool.tile([112, NBLK * 7 * P], f32)
        nc.sync.dma_start(AIN[:], aint)
        SM = pool.tile([128, 226], f32)
        nc.sync.dma_start(SM[:], smt)
        FM = pool.tile([112, 7 * B * D], bf16)
        for fc in range(4):
            c0, c1 = fc * 1024, min((fc + 1) * 1024, 7 * B * D)
            nc.sync.dma_start(FM[:, c0:c1], fmi[:, c0:c1])
        PJT = SM[:, 0:40]            # [128, (dc,c2)] proj^T chunks
        ONES112 = SM[0:112, 40:41]
        EYE = SM[0:C, 41:61]
        LAB2 = SM[0:C, 61:63]
        BSEL = SM[0:P, 63:65]
        EYEBC = SM[0:P, 65:85]
        ONES20 = SM[0:C, 85:86]
        SH0 = SM[0:C, 86:126]
        SH1 = SM[0:C, 126:166]
        ONESM = SM[0:C, 166:186]
        ONES1R = SM[0:1, 186:226]    # [1, 40] ones

        # ---- early independent: n_b, 1/max(n_b,1), step-0 ccf constant ----
        NSps = psum.tile([1, 2], f32, name="nsps")
        nc.tensor.matmul(NSps[:], lhsT=ONES20, rhs=LAB2, start=True, stop=True)
        NS = pool.tile([1, 2], f32)
        nv.tensor_copy(NS[:], NSps[:])
        DN = pool.tile([1, 2], f32)
        nv.tensor_scalar(out=DN[:], in0=NS[:], scalar1=1.0, scalar2=None,
                         op0=ALU.max)
        RDN = pool.tile([1, 2], f32)
        nv.reciprocal(RDN[:], DN[:])
        CCF = pool.tile([1, 1], f32)
        nv.tensor_scalar(out=CCF[:], in0=NS[:, 0:1],
                         scalar1=-(L5 - L1M) / (C * C), scalar2=-L1M,
                         op0=ALU.mult, op1=ALU.add)

        # ---- raw coef^T = sum over 4 row-blocks (tree); scale LR rides later --
        H1 = pool.tile([112, 2 * 7 * P], f32)
        nv.tensor_tensor(out=H1[:], in0=AIN[:, :2 * 7 * P],
                         in1=AIN[:, 2 * 7 * P:], op=ALU.add)
        CTR = pool.tile([112, 7 * P], bf16)
        nv.tensor_tensor(out=CTR[:], in0=H1[:, :7 * P], in1=H1[:, 7 * P:],
                         op=ALU.add)
        ONE112B = pool.tile([112, 1], bf16)
        nv.memset(ONE112B[:], 1.0)

        # ---- counts -> LR = label/max(cnt,1) as a [40,1] column via PE ----
        CNTps = psum.tile([1, 7 * P], f32, name="cntps")
        nc.tensor.matmul(CNTps[:], lhsT=ONE112B[:], rhs=CTR[:], start=True,
                         stop=True)
        CNTR = pool.tile([1, 7 * P], f32)
        nv.tensor_copy(CNTR[:], CNTps[:])
        CNT = _emit_tree(nc, pool, CNTR, 7, P, ALU.add, "cnt", dt=f32, part=1)
        nc.sync.dma_start(o_cnt, CNT[:])
        DENR = pool.tile([1, P], f32)
        nv.tensor_scalar(out=DENR[:], in0=CNT[:], scalar1=1.0, scalar2=None,
                         op0=ALU.max)
        RDR = pool.tile([1, P], f32)
        nv.reciprocal(RDR[:], DENR[:])
        LRps = psum.tile([P, P], f32, name="lrps")
        nc.tensor.matmul(LRps[:], lhsT=RDR[:], rhs=ONES1R, start=True, stop=True)
        LR40 = pool.tile([P, 1], f32)
        nv.tensor_copy(LR40[:], LRps[:, 0:1])

        # ---- fsm^T (raw scale) = fmap_ds^T @ coef_raw^T ----
        FTps = [psum.tile([128, P], f32, name=f"ftps{dc}", tag="ftps", bufs=2)
                for dc in range(2)]
        for dc in range(2):
            for b2 in range(B):
                for u in range(7):
                    nc.tensor.matmul(
                        FTps[dc][:, b2 * C:(b2 + 1) * C],
                        lhsT=FM[:, u * (B * D) + b2 * D + dc * 128:
                                u * (B * D) + b2 * D + (dc + 1) * 128],
                        rhs=CTR[:, u * P + b2 * C:u * P + (b2 + 1) * C],
                        start=(u == 0), stop=(u == 6))
        FSMT = pool.tile([128, 2 * P], f32)   # [d, (dc, b, c)]
        nv.tensor_copy(FSMT[:, 0:P], FTps[0][:])
        nv.tensor_copy(FSMT[:, P:2 * P], FTps[1][:])

        # ---- Gram diagonals (raw norms) + raw cos dot + logits ----
        SMLps = psum.tile([C, 4 * C], f32, name="smlps")
        for b2 in range(B):
            for dc in range(2):
                nc.tensor.matmul(
                    SMLps[:, b2 * C:(b2 + 1) * C],
                    lhsT=FSMT[:, dc * P + b2 * C:dc * P + (b2 + 1) * C],
                    rhs=FSMT[:, dc * P + b2 * C:dc * P + (b2 + 1) * C],
                    start=(dc == 0), stop=(dc == 1))
        RAWps = SMLps[:, 2 * C:3 * C]
        for dc in range(2):
            nc.tensor.matmul(RAWps,
                             lhsT=FSMT[:, dc * P + C:(dc + 1) * P],
                             rhs=FSMT[:, dc * P:dc * P + C],
                             start=(dc == 0), stop=(dc == 1))
        MMps = psum.tile([P, C], f32, name="mmps")
        LOGps = MMps[:, 0:C]
        for dc in range(2):
            nc.tensor.matmul(LOGps, lhsT=FSMT[:, dc * P:(dc + 1) * P],
                             rhs=PJT[:, dc * C:(dc + 1) * C],
                             start=(dc == 0), stop=(dc == 1))

        # LIN = [ |dot| , guarded nrm2 ] -> single Ln keeps one table phase
        LIN = pool.tile([C, C + 2], f32)
        SCRD = pool.tile([C, C], f32)
        for b2 in range(B):
            nv.tensor_tensor(out=SCRD[:], in0=SMLps[:, b2 * C:(b2 + 1) * C],
                             in1=EYE, op=ALU.mult)
            nv.tensor_reduce(out=LIN[:, C + b2:C + b2 + 1], in_=SCRD[:],
                             axis=AX.X, op=ALU.add)
        nv.tensor_scalar(out=LIN[:, C:C + 2], in0=LIN[:, C:C + 2], scalar1=1e-24,
                         scalar2=None, op0=ALU.max)
        hp = ExitStack()
        hp.enter_context(tc.high_priority())
        nv.tensor_scalar(out=LIN[:, 0:C], in0=RAWps, scalar1=-1.0, scalar2=None,
                         op0=ALU.mult)
        nv.tensor_tensor(out=LIN[:, 0:C], in0=LIN[:, 0:C], in1=RAWps,
                         op=ALU.max)
        nv.tensor_scalar(out=LIN[:, 0:C], in0=LIN[:, 0:C], scalar1=1e-30,
                         scalar2=None, op0=ALU.max)
        LOUT = pool.tile([C, C + 2], f32)
        ns.activation(LOUT[:], LIN[:], AFT.Ln)
        LNC = pool.tile([C, C], f32)
        LN2 = LOUT[:, C:C + 2]
        # row term: -0.5*ln n1_c ; column term via PE: -0.5*ln n0_j - 50*(1-p0_j)
        COLV = pool.tile([C, 1], f32)
        nv.tensor_scalar(out=COLV[:], in0=LAB2[:, 0:1], scalar1=50.0,
                         scalar2=-50.0, op0=ALU.mult, op1=ALU.add)
        nv.scalar_tensor_tensor(out=COLV[:], in0=LOUT[:, C:C + 1], scalar=-0.5,
                                in1=COLV[:], op0=ALU.mult, op1=ALU.add)
        DIAGC = pool.tile([C, C], f32)
        nv.tensor_scalar(out=DIAGC[:], in0=EYE, scalar1=COLV[:], scalar2=None,
                         op0=ALU.mult)
        CSMps = SMLps[:, 3 * C:4 * C]
        nc.tensor.matmul(CSMps, lhsT=ONESM, rhs=DIAGC[:], start=True, stop=True)
        RV = pool.tile([C, 1], f32)
        nv.tensor_scalar(out=RV[:], in0=LOUT[:, C + 1:C + 2], scalar1=-0.5,
                         scalar2=None, op0=ALU.mult)
        nv.tensor_scalar(out=LNC[:], in0=LOUT[:, 0:C], scalar1=RV[:],
                         scalar2=None, op0=ALU.add)
        nv.tensor_tensor(out=LNC[:], in0=LNC[:], in1=CSMps, op=ALU.add)
        nv.tensor_scalar(out=LNC[:], in0=LNC[:], scalar1=LNLO, scalar2=LNHI,
                         op0=ALU.max, op1=ALU.min)
        hp.close()

        # ---- softmax-BCE; one fused Exp [XT | LNC], one fused Ln [SME|SMR|OM]
        XB = pool.tile([P, 2 * C], f32)
        nv.memset(XB[:], 0.0)
        nv.tensor_scalar(out=XB[:, C:2 * C][0:C, :], in0=LNC[:], scalar1=1.0,
                         scalar2=None, op0=ALU.mult)
        LOG = pool.tile([P, C], f32)
        nv.tensor_scalar(out=LOG[:], in0=LOGps, scalar1=LR40[:], scalar2=None,
                         op0=ALU.mult)
        MX = pool.tile([P, 1], f32)
        nv.tensor_reduce(out=MX[:], in_=LOG[:], axis=AX.X, op=ALU.max)
        XT = XB[:, 0:C]
        nv.tensor_scalar(out=XT, in0=LOG[:], scalar1=MX[:], scalar2=None,
                         op0=ALU.subtract)
        EB = pool.tile([P, 2 * C], f32)
        ns.activation(EB[:], XB[:], AFT.Exp)
        ET = EB[:, 0:C]
        COSC = pool.tile([C, C], f32)
        nv.tensor_copy(COSC[:], EB[0:C, C:2 * C])
        SMR = pool.tile([P, 1], f32)
        nv.tensor_reduce(out=SMR[:], in_=ET, axis=AX.X, op=ALU.add)
        # LNIN = [SME | SMR | OM(padded with 1s)] -> single Ln
        LNIN = pool.tile([P, 2 * C + 1], f32)
        nv.memset(LNIN[:], 1.0)
        nv.scalar_tensor_tensor(out=LNIN[:, 0:C], in0=ET, scalar=-1.0,
                                in1=SMR[:].broadcast_to([P, C]),
                                op0=ALU.mult, op1=ALU.add)
        nv.tensor_copy(LNIN[:, C:C + 1], SMR[:])
        nv.tensor_scalar(out=LNIN[:, C + 1:2 * C + 1][0:C, :], in0=COSC[:],
                         scalar1=-1.0, scalar2=1.0, op0=ALU.mult, op1=ALU.add)
        LNO = pool.tile([P, 2 * C + 1], f32)
        ns.activation(LNO[:], LNIN[:], AFT.Ln)
        LGS = LNO[:, C:C + 1]
        LGP = pool.tile([P, C], f32)
        nv.tensor_scalar(out=LGP[:], in0=XT, scalar1=LGS, scalar2=-100.0,
                         op0=ALU.subtract, op1=ALU.max)
        L1P = pool.tile([P, C], f32)
        nv.tensor_scalar(out=L1P[:], in0=LNO[:, 0:C], scalar1=LGS, scalar2=-100.0,
                         op0=ALU.subtract, op1=ALU.max)
        DD = pool.tile([P, C], f32)
        nv.tensor_tensor(out=DD[:], in0=LGP[:], in1=L1P[:], op=ALU.subtract)
        SCRP = pool.tile([P, C], f32)
        nv.tensor_tensor(out=SCRP[:], in0=DD[:], in1=EYEBC, op=ALU.mult)
        DDG = pool.tile([P, 1], f32)
        nv.tensor_reduce(out=DDG[:], in_=SCRP[:], axis=AX.X, op=ALU.add)
        RSM = pool.tile([P, 1], f32)
        nv.tensor_reduce(out=RSM[:], in_=L1P[:], axis=AX.X, op=ALU.add)
        TERM = pool.tile([P, 1], f32)
        nv.tensor_tensor(out=TERM[:], in0=DDG[:], in1=RSM[:], op=ALU.add)
        nv.tensor_scalar(out=TERM[:], in0=TERM[:], scalar1=-1.0 / C, scalar2=None,
                         op0=ALU.mult)

        # ---- qualify: b0 = present_0; b1 = present_1 & offdiag max < 0.6 ----
        COSM = pool.tile([C, C], f32)
        nv.scalar_tensor_tensor(out=COSM[:], in0=EYE, scalar=-1e9, in1=COSC[:],
                                op0=ALU.mult, op1=ALU.add)
        OFF = pool.tile([C, 1], f32)
        nv.tensor_reduce(out=OFF[:], in_=COSM[:], axis=AX.X, op=ALU.max)
        QB1 = pool.tile([C, 1], f32)
        nv.tensor_scalar(out=QB1[:], in0=OFF[:], scalar1=0.6, scalar2=None,
                         op0=ALU.is_lt)
        nv.tensor_tensor(out=QB1[:], in0=QB1[:], in1=LAB2[:, 1:2], op=ALU.mult)
        QRps = psum.tile([P, 6], f32, name="qrps")
        Q40ps = QRps[:, 0:1]
        nc.tensor.matmul(Q40ps, lhsT=SH0, rhs=LAB2[:, 0:1], start=True,
                         stop=False)
        nc.tensor.matmul(Q40ps, lhsT=SH1, rhs=QB1[:], start=False, stop=True)
        CONTR = pool.tile([P, 1], f32)
        nv.tensor_tensor(out=CONTR[:], in0=TERM[:], in1=Q40ps, op=ALU.mult)

        # ---- ccf step 1 (LGC == clipped LNC; LOM from the fused Ln) ----
        LOM = LNO[0:C, C + 1:2 * C + 1]
        R1 = pool.tile([C, 1], f32)
        nv.tensor_reduce(out=R1[:], in_=LOM, axis=AX.X, op=ALU.add)
        DIF = pool.tile([C, C], f32)
        nv.tensor_tensor(out=DIF[:], in0=LNC[:], in1=LOM, op=ALU.subtract)
        nv.tensor_scalar(out=DIF[:], in0=DIF[:], scalar1=LAB2[:, 1:2],
                         scalar2=None, op0=ALU.mult)
        CC1 = pool.tile([C, 1], f32)
        nv.tensor_tensor(out=SCRD[:], in0=DIF[:], in1=EYE, op=ALU.mult)
        nv.tensor_reduce(out=CC1[:], in_=SCRD[:], axis=AX.X, op=ALU.add)
        nv.tensor_tensor(out=CC1[:], in0=CC1[:], in1=R1[:], op=ALU.add)

        # ---- partition sums via PE, final scalar chain ----
        REDps = QRps[0:1, 1:6]
        nc.tensor.matmul(REDps[:, 2:4], lhsT=CONTR[:], rhs=BSEL, start=True,
                         stop=True)
        nc.tensor.matmul(REDps[:, 4:5], lhsT=ONES20, rhs=CC1[:], start=True,
                         stop=True)
        RED = pool.tile([1, 5], f32)
        nv.tensor_copy(RED[:, 2:5], REDps[:, 2:5])
        LC = pool.tile([1, 1], f32)
        nv.tensor_tensor(out=LC[:], in0=RED[:, 2:3], in1=RDN[:, 0:1], op=ALU.mult)
        nv.tensor_tensor(out=LC[:], in0=LC[:], in1=RED[:, 3:4], op=ALU.add)
        nv.tensor_tensor(out=LC[:], in0=LC[:], in1=RDN[:, 1:2], op=ALU.mult)
        CC1S = pool.tile([1, 1], f32)
        nv.tensor_scalar(out=CC1S[:], in0=RED[:, 4:5], scalar1=-1.0 / (C * C),
                         scalar2=None, op0=ALU.mult)
        OUT = pool.tile([1, 1], f32)
        nv.tensor_tensor(out=OUT[:], in0=LC[:], in1=CCF[:], op=ALU.add)
        nv.tensor_tensor(out=OUT[:], in0=OUT[:], in1=CC1S[:], op=ALU.add)
        nc.sync.dma_start(o_loss, OUT[:])

    nc.compile()
    return nc


def _marshal_a_fast(cam, CP, idxs):
    bf = mybir.dt.np(bf16)
    fh = mybir.dt.np(f16)
    in_maps = []
    for core in range(8):
        b, blk = core // NBLK, core % NBLK
        idx = idxs[b]
        camv = np.zeros((CP, NPIX), bf)
        if len(idx):
            camv[:len(idx)] = cam[b, idx, blk * RB:(blk + 1) * RB, :].reshape(
                len(idx), NPIX).astype(bf)
        wct = np.ascontiguousarray(
            W1D.reshape(4, RB, 28).transpose(1, 0, 2).reshape(RB, 4 * 28))
        in_maps.append({
            "camv": camv,
            "wrt": np.ascontiguousarray(W1D[blk * RB:(blk + 1) * RB, :]).astype(bf),
            "wct": wct.astype(fh),
        })
    return in_maps


def _marshal_b_fast(res_a, fmap, cls_label, proj_weight, CP, idxs):
    P = B * C
    a8 = np.stack([res_a[k]["o_a"] for k in range(8)])          # [8, 28, CP*28]
    a8 = a8.reshape(B, NBLK, 28, CP, 28)
    afull = np.zeros((P, NBLK, 784), np.float32)
    for b in range(B):
        idx = idxs[b]
        if len(idx):
            # [blk, 28i, slot, 28j] -> [slot, blk, (i,j)]
            afull[b * C + idx] = a8[b, :, :, :len(idx), :].transpose(
                2, 0, 1, 3).reshape(len(idx), NBLK, 784)
    # aint[p, k*280 + u*40 + pair] = afull[pair, k, u*112 + p]
    aint = np.ascontiguousarray(
        afull.reshape(P, NBLK, 7, 112).transpose(3, 1, 2, 0)
    ).reshape(112, NBLK * 7 * P)

    fm = np.asarray(fmap, np.float32).reshape(B, D, 784)
    # fmi[p, u*512 + b*256 + d] = fmap[b, d, u*112 + p]
    fmi = np.ascontiguousarray(
        fm.transpose(2, 0, 1).reshape(7, 112, B, D).transpose(1, 0, 2, 3)
    ).reshape(112, 7 * B * D).astype(mybir.dt.np(bf16))

    lab = np.asarray(cls_label, np.float32)
    smt = np.zeros((128, 226), np.float32)
    smt[:, 0:40] = np.ascontiguousarray(
        np.asarray(proj_weight, np.float32).T
    ).reshape(2, 128, C).transpose(1, 0, 2).reshape(128, 2 * C)
    smt[0:112, 40:41] = 1.0
    smt[0:C, 41:61] = np.eye(C, dtype=np.float32)
    smt[0:C, 61:63] = lab.T
    smt[0:P, 63:65] = (np.arange(P)[:, None] // C ==
                       np.arange(2)[None, :]).astype(np.float32)
    smt[0:P, 65:85] = np.tile(np.eye(C, dtype=np.float32), (B, 1))
    smt[0:C, 85:86] = 1.0
    smt[0:C, 86:126] = np.eye(C, P, dtype=np.float32)
    smt[0:C, 126:166] = np.eye(C, P, k=C, dtype=np.float32)
    smt[0:C, 166:186] = 1.0
    smt[0:1, 186:226] = 1.0
    return {"aint": aint, "fmi": fmi, "smt": smt}


# --------------------------------------------------------------------------
# Host marshaling + driver
# --------------------------------------------------------------------------

_CACHE = {}


def _get_programs(hig, low, bg, CP):
    stage = int(os.environ.get("BASSK_B_STAGE", "99"))
    key = ("slow", float(hig), float(low), float(bg), stage, CP)
    if key not in _CACHE:
        _CACHE[key] = (_build_a(hig, low, bg, CP), _build_b(stage))
    return _CACHE[key]


def _get_programs_fast(hig, low, bg, CP):
    key = (float(hig), float(low), float(bg), CP)
    if key not in _CACHE:
        _CACHE[key] = (_build_a_fast(hig, low, bg, CP), _build_b_fast())
    return _CACHE[key]


def _marshal_a(cam, cls_label, CP, idxs):
    eye128 = np.eye(128, dtype=np.float32)
    clst = np.tile((np.arange(CP, dtype=np.float32) + 1.0)[None, :], (RB, 1))
    iodt = np.tile((float(CP) - np.arange(CP, dtype=np.float32))[None, :], (RB, 1))
    wct = np.ascontiguousarray(
        W1D.reshape(4, RB, 28).transpose(1, 0, 2).reshape(RB, 4 * 28))
    in_maps = []
    for core in range(8):
        b, blk = core // NBLK, core % NBLK
        idx = idxs[b]
        camv = np.zeros((CP, NPIX), np.float32)
        if len(idx):
            camv[:len(idx)] = cam[b, idx, blk * RB:(blk + 1) * RB, :].reshape(
                len(idx), NPIX)
        labt = np.tile((np.arange(CP) < len(idx)).astype(np.float32)[None, :],
                       (RB, 1))
        in_maps.append({
            "camv": camv,
            "labt": labt,
            "clst": clst,
            "iodt": iodt,
            "wrt": np.ascontiguousarray(W1D[blk * RB:(blk + 1) * RB, :]),
            "wct": wct,
            "idn": eye128,
        })
    return in_maps


def _marshal_b(res_a, fmap, cls_label, proj_weight, feature_contrast, CP, idxs):
    P = B * C
    ntk = (CP + 7) // 8
    # scatter packed per-slot A partials back to global classes
    a8 = np.stack([res_a[k]["o_a"] for k in range(8)])          # [8, 28, CP*28]
    a8 = a8.reshape(B, NBLK, 28, CP, 28)
    afull = np.zeros((B, C, 28, 28, NBLK), np.float32)
    for b in range(B):
        idx = idxs[b]
        if len(idx):
            # [blk, 28, slot, 28] -> [slot, 28, 28, blk]
            afull[b, idx] = a8[b, :, :, :len(idx), :].transpose(2, 1, 3, 0)
    ain = np.ascontiguousarray(afull).reshape(P, 784 * NBLK)

    cand_v = np.zeros((P, NBLK * NCAND), np.float32)
    cand_i = np.zeros((P, NBLK * NCAND), np.uint32)
    for core in range(8):
        b, blk = core // NBLK, core % NBLK
        tks = [res_a[core][f"o_tk{t}"] for t in range(ntk)]
        for j, c in enumerate(idxs[b]):
            tk = tks[j // 8]
            rb = (j % 8) * 16
            vals = np.concatenate([tk[rb + 14, 0:16], tk[rb + 15, 0:16]])
            gidx = np.concatenate([tk[rb + 14, 16:32], tk[rb + 15, 16:32]])
            cand_v[b * C + c, blk * NCAND:(blk + 1) * NCAND] = vals.view(np.float32)
            cand_i[b * C + c, blk * NCAND:(blk + 1) * NCAND] = gidx

    bbs = np.zeros((P, NBLK * NCAND), np.float32)
    for blk in range(NBLK):
        bbs[:, blk * NCAND:(blk + 1) * NCAND] = blk * RB * W

    # pre-transposed fmap: fmt[sp, u*(B*D) + b*D + d] = fmap[b, d, u*112+sp]
    fm = np.asarray(fmap, np.float32).reshape(B, D, 7, 112)
    fmi = np.ascontiguousarray(fm.transpose(3, 2, 0, 1)).reshape(112, 7 * B * D)

    rnk = np.zeros((P, NCAND), np.float32)
    rnk[:, :K_TOP] = 1.0 / K_TOP

    return {
        "ain": ain,
        "cdv": cand_v,
        "cdi": cand_i,
        "bbs": bbs,
        "fmi": fmi,
        "prj": np.ascontiguousarray(
            np.asarray(proj_weight, np.float32).T.reshape(2, 128, C)
            .transpose(1, 0, 2)).reshape(128, 2 * C),
        "lab": np.asarray(cls_label, np.float32).reshape(P, 1),
        "lab2": np.ascontiguousarray(np.asarray(cls_label, np.float32).T),
        "fc0": np.asarray(feature_contrast, np.float32),
        "eye": np.eye(C, dtype=np.float32),
        "i28": np.tile(np.arange(28, dtype=np.float32)[None, :], (128, 1)),
        "i128": np.tile(np.arange(128, dtype=np.float32)[None, :], (P, 1)),
        "mmb": (np.arange(128)[:, None] // NCAND ==
                np.arange(76)[None, :] - 36).astype(np.float32),
        "rnk": rnk,
        "idn": np.eye(128, dtype=np.float32),
    }


LAST_EXEC_NS = {}


def _run(nc, in_maps, core_ids, tag="k"):
    if os.environ.get("BASSK_SIM") == "1":
        from concourse.bass_interp import CoreSim, MultiCoreSim
        if len(core_ids) == 1:
            sim = CoreSim(nc, trace=False, require_finite=False)
            sims = [sim]
        else:
            msim = MultiCoreSim(nc, num_cores=len(core_ids), trace=False,
                                require_finite=False)
            sims = [msim.cores[i] for i in core_ids]
            sim = msim
        for s, m in zip(sims, in_maps):
            for name, arr in m.items():
                s.tensor(name)[:] = arr
        sim.simulate(check_with_hw=False)
        outs = []
        for s in sims:
            d = {}
            for alloc in nc.m.functions[0].allocations:
                if getattr(alloc, "kind", None) == "ExternalOutput":
                    nm = alloc.memorylocations[0].name
                    d[nm] = np.array(s.tensor(nm))
            outs.append(d)
        return outs
    trace = os.environ.get("BASSK_TRACE") == "1"
    if trace:
        try:
            from antenv.axon_hooks import get_axon_ntff_profile_hook  # noqa: F401
        except Exception:
            trace = False
    res = run_bass_kernel_spmd(nc, in_maps, core_ids, trace=trace)
    if res.exec_time_ns is not None:
        LAST_EXEC_NS[tag] = res.exec_time_ns
    return res.results


def _kernel_slow(fmap, cam, cls_label, proj_weight, feature_contrast,
                 hig_thre, low_thre, bg_thre, idxs, CP):
    nca, ncb = _get_programs(float(hig_thre), float(low_thre), float(bg_thre), CP)
    res_a = _run(nca, _marshal_a(cam, cls_label, CP, idxs), list(range(8)), tag="A")
    in_b = _marshal_b(res_a, fmap, cls_label, proj_weight, feature_contrast, CP, idxs)
    res_b = _run(ncb, [in_b], [0], tag="B")
    return np.float32(res_b[0]["o_loss"].reshape(-1)[0])


def kernel(fmap, cam, cls_label, proj_weight, feature_contrast,
           hig_thre, low_thre, bg_thre):
    fmap = np.asarray(fmap, np.float32)
    cam = np.asarray(cam, np.float32)
    lab = np.asarray(cls_label, np.float32)
    idxs = [np.where(lab[b] > 0.5)[0] for b in range(B)]
    cp_act = max((len(i) for i in idxs), default=0)
    CP = min(C, max(4, ((cp_act + 3) // 4) * 4))

    fc_zero = not np.any(np.asarray(feature_contrast, np.float32))
    if fc_zero and os.environ.get("BASSK_FORCE_SLOW") != "1":
        CPF = max(1, cp_act)
        nca, ncb = _get_programs_fast(float(hig_thre), float(low_thre),
                                      float(bg_thre), CPF)
        res_a = _run(nca, _marshal_a_fast(cam, CPF, idxs), list(range(8)), tag="A")
        in_b = _marshal_b_fast(res_a, fmap, cls_label, proj_weight, CPF, idxs)
        res_b = _run(ncb, [in_b], [0], tag="B")
        cnt = res_b[0]["o_cnt"].reshape(B, C)
        # fast path assumed every present class has masked pixels; verify.
        if not np.any((lab > 0.5) & (cnt < 0.5)):
            loss = np.float32(res_b[0]["o_loss"].reshape(-1)[0])
            return np.asarray(loss, dtype=np.float32).reshape(())
    loss = _kernel_slow(fmap, cam, cls_label, proj_weight, feature_contrast,
                        hig_thre, low_thre, bg_thre, idxs, CP)
    return np.asarray(loss, dtype=np.float32).reshape(())

